# revision 5
# baseline (speedup 1.0000x reference)
"""Trainium2 Bass kernel for the DCF (dynamic conv filter) module.

Sharding: pure data-parallel over batch N=8 across 8 NeuronCores (one image
per core); all parameters replicated.

Pipeline per core (one 128x96x96 image):
  A:  conv1 (3x3, 192->128) + tanh -> hmid;  conv2 (1x1, 128->36) + tanh -> b
  A3: transpose b columns into per-pixel scalar table scT
  B:  per 126-pixel tile t:
        - F_k = fixed-basis convs of feat via banded matmuls on host-prepped
          row-shifted transposed feature chunks (fTd), PSUM-accumulated
        - acc_m^T = sum_k F_k^T @ diag(s_{m,k})  -- the per-pixel scale and
          k-reduction run on the PE array via diagonal moving operands;
          result lands PSUM-accumulated and already channel-major
        - out_tile = sum_m coef_m @ acc_m^T (+bias), stored fp16

Diagonals are built as tensor_scalar(identity * s) which hits the DVE 4x
perf mode; builds are spread across DVE/Pool/Act to balance engine load."""

from itertools import product

import numpy as np

import concourse.bass as bass
import concourse.tile as tile
from concourse import bacc, mybir
from concourse.bass_utils import run_bass_kernel_spmd
from concourse.masks import make_identity

fp16 = mybir.dt.float16
fp32 = mybir.dt.float32
fp8 = mybir.dt.float8e4
W1SCALE = 32.0  # conv1 weights pre-scaled into fp8's normal range

N_CORES = 8
C = 128
CW = 64
H = W = 96
HP = WP = 98
NPIX = H * W
NPAD = HP * WP  # 9604
NB = 6
TEM = 6
L = 9
NBT = NB * TEM  # 36
RT = 4
FT = RT * W  # 384
NT = H // RT  # 24
TP = 126          # output pixels per flat tile
NTF = 77          # flat tiles (covers padded idx 1 .. 1+77*126 = 9703)
BP = 9732         # padded bsb/out length
FEXT = 10000      # extended (host-side) padded feat length for fTd windows
FOFF = 98         # fTd window base offset inside the extended buffer
SGRP = 4          # output tiles per store

# diag-build engine assignment for the 30 non-m5 builds: 20 DVE (cheapest:
# 102ns in 4x mode), 6 Pool, 4 Act, interleaved so consecutive builds spread
_seq = ["D", "P", "D", "D", "A", "D", "D", "P", "D", "D", "A", "D", "D", "P",
       "D"] * 2
_ENG_PAT = [None] * 36
_idx = 0
for _j in range(36):
    if _j % 6 != 5:
        _ENG_PAT[_j] = _seq[_idx]
        _idx += 1
    else:
        _ENG_PAT[_j] = "D"  # unused (m=5 handled by the ts route)

_CACHE = {}


def build_nc():
    nc = bacc.Bacc("TRN2", target_bir_lowering=False, debug=False)

    featp = nc.dram_tensor("featp", [C, NPAD], fp16, kind="ExternalInput").ap()
    wgtq = nc.dram_tensor("wgtq", [C, NPAD], fp16, kind="ExternalInput").ap()
    fTd = nc.dram_tensor("fTd", [C, 3 * NTF * C], fp16, kind="ExternalInput").ap()
    wgtq2 = nc.dram_tensor("wgtq2", [C, NPAD], fp16, kind="ExternalInput").ap()
    w1f = nc.dram_tensor("w1f", [C, L * C], fp16, kind="ExternalInput").ap()
    # conv1 weight-branch params: w1wq|w1wq2 = 384+128
    pkw = nc.dram_tensor("pkw", [C, 512], fp16, kind="ExternalInput").ap()
    # fp16 params packed: w2|coefT|bndf = 36+768+2304
    pk = nc.dram_tensor("pk", [C, 3108], fp16, kind="ExternalInput").ap()
    w1w = nc.dram_tensor("w1w", [CW, L * C], fp16, kind="ExternalInput").ap()
    pb = nc.dram_tensor("pb", [C, 3], fp32, kind="ExternalInput").ap()
    out = nc.dram_tensor("out", [C, BP], fp16, kind="ExternalOutput").ap()

    Tanh = mybir.ActivationFunctionType.Tanh
    Ident = mybir.ActivationFunctionType.Identity
    Copy = mybir.ActivationFunctionType.Copy
    MUL = mybir.AluOpType.mult
    ADD = mybir.AluOpType.add

    with tile.TileContext(nc) as tc:
        with (
            tc.tile_pool(name="const", bufs=1) as const,
            tc.tile_pool(name="big", bufs=1) as big,
        ):
            featp_sb = big.tile([C, NPAD], fp16)
            wgtp_sb = big.tile([C, NPAD], fp16)
            fTd_sb = big.tile([C, 3 * NTF * C], fp16)
            cuts = [0, 2404, 4808, 7212, NPAD]
            w1f_sb = const.tile([C, L * C], fp16)
            nc.sync.dma_start(w1f_sb[:], w1f)
            nc.sync.dma_start(featp_sb[:, : cuts[1]], featp[:, : cuts[1]])
            pkw_sb = const.tile([C, 512], fp16)
            nc.sync.dma_start(pkw_sb[:], pkw)
            w1wq_sb = pkw_sb[:, 0:384]
            w1wq2_sb = pkw_sb[:, 384:512]
            pb_sb = const.tile([C, 3], fp32)
            nc.sync.dma_start(pb_sb[:], pb)
            b1_sb = pb_sb[:, 0:1]
            b3_sb = pb_sb[:, 1:2]
            b2_sb = pb_sb[:NBT, 2:3]
            w1w_sb = const.tile([CW, L * C], fp16)
            nc.sync.dma_start(w1w_sb[:], w1w)
            nc.sync.dma_start(wgtp_sb[:, : cuts[2]], wgtq[:, : cuts[2]])
            wgtq2_sb = big.tile([C, NPAD], fp16)
            nc.sync.dma_start(wgtq2_sb[:, : cuts[1]], wgtq2[:, : cuts[1]])
            pk_sb = const.tile([C, 3108], fp16)
            nc.sync.dma_start(pk_sb[:], pk)
            w2_sb = pk_sb[:, 0:36]
            coefT_sb = pk_sb[:, 36:804]
            bndf_sb = pk_sb[:, 804:3108]
            # fTd is t-major: stream it in 11-tile chunks interleaved with the
            # remaining image chunks so F(0) can start ~10us in
            FCH = 11 * 3 * C
            nc.sync.dma_start(fTd_sb[:, :FCH], fTd[:, :FCH])
            nc.sync.dma_start(
                featp_sb[:, cuts[1] : cuts[2]], featp[:, cuts[1] : cuts[2]]
            )
            nc.sync.dma_start(fTd_sb[:, FCH : 2 * FCH], fTd[:, FCH : 2 * FCH])
            nc.sync.dma_start(
                featp_sb[:, cuts[2] : cuts[3]], featp[:, cuts[2] : cuts[3]]
            )
            nc.sync.dma_start(wgtp_sb[:, cuts[2] :], wgtq[:, cuts[2] :])
            nc.sync.dma_start(wgtq2_sb[:, cuts[1] :], wgtq2[:, cuts[1] :])
            nc.sync.dma_start(
                featp_sb[:, cuts[3] :], featp[:, cuts[3] :]
            )
            for q in range(2, 7):
                nc.sync.dma_start(
                    fTd_sb[:, q * FCH : (q + 1) * FCH], fTd[:, q * FCH : (q + 1) * FCH]
                )
            identNBT = const.tile([NBT, NBT], fp16)
            make_identity(nc, identNBT[:])
            identTP = const.tile([TP, TP], fp16)
            make_identity(nc, identTP[:])

            bsb = big.tile([NBT, BP], fp16)
            # zero only the border/tail cells conv2 never writes (full memset
            # would hold Pool for 8us before the first b write)
            nc.gpsimd.memset(bsb[:, : WP + 2], 0.0)
            edge = bsb[:, 97 : 97 + 97 * WP].rearrange("c (r w) -> c r w", w=WP)
            nc.gpsimd.memset(edge[:, :, 0:2], 0.0)
            nc.gpsimd.memset(bsb[:, 97 * WP :], 0.0)
            scT = big.tile([TP, NTF * NBT], fp32)

            b3d = bsb[:, :NPAD].rearrange("c (r w) -> c r w", w=WP)
            f3 = featp_sb[:].rearrange("c (r w) -> c r w", w=WP)
            w3 = wgtp_sb[:].rearrange("c (r w) -> c r w", w=WP)
            wq2 = wgtq2_sb[:].rearrange("c (r w) -> c r w", w=WP)


            # ---- fused pipeline: conv rows (phase A) stream in between the
            # software-pipelined per-tile stages of phase B, so the PE never
            # drains between phases.
            with (
                tc.tile_pool(name="hmp", bufs=3) as hmp,
                tc.tile_pool(name="fbp", bufs=2) as fbp,
                tc.tile_pool(name="dgp", bufs=2) as dgp,
                tc.tile_pool(name="bop", bufs=2) as bop,
                tc.tile_pool(name="p5p", bufs=2) as p5p,
                tc.tile_pool(name="orp", bufs=2) as orp,
                tc.tile_pool(name="psB2", bufs=1, space="PSUM") as psB2,
            ):
                fbS_r, dg_r, boS_r, pso_r, acc_r = {}, {}, {}, {}, {}
                orow_bufs = {}

                def emit_arow_f(t):
                    r0 = t * RT
                    ps = psB2.tile([C, FT], fp32, tag="psA", bufs=1, name="ps")
                    for kk, (i, j) in enumerate(product(range(3), range(3))):
                        nc.tensor.matmul(
                            ps[:],
                            w1f_sb[:, (i * 3 + j) * C : (i * 3 + j + 1) * C],
                            f3[:, r0 + i : r0 + i + RT, j : j + W],
                            start=(kk == 0),
                            stop=False,
                        )
                    return ps

                def emit_arow_w(t, ps):
                    r0 = t * RT
                    for j in range(3):
                        nc.tensor.matmul(
                            ps[:],
                            w1wq_sb[:, j * C : (j + 1) * C],
                            w3[:, r0 : r0 + RT, j : j + W],
                            start=False,
                            stop=False,
                        )
                    nc.tensor.matmul(
                        ps[:],
                        w1wq2_sb,
                        wq2[:, r0 + 2 : r0 + 2 + RT, 0 : W],
                        start=False,
                        stop=False,
                    )
                    nc.tensor.matmul(
                        ps[:],
                        w1w_sb[:, 8 * C : 9 * C],
                        w3[:64, r0 + 2 : r0 + 2 + RT, 2 : 2 + W],
                        start=False,
                        stop=True,
                    )
                    hm = hmp.tile([C, FT], fp16, tag="hm")
                    nc.scalar.activation(hm[:], ps[:], Tanh, bias=b1_sb)
                    ps2 = psB2.tile([NBT, FT], fp32, tag="psB", bufs=1, name="ps2")
                    nc.tensor.matmul(ps2[:], w2_sb, hm[:], start=True, stop=True)
                    nc.scalar.activation(
                        b3d[:, r0 + 1 : r0 + 1 + RT, 1 : 1 + W],
                        ps2[:].rearrange("c (r w) -> c r w", w=W),
                        Tanh,
                        bias=b2_sb,
                    )

                def emit_a3(t):
                    pss = psB2.tile([TP, NBT], fp16, tag="pst", bufs=1, name="pss")
                    nc.tensor.transpose(
                        pss[:], bsb[:, t * TP + 1 : t * TP + 1 + TP], identNBT[:]
                    )
                    nc.vector.tensor_copy(scT[:, t * NBT : (t + 1) * NBT], pss[:])

                def emit_f(t):
                    psf = psB2.tile([C, TEM * C], fp32, tag="psf", bufs=1, name="psf")
                    for k in range(TEM):
                        for di in range(3):
                            nc.tensor.matmul(
                                psf[:, k * C : (k + 1) * C],
                                bndf_sb[:, (k * 3 + di) * C : (k * 3 + di + 1) * C],
                                fTd_sb[:, (t * 3 + di) * C : (t * 3 + di + 1) * C],
                                start=(di == 0),
                                stop=(di == 2),
                            )
                    return psf

                def emit_builds(t, dg, js):
                    for j in js:
                        k, m = divmod(j, NB)
                        if True:
                            sc = scT[
                                :, t * NBT + m * TEM + k : t * NBT + m * TEM + k + 1
                            ]
                            dslice = dg[:, j * TP : (j + 1) * TP]
                            eng = _ENG_PAT[j]
                            if eng == "D":
                                nc.vector.tensor_scalar(
                                    dslice, identTP[:], sc, None, MUL
                                )
                            elif eng == "P":
                                nc.gpsimd.tensor_scalar(
                                    dslice, identTP[:], sc, None, MUL
                                )
                            else:
                                nc.scalar.activation(dslice, identTP[:], Copy, scale=sc)

                for r in range(4):
                    emit_arow_w(r, emit_arow_f(r))
                emit_a3(0)
                emit_a3(1)
                for i in range(NTF + 3):
                    # stream in the next conv row-tile in two half-blocks
                    # (stays ~2 row-tiles ahead of the A3 lookahead)
                    if i % 3 == 0 and i // 3 + 4 < NT:
                        arow_ps = emit_arow_f(i // 3 + 4)
                    elif i % 3 == 1 and i // 3 + 4 < NT:
                        emit_arow_w(i // 3 + 4, arow_ps)
                    # acc^T(i-2) psum -> SBUF (frees accT for this round's diag)
                    if 0 <= i - 2 < NTF:
                        boS = bop.tile([C, NB * C], fp16, tag="boS")
                        boS_r[i - 2] = boS
                        acc = acc_r.pop(i - 2)
                        nc.gpsimd.tensor_copy(boS[:], acc[:])
                    # m=5 ts-scales for tile i-1 (early, so the add-chain and
                    # identity-transpose land before the PE needs them)
                    if 0 <= i - 1 < NTF:
                        tm = i - 1
                        fbS5 = fbS_r[tm]
                        P5 = p5p.tile([TP, TEM * C], fp16, tag="P5")
                        for k in range(TEM):
                            sc = scT[
                                :, tm * NBT + 5 * TEM + k : tm * NBT + 5 * TEM + k + 1
                            ]
                            dst = P5[:, k * C : (k + 1) * C]
                            srcf = fbS5[:, k * C : (k + 1) * C]
                            nc.vector.tensor_scalar(dst, srcf, sc, None, MUL)
                    # orow(i-3) + store
                    if 0 <= i - 3 < NTF:
                        j = i - 3
                        g = j % SGRP
                        if g == 0:
                            orow_bufs[j] = orp.tile(
                                [C, SGRP * TP], fp16, tag="orow", name="orow_buf"
                            )
                        ob = orow_bufs[j - g]
                        nc.scalar.activation(
                            ob[:, g * TP : (g + 1) * TP], pso_r.pop(j), Ident,
                            bias=b3_sb,
                        )
                        if g == SGRP - 1 or j == NTF - 1:
                            t0 = j - g
                            nc.sync.dma_start(
                                out[:, t0 * TP + 1 : t0 * TP + 1 + (g + 1) * TP],
                                ob[:, : (g + 1) * TP],
                            )
                            del orow_bufs[t0]
                    # per-pixel scalar table two tiles ahead
                    if i + 2 < NTF:
                        emit_a3(i + 2)
                    # F(i), diag builds(i), F evac(i) -- evac emitted between
                    # build batches so it completes mid-period (F(i+1) then
                    # never waits on the psf buffer)
                    if i < NTF:
                        psf = emit_f(i)
                        dg = dgp.tile([TP, NBT * TP], fp16, tag="dg")
                        dg_r[i] = dg
                        emit_builds(i, dg, [j for j in range(12) if j % NB != 5])
                        if 0 <= i - 1 < NTF:
                            R3 = p5p.tile([TP, 3 * C], fp16, tag="R3")
                            nc.vector.tensor_tensor(
                                R3[:], P5[:, : 3 * C], P5[:, 3 * C :], ADD
                            )
                            R1 = p5p.tile([TP, C], fp16, tag="R1")
                            nc.vector.tensor_tensor(
                                R1[:], R3[:, :C], R3[:, C : 2 * C], ADD
                            )
                            nc.vector.tensor_tensor(
                                R1[:], R1[:], R3[:, 2 * C :], ADD
                            )
                        fbS = fbp.tile([TP, TEM * C], fp16, tag="fbS")
                        nc.scalar.copy(fbS[:], psf[:TP, :])
                        fbS_r[i] = fbS
                        emit_builds(i, dg, [j for j in range(12, NBT) if j % NB != 5])
                    # diag matmuls (i-1): acc_m^T = sum_k F_k^T @ diag(s_mk)
                    if 0 <= i - 1 < NTF:
                        acc = psB2.tile([C, NB * C], fp32, tag="accT", bufs=1,
                                        name="acc")
                        acc_r[i - 1] = acc
                        fbS = fbS_r.pop(i - 1)
                        dg = dg_r.pop(i - 1)
                        for m in range(NB - 1):
                            for k in range(TEM):
                                j = k * NB + m
                                nc.tensor.matmul(
                                    acc[:, m * C : m * C + TP],
                                    fbS[:, k * C : (k + 1) * C],
                                    dg[:, j * TP : (j + 1) * TP],
                                    start=(k == 0),
                                    stop=(k == TEM - 1),
                                )

                    # coef matmuls (i-2)
                    if 0 <= i - 2 < NTF:
                        psoT = psB2.tile([C, C], fp32, tag="pso", bufs=1,
                                         name="psoT")
                        pso = psoT[:, :TP]
                        pso_r[i - 2] = pso
                        boS = boS_r.pop(i - 2)
                        for m in range(NB):
                            nc.tensor.matmul(
                                pso,
                                coefT_sb[:, m * C : (m + 1) * C],
                                boS[:, m * C : m * C + TP],
                                start=(m == 0),
                                stop=(m == NB - 1),
                            )
                    # m=5 identity-transpose into its accT slot (deps land by now)
                    if 0 <= i - 1 < NTF:
                        nc.tensor.matmul(
                            acc[:, 5 * C : 5 * C + TP],
                            R1[:],
                            identTP[:],
                            start=True,
                            stop=True,
                        )

    nc.compile()
    return nc


def _get_nc():
    if "nc" not in _CACHE:
        _CACHE["nc"] = build_nc()
    return _CACHE["nc"]


def _prep_maps(feat, weight, conv1_w, conv1_b, conv2_w, conv2_b, bases_buf, coef, bias):
    feat = np.asarray(feat, np.float32)
    weight = np.asarray(weight, np.float32)
    conv1_w = np.asarray(conv1_w, np.float32)
    conv2_w = np.asarray(conv2_w, np.float32)
    bases_buf = np.asarray(bases_buf, np.float32)
    coef = np.asarray(coef, np.float32)

    np8 = mybir.dt.np(fp8)
    n = feat.shape[0]
    featp = np.zeros((n, C, HP, WP), np.float16)
    featp[:, :, 1 : H + 1, 1 : W + 1] = feat
    wgtp = np.zeros((n, CW, HP, WP), np.float16)
    wgtp[:, :, 1 : H + 1, 1 : W + 1] = weight

    # host-prepped row-shifted transposed feature chunks:
    # fTd[p, (di*NTF + t)*C + c] = fe[c, FOFF + t*TP + (di-1)*WP + p]
    fe = np.zeros((n, C, FEXT), np.float16)
    fe[:, :, FOFF : FOFF + NPAD] = featp.reshape(n, C, NPAD)
    fTdh = np.empty((n, 3, NTF, C, C), np.float16)
    for di in range(3):
        for t in range(NTF):
            s0 = FOFF + t * TP + (di - 1) * WP
            fTdh[:, di, t] = fe[:, :, s0 : s0 + C].transpose(0, 2, 1)
    fTdh = np.ascontiguousarray(
        fTdh.transpose(0, 3, 2, 1, 4).reshape(n, C, 3 * NTF * C)
    )

    w1f = np.ascontiguousarray(
        conv1_w[:, :C].transpose(1, 2, 3, 0).reshape(C, L * C)
    ).astype(np.float16)
    w1w = np.ascontiguousarray(
        conv1_w[:, C:].transpose(1, 2, 3, 0).reshape(CW, L * C)
    ).astype(np.float16)
    w2h = np.ascontiguousarray(conv2_w[:, :, 0, 0].T).astype(np.float16)
    # flat band matrices: bndf[q, (k,di)*C + p] = bases_buf[k, di*3 + (q-p)]
    bndfh = np.zeros((C, TEM, 3, C), np.float32)
    for k in range(TEM):
        for di in range(3):
            for dj in range(3):
                for p in range(TP):
                    bndfh[p + dj, k, di, p] = bases_buf[k, di * 3 + dj]
    bndfh = bndfh.reshape(C, TEM * 3 * C).astype(np.float16)
    coefTh = np.ascontiguousarray(
        coef[:, :, 0, 0].reshape(C, C, NB).transpose(1, 2, 0).reshape(C, NB * C)
    ).astype(np.float16)
    b1h = np.asarray(conv1_b, np.float32).reshape(C, 1)
    b2h = np.asarray(conv2_b, np.float32).reshape(NBT, 1)
    b3h = np.asarray(bias, np.float32).reshape(C, 1)

    wgtq = np.zeros((n, C, NPAD), np.float16)
    wgtq[:, :CW] = wgtp.reshape(n, CW, NPAD)
    wgtq[:, CW:, : NPAD - WP] = wgtp.reshape(n, CW, NPAD)[:, :, WP:]
    # paired weights: rows 0-63 = tap (0,j), rows 64-127 = tap (1,j)
    wgtq2 = np.zeros((n, C, NPAD), np.float16)
    wgtq2[:, :CW] = wgtp.reshape(n, CW, NPAD)
    wgtq2[:, CW:, : NPAD - 1] = wgtp.reshape(n, CW, NPAD)[:, :, 1:]
    w1wq2 = np.concatenate(
        [
            w1w.reshape(CW, 3, 3, C)[:, 2, 0],
            w1w.reshape(CW, 3, 3, C)[:, 2, 1],
        ],
        axis=0,
    ).reshape(C, C)
    w1wq = np.concatenate(
        [
            w1w.reshape(CW, 3, 3, C)[:, 0],
            w1w.reshape(CW, 3, 3, C)[:, 1],
        ],
        axis=0,
    ).reshape(C, 3 * C)
    pkw = np.concatenate([w1wq, w1wq2], axis=1)
    pk = np.concatenate([w2h, coefTh, bndfh], axis=1)
    pb = np.zeros((C, 3), np.float32)
    pb[:, 0:1] = b1h
    pb[:, 1:2] = b3h
    pb[:NBT, 2:3] = b2h
    shared = {"w1f": w1f, "pkw": pkw, "pk": pk, "w1w": w1w, "pb": pb}
    return [
        {
            "featp": featp[i].reshape(C, NPAD).astype(np.float16),
            "wgtq": wgtq[i], "wgtq2": wgtq2[i],
            "fTd": fTdh[i],
            **shared,
        }
        for i in range(n)
    ]


def kernel(feat, weight, conv1_w, conv1_b, conv2_w, conv2_b, bases_buf, coef, bias,
           **run_kwargs):
    in_maps = _prep_maps(
        feat, weight, conv1_w, conv1_b, conv2_w, conv2_b, bases_buf, coef, bias
    )
    res = run_bass_kernel_spmd(
        _get_nc(), in_maps, core_ids=list(range(len(in_maps))), **run_kwargs
    )
    outp = np.stack([r["out"] for r in res.results], 0).astype(np.float32)
    outp = outp[:, :, :NPAD].reshape(-1, C, HP, WP)[:, :, 1 : H + 1, 1 : W + 1]
    _CACHE["last_results"] = res
    return np.ascontiguousarray(outp)



# revision 19
# speedup vs baseline: 1.2811x; 1.2811x over previous
"""Trainium2 Bass kernel for the DCF (dynamic conv filter) module.

Sharding: pure data-parallel over batch N=8 across 8 NeuronCores (one image
per core); all parameters replicated.

Pipeline per core (one 128x96x96 image):
  A:  conv1 (3x3, 192->128) + tanh -> hmid;  conv2 (1x1, 128->36) + tanh -> b
  B:  per 64-pixel group g:
        - SV: double transpose of the b rows + strided half-copies give the
          per-pixel scales in (k-parity x 64px, k-pair*6+m) layout
        - F: fixed-basis convs of feat via k-pair-interleaved banded matmuls
          on host-prepped transposed feature chunks (fTd), 3 di accumulated;
          output partitions are (k-parity*64 + pixel)
        - acc_m = sum_kpair fbS_kpair^T @ blockdiag2(s) -- each matmul
          contracts 2 k's at once (k lives in the contraction dim), so the
          k-reduction costs half the rows of the per-(m,k) diag scheme
        - out_group = sum_m coef_m @ acc_m (+bias)

Block-diagonals are built as tensor_scalar(mask2 * s) (DVE 4x mode); acc and
pso are double-buffered in PSUM so evacuation copies never stall the PE."""

from itertools import product

import numpy as np

import concourse.bass as bass
import concourse.tile as tile
from concourse import bacc, mybir
from concourse.bass_utils import run_bass_kernel_spmd
from concourse.masks import make_identity

fp16 = mybir.dt.float16
fp32 = mybir.dt.float32

N_CORES = 8
C = 128
CW = 64
H = W = 96
HP = WP = 98
NPIX = H * W
NPAD = HP * WP  # 9604
NB = 6
TEM = 6
L = 9
NBT = NB * TEM  # 36
RT = 4
FT = RT * W  # 384
NT = H // RT  # 24
GP = 64           # output pixels per group
NG = 152          # groups (cover padded idx 1 .. 1+152*64 = 9729)
BP = 9732         # padded bsb/out length
FEXT = 10000      # extended (host-side) padded feat length for fTd windows
FOFF = 98         # fTd window base offset inside the extended buffer
OG = 8            # output groups per store
FCH8 = 8 * 3 * C  # fTd cols per streamed chunk (8 groups)
NCH = (NG + 7) // 8  # 19 chunks

# build engine mix for the 18 blockdiag builds per group: 12 DVE (86ns in 4x
# mode), 4 Pool, 2 Act
_B18 = ["D", "P", "D", "P", "D", "P", "D", "P", "D",
        "A", "P", "D", "D", "P", "D", "P", "D", "D"]

_CACHE = {}


def build_nc():
    nc = bacc.Bacc("TRN2", target_bir_lowering=False, debug=False)

    featp = nc.dram_tensor("featp", [C, NPAD], fp16, kind="ExternalInput").ap()
    wgtq = nc.dram_tensor("wgtq", [C, NPAD], fp16, kind="ExternalInput").ap()
    fTd = nc.dram_tensor("fTd", [C, 3 * NG * C], fp16, kind="ExternalInput").ap()
    wgtq2 = nc.dram_tensor("wgtq2", [C, NPAD], fp16, kind="ExternalInput").ap()
    w1f = nc.dram_tensor("w1f", [C, L * C], fp16, kind="ExternalInput").ap()
    # conv1 weight-branch params: w1wq|w1wq2 = 384+128
    pkw = nc.dram_tensor("pkw", [C, 512], fp16, kind="ExternalInput").ap()
    # fp16 params packed: w2|coefT|bnd64|mask2 = 36+768+1152+64
    pk = nc.dram_tensor("pk", [C, 2020], fp16, kind="ExternalInput").ap()
    w1w = nc.dram_tensor("w1w", [CW, C], fp16, kind="ExternalInput").ap()
    pb = nc.dram_tensor("pb", [C, 3], fp32, kind="ExternalInput").ap()
    out = nc.dram_tensor("out", [C, BP], fp16, kind="ExternalOutput").ap()

    Tanh = mybir.ActivationFunctionType.Tanh
    Ident = mybir.ActivationFunctionType.Identity
    Copy = mybir.ActivationFunctionType.Copy
    MUL = mybir.AluOpType.mult
    ADD = mybir.AluOpType.add

    with tile.TileContext(nc) as tc:
        with (
            tc.tile_pool(name="const", bufs=1) as const,
            tc.tile_pool(name="big", bufs=1) as big,
        ):
            featp_sb = big.tile([C, NPAD], fp16)
            wgtp_sb = big.tile([C, NPAD], fp16)
            cuts = [0, 2404, 4808, 7212, NPAD]
            # startup: tiny "head" pieces first so the first conv row-tile's
            # operands land within ~2.5us (DMA pipeline latency floor)
            HD = 686  # cols covering feat/wgt rows 0..6 (row-tile 0 + halo)
            w1f_sb = const.tile([C, L * C], fp16)
            nc.sync.dma_start(w1f_sb[:, : 4 * C], w1f[:, : 4 * C])
            nc.sync.dma_start(featp_sb[:, :HD], featp[:, :HD])
            pkw_sb = const.tile([C, 512], fp16)
            nc.sync.dma_start(pkw_sb[:], pkw)
            w1wq_sb = pkw_sb[:, 0:384]
            w1wq2_sb = pkw_sb[:, 384:512]
            nc.sync.dma_start(w1f_sb[:, 4 * C :], w1f[:, 4 * C :])
            wgtq2_sb = big.tile([C, NPAD], fp16)
            nc.sync.dma_start(wgtp_sb[:, :HD], wgtq[:, :HD])
            nc.sync.dma_start(wgtq2_sb[:, :HD], wgtq2[:, :HD])
            pb_sb = const.tile([C, 3], fp32)
            nc.sync.dma_start(pb_sb[:], pb)
            b1_sb = pb_sb[:, 0:1]
            b3_sb = pb_sb[:, 1:2]
            b2_sb = pb_sb[:NBT, 2:3]
            w1w_sb = const.tile([CW, C], fp16)
            nc.sync.dma_start(w1w_sb[:], w1w)
            nc.sync.dma_start(featp_sb[:, HD : cuts[1]], featp[:, HD : cuts[1]])
            pk_sb = const.tile([C, 2020], fp16)
            nc.sync.dma_start(pk_sb[:, 804:], pk[:, 804:])
            nc.sync.dma_start(pk_sb[:, :804], pk[:, :804])
            w2_sb = pk_sb[:, 0:36]
            coefT_sb = pk_sb[:, 36:804]
            bnd64_sb = pk_sb[:, 804:1956]
            mask2_sb = pk_sb[:, 1956:2020]
            nc.sync.dma_start(wgtp_sb[:, HD : cuts[2]], wgtq[:, HD : cuts[2]])
            nc.sync.dma_start(
                featp_sb[:, cuts[1] : cuts[2]], featp[:, cuts[1] : cuts[2]]
            )
            nc.sync.dma_start(
                featp_sb[:, cuts[2] : cuts[3]], featp[:, cuts[2] : cuts[3]]
            )
            nc.sync.dma_start(wgtp_sb[:, cuts[2] :], wgtq[:, cuts[2] :])
            nc.sync.dma_start(wgtq2_sb[:, HD : cuts[1]], wgtq2[:, HD : cuts[1]])
            nc.sync.dma_start(wgtq2_sb[:, cuts[1] :], wgtq2[:, cuts[1] :])
            nc.sync.dma_start(
                featp_sb[:, cuts[3] :], featp[:, cuts[3] :]
            )
            identNBT = const.tile([NBT, NBT], fp16)
            make_identity(nc, identNBT[:])

            bsb = big.tile([NBT, BP], fp16)
            # zero only the border/tail cells conv2 never writes (full memset
            # would hold Pool for 8us before the first b write)
            nc.gpsimd.memset(bsb[:, : WP + 2], 0.0)
            edge = bsb[:, 97 : 97 + 97 * WP].rearrange("c (r w) -> c r w", w=WP)
            nc.gpsimd.memset(edge[:, :, 0:2], 0.0)
            nc.gpsimd.memset(bsb[:, 97 * WP :], 0.0)

            b3d = bsb[:, :NPAD].rearrange("c (r w) -> c r w", w=WP)
            f3 = featp_sb[:].rearrange("c (r w) -> c r w", w=WP)
            w3 = wgtp_sb[:].rearrange("c (r w) -> c r w", w=WP)
            wq2 = wgtq2_sb[:].rearrange("c (r w) -> c r w", w=WP)

            # ---- fused pipeline: conv rows (phase A) stream in between the
            # software-pipelined per-group stages of phase B.
            with (
                tc.tile_pool(name="hmp", bufs=3) as hmp,
                tc.tile_pool(name="ftp", bufs=3) as ftp,
                tc.tile_pool(name="svsp", bufs=3) as svsp,
                tc.tile_pool(name="fbp", bufs=2) as fbp,
                tc.tile_pool(name="dgp", bufs=2) as dgp,
                tc.tile_pool(name="bop", bufs=2) as bop,
                tc.tile_pool(name="orp", bufs=2) as orp,
                tc.tile_pool(name="psB2", bufs=1, space="PSUM") as psB2,
            ):
                fbS_r, dg_r, boS_r, pso_r, acc_r = {}, {}, {}, {}, {}
                svS_r, ft_r = {}, {}
                orow_bufs = {}

                def emit_arow_f(t):
                    r0 = t * RT
                    ps = psB2.tile([C, FT], fp32, tag="psA", bufs=1, name="ps")
                    for kk, (i, j) in enumerate(product(range(3), range(3))):
                        nc.tensor.matmul(
                            ps[:],
                            w1f_sb[:, (i * 3 + j) * C : (i * 3 + j + 1) * C],
                            f3[:, r0 + i : r0 + i + RT, j : j + W],
                            start=(kk == 0),
                            stop=False,
                        )
                    return ps

                def emit_arow_w(t, ps):
                    r0 = t * RT
                    for j in range(3):
                        nc.tensor.matmul(
                            ps[:],
                            w1wq_sb[:, j * C : (j + 1) * C],
                            w3[:, r0 : r0 + RT, j : j + W],
                            start=False,
                            stop=False,
                        )
                    nc.tensor.matmul(
                        ps[:],
                        w1wq2_sb,
                        wq2[:, r0 + 2 : r0 + 2 + RT, 0 : W],
                        start=False,
                        stop=False,
                    )
                    nc.tensor.matmul(
                        ps[:],
                        w1w_sb[:],
                        w3[:64, r0 + 2 : r0 + 2 + RT, 2 : 2 + W],
                        start=False,
                        stop=True,
                    )
                    hm = hmp.tile([C, FT], fp16, tag="hm")
                    nc.scalar.activation(hm[:], ps[:], Tanh, bias=b1_sb)
                    ps2 = psB2.tile([NBT, FT], fp32, tag="psB", bufs=1, name="ps2")
                    nc.tensor.matmul(ps2[:], w2_sb, hm[:], start=True, stop=True)
                    nc.scalar.activation(
                        b3d[:, r0 + 1 : r0 + 1 + RT, 1 : 1 + W],
                        ps2[:].rearrange("c (r w) -> c r w", w=W),
                        Tanh,
                        bias=b2_sb,
                    )

                def emit_sv(g):
                    # b rows (k-major: row = k*6+m) -> per-pixel scales in
                    # (k-parity*64+p, kpair*6+m) layout via double transpose
                    # + strided half copies
                    svp = psB2.tile([C, NBT], fp32, tag="svp", bufs=1,
                                    name="svp")
                    src = bsb[:, 1 + g * GP : 1 + (g + 1) * GP]
                    nc.tensor.matmul(svp[:GP, :], src, identNBT[:],
                                     start=True, stop=True)
                    nc.tensor.matmul(svp[GP:, :], src, identNBT[:],
                                     start=True, stop=True)
                    svS = svsp.tile([C, 18], fp32, tag="svS")
                    svS_r[g] = svS
                    lo = svp[:GP, :].rearrange("p (kp ki m) -> p kp ki m",
                                               ki=2, m=NB)
                    hi = svp[GP:, :].rearrange("p (kp ki m) -> p kp ki m",
                                               ki=2, m=NB)
                    dlo = svS[:GP, :].rearrange("p (kp m) -> p kp m", m=NB)
                    dhi = svS[GP:, :].rearrange("p (kp m) -> p kp m", m=NB)
                    nc.vector.tensor_copy(dlo, lo[:, :, 0, :])
                    nc.vector.tensor_copy(dhi, hi[:, :, 1, :])

                def emit_builds(g, dg, js):
                    svS = svS_r[g]
                    for j in js:
                        m, kp = divmod(j, 3)
                        sc = svS[:, kp * NB + m : kp * NB + m + 1]
                        dslice = dg[:, j * GP : (j + 1) * GP]
                        eng = _B18[j]
                        if eng == "D":
                            nc.vector.tensor_scalar(
                                dslice, mask2_sb, sc, None, MUL
                            )
                        elif eng == "P":
                            nc.gpsimd.tensor_scalar(
                                dslice, mask2_sb, sc, None, MUL
                            )
                        else:
                            nc.scalar.activation(dslice, mask2_sb, Copy,
                                                 scale=sc)

                # conv row-tile slots spread at cadence ~6.5 so conv filler
                # lasts until iteration ~138 (row 23 deadline is ~141)
                _fs = {int(_r * 6.5 + 0.5): _r + 2 for _r in range(NT - 2)}
                _ws = {k + 1: v for k, v in _fs.items()}

                for r in range(2):
                    emit_arow_w(r, emit_arow_f(r))
                # fTd chunks 0,1 + SV for groups 0,1
                for cch in range(2):
                    ft = ftp.tile([C, FCH8], fp16, tag="ft")
                    ft_r[cch] = ft
                    nc.sync.dma_start(ft[:], fTd[:, cch * FCH8 : (cch + 1) * FCH8])
                emit_sv(0)
                emit_sv(1)

                for i in range(NG + 3):
                    if i in _fs:
                        arow_ps = emit_arow_f(_fs[i])
                    elif i in _ws:
                        emit_arow_w(_ws[i], arow_ps)
                    # acc(i-2) psum -> SBUF (Pool)
                    if 0 <= i - 2 < NG:
                        boS = bop.tile([C, NB * GP], fp16, tag="boS")
                        boS_r[i - 2] = boS
                        acc = acc_r.pop(i - 2)
                        nc.vector.tensor_copy(boS[:, : 3 * GP], acc[:, : 3 * GP])
                        nc.scalar.copy(boS[:, 3 * GP :], acc[:, 3 * GP :])
                    # orow(i-3) + store
                    if 0 <= i - 3 < NG:
                        j = i - 3
                        g8 = j % OG
                        if g8 == 0:
                            orow_bufs[j] = orp.tile(
                                [C, OG * GP], fp16, tag="orow", name="orow_buf"
                            )
                        ob = orow_bufs[j - g8]
                        nc.scalar.activation(
                            ob[:, g8 * GP : (g8 + 1) * GP], pso_r.pop(j),
                            Ident, bias=b3_sb,
                        )
                        if g8 == OG - 1 or j == NG - 1:
                            t0 = j - g8
                            nc.sync.dma_start(
                                out[:, 1 + t0 * GP : 1 + t0 * GP + (g8 + 1) * GP],
                                ob[:, : (g8 + 1) * GP],
                            )
                            del orow_bufs[t0]
                    # stream next fTd chunk
                    if i % 8 == 0 and i // 8 + 2 < NCH:
                        cch = i // 8 + 2
                        ft = ftp.tile([C, FCH8], fp16, tag="ft")
                        ft_r[cch] = ft
                        nc.sync.dma_start(
                            ft[:], fTd[:, cch * FCH8 : (cch + 1) * FCH8]
                        )
                    # per-pixel scales two groups ahead
                    if i + 2 < NG:
                        emit_sv(i + 2)
                    # F(i), blockdiag builds(i), F evac(i)
                    if i < NG:
                        psf = psB2.tile([C, 3 * C], fp32, tag="psf", bufs=1,
                                        name="psf")
                        ft = ft_r[i // 8]
                        for kp in range(3):
                            for di in range(3):
                                nc.tensor.matmul(
                                    psf[:, kp * C : (kp + 1) * C],
                                    bnd64_sb[:, (kp * 3 + di) * C
                                             : (kp * 3 + di + 1) * C],
                                    ft[:, ((i % 8) * 3 + di) * C
                                       : ((i % 8) * 3 + di + 1) * C],
                                    start=(di == 0),
                                    stop=(di == 2),
                                )
                        dg = dgp.tile([C, 18 * GP], fp16, tag="dg")
                        dg_r[i] = dg
                        emit_builds(i, dg, range(9))
                        fbS = fbp.tile([C, 3 * C], fp16, tag="fbS")
                        nc.scalar.copy(fbS[:], psf[:])
                        fbS_r[i] = fbS
                        emit_builds(i, dg, range(9, 18))
                        if i % 8 == 7 or i == NG - 1:
                            ft_r.pop(i // 8)
                    # blockdiag matmuls (i-1): acc_m = sum_kp fbS_kp^T @ bd2
                    if 0 <= i - 1 < NG:
                        acc = psB2.tile([C, NB * GP], fp32, tag="accT", bufs=2,
                                        name="acc")
                        acc_r[i - 1] = acc
                        fbS = fbS_r.pop(i - 1)
                        dg = dg_r.pop(i - 1)
                        for m in range(NB):
                            for kp in range(3):
                                j = m * 3 + kp
                                nc.tensor.matmul(
                                    acc[:, m * GP : (m + 1) * GP],
                                    fbS[:, kp * C : (kp + 1) * C],
                                    dg[:, j * GP : (j + 1) * GP],
                                    start=(kp == 0),
                                    stop=(kp == 2),
                                )
                    # coef matmuls (i-2)
                    if 0 <= i - 2 < NG:
                        pso = psB2.tile([C, GP], fp32, tag="pso", bufs=2,
                                        name="pso")
                        pso_r[i - 2] = pso
                        boS = boS_r.pop(i - 2)
                        for m in range(NB):
                            nc.tensor.matmul(
                                pso[:],
                                coefT_sb[:, m * C : (m + 1) * C],
                                boS[:, m * GP : (m + 1) * GP],
                                start=(m == 0),
                                stop=(m == NB - 1),
                            )

    nc.compile()
    return nc


def _get_nc():
    if "nc" not in _CACHE:
        _CACHE["nc"] = build_nc()
    return _CACHE["nc"]


def _prep_maps(feat, weight, conv1_w, conv1_b, conv2_w, conv2_b, bases_buf, coef, bias):
    feat = np.asarray(feat, np.float32)
    weight = np.asarray(weight, np.float32)
    conv1_w = np.asarray(conv1_w, np.float32)
    conv2_w = np.asarray(conv2_w, np.float32)
    bases_buf = np.asarray(bases_buf, np.float32)
    coef = np.asarray(coef, np.float32)

    n = feat.shape[0]
    featp = np.zeros((n, C, HP, WP), np.float16)
    featp[:, :, 1 : H + 1, 1 : W + 1] = feat
    wgtp = np.zeros((n, CW, HP, WP), np.float16)
    wgtp[:, :, 1 : H + 1, 1 : W + 1] = weight

    # host-prepped transposed feature chunks per (group, di):
    # fTd[q, (g*3+di)*C + c] = fe[c, FOFF + g*GP + (di-1)*WP + q]
    fe = np.zeros((n, C, FEXT), np.float16)
    fe[:, :, FOFF : FOFF + NPAD] = featp.reshape(n, C, NPAD)
    fTdh = np.empty((n, 3 * NG, C, C), np.float16)
    for g in range(NG):
        for di in range(3):
            s0 = FOFF + g * GP + (di - 1) * WP
            fTdh[:, g * 3 + di] = fe[:, :, s0 : s0 + C].transpose(0, 2, 1)
    fTdh = np.ascontiguousarray(
        fTdh.transpose(0, 2, 1, 3).reshape(n, C, 3 * NG * C)
    )

    w1f = np.ascontiguousarray(
        conv1_w[:, :C].transpose(1, 2, 3, 0).reshape(C, L * C)
    ).astype(np.float16)
    w1w = np.ascontiguousarray(
        conv1_w[:, C:].transpose(1, 2, 3, 0).reshape(CW, L * C)
    ).astype(np.float16)
    # conv2 weights with k-major output-channel ordering (col = k*NB + m)
    w2h = np.ascontiguousarray(conv2_w[:, :, 0, 0].T).astype(np.float16)
    w2i = np.ascontiguousarray(
        w2h.reshape(C, NB, TEM).transpose(0, 2, 1).reshape(C, NBT)
    )
    # interleaved band blocks: bnd64[q, (kp*3+di)*C + ki*GP + p]
    #   = bases_buf[2*kp+ki, di*3 + (q-p)] for q-p in 0..2
    bndh = np.zeros((C, 3, 3, 2, GP), np.float32)
    for kp in range(3):
        for di in range(3):
            for ki in range(2):
                k = 2 * kp + ki
                for dj in range(3):
                    for p in range(GP):
                        bndh[p + dj, kp, di, ki, p] = bases_buf[k, di * 3 + dj]
    bndh = bndh.reshape(C, 9 * C).astype(np.float16)
    # mask2: ones at (p,p) and (GP+p, p)
    mask2 = np.zeros((C, GP), np.float16)
    for p in range(GP):
        mask2[p, p] = 1.0
        mask2[GP + p, p] = 1.0
    coefTh = np.ascontiguousarray(
        coef[:, :, 0, 0].reshape(C, C, NB).transpose(1, 2, 0).reshape(C, NB * C)
    ).astype(np.float16)
    b1h = np.asarray(conv1_b, np.float32).reshape(C, 1)
    b2h = np.asarray(conv2_b, np.float32).reshape(NB, TEM).T.reshape(NBT, 1)
    b3h = np.asarray(bias, np.float32).reshape(C, 1)

    wgtq = np.zeros((n, C, NPAD), np.float16)
    wgtq[:, :CW] = wgtp.reshape(n, CW, NPAD)
    wgtq[:, CW:, : NPAD - WP] = wgtp.reshape(n, CW, NPAD)[:, :, WP:]
    # paired weights: rows 0-63 = tap (0,j), rows 64-127 = tap (1,j)
    wgtq2 = np.zeros((n, C, NPAD), np.float16)
    wgtq2[:, :CW] = wgtp.reshape(n, CW, NPAD)
    wgtq2[:, CW:, : NPAD - 1] = wgtp.reshape(n, CW, NPAD)[:, :, 1:]
    w1wq2 = np.concatenate(
        [
            w1w.reshape(CW, 3, 3, C)[:, 2, 0],
            w1w.reshape(CW, 3, 3, C)[:, 2, 1],
        ],
        axis=0,
    ).reshape(C, C)
    w1wq = np.concatenate(
        [
            w1w.reshape(CW, 3, 3, C)[:, 0],
            w1w.reshape(CW, 3, 3, C)[:, 1],
        ],
        axis=0,
    ).reshape(C, 3 * C)
    pkw = np.concatenate([w1wq, w1wq2], axis=1)
    pk = np.concatenate([w2i, coefTh, bndh, mask2], axis=1)
    pb = np.zeros((C, 3), np.float32)
    pb[:, 0:1] = b1h
    pb[:, 1:2] = b3h
    pb[:NBT, 2:3] = b2h
    shared = {"w1f": w1f, "pkw": pkw, "pk": pk,
              "w1w": np.ascontiguousarray(w1w[:, 8 * C:]), "pb": pb}
    return [
        {
            "featp": featp[i].reshape(C, NPAD).astype(np.float16),
            "wgtq": wgtq[i], "wgtq2": wgtq2[i],
            "fTd": fTdh[i],
            **shared,
        }
        for i in range(n)
    ]


def kernel(feat, weight, conv1_w, conv1_b, conv2_w, conv2_b, bases_buf, coef, bias,
           **run_kwargs):
    in_maps = _prep_maps(
        feat, weight, conv1_w, conv1_b, conv2_w, conv2_b, bases_buf, coef, bias
    )
    res = run_bass_kernel_spmd(
        _get_nc(), in_maps, core_ids=list(range(len(in_maps))), **run_kwargs
    )
    outp = np.stack([r["out"] for r in res.results], 0).astype(np.float32)
    outp = outp[:, :, :NPAD].reshape(-1, C, HP, WP)[:, :, 1 : H + 1, 1 : W + 1]
    _CACHE["last_results"] = res
    return np.ascontiguousarray(outp)


# revision 21
# speedup vs baseline: 1.3156x; 1.0269x over previous
"""Trainium2 Bass kernel for the DCF (dynamic conv filter) module.

Sharding: pure data-parallel over batch N=8 across 8 NeuronCores (one image
per core); all parameters replicated.

Pipeline per core (one 128x96x96 image):
  A:  conv1 (3x3, 192->128) + tanh -> hmid;  conv2 (1x1, 128->36) + tanh -> b
  B:  per 64-pixel group g:
        - SV: double transpose of the b rows + strided half-copies give the
          per-pixel scales in (k-parity x 64px, k-pair*6+m) layout
        - F: fixed-basis convs of feat via k-pair-interleaved banded matmuls
          on host-prepped transposed feature chunks (fTd), 3 di accumulated;
          output partitions are (k-parity*64 + pixel)
        - acc_m = sum_kpair fbS_kpair^T @ blockdiag2(s) -- each matmul
          contracts 2 k's at once (k lives in the contraction dim), so the
          k-reduction costs half the rows of the per-(m,k) diag scheme
        - out_group = sum_m coef_m @ acc_m (+bias)

Block-diagonals are built as tensor_scalar(mask2 * s) (DVE 4x mode); acc and
pso are double-buffered in PSUM so evacuation copies never stall the PE."""

from itertools import product

import numpy as np

import concourse.bass as bass
import concourse.tile as tile
from concourse import bacc, mybir
from concourse.bass_utils import run_bass_kernel_spmd
from concourse.masks import make_identity

fp16 = mybir.dt.float16
fp32 = mybir.dt.float32

N_CORES = 8
C = 128
CW = 64
H = W = 96
HP = WP = 98
NPIX = H * W
NPAD = HP * WP  # 9604
NB = 6
TEM = 6
L = 9
NBT = NB * TEM  # 36
RT = 4
FT = RT * W  # 384
NT = H // RT  # 24
GP = 64           # output pixels per group
NG = 152          # groups (cover padded idx 1 .. 1+152*64 = 9729)
BP = 9732         # padded bsb/out length
FEXT = 10000      # extended (host-side) padded feat length for fTd windows
FOFF = 98         # fTd window base offset inside the extended buffer
OG = 8            # output groups per store
FCH8 = 8 * 3 * C  # fTd cols per streamed chunk (8 groups)
NCH = (NG + 7) // 8  # 19 chunks

# build engine mix for the 18 blockdiag builds per group: 12 DVE (86ns in 4x
# mode), 4 Pool, 2 Act
_B18 = ["D", "P", "D", "P", "D", "P", "D", "P", "D",
        "P", "P", "D", "D", "P", "D", "P", "D", "D"]

_CACHE = {}


def build_nc():
    nc = bacc.Bacc("TRN2", target_bir_lowering=False, debug=False)

    featp = nc.dram_tensor("featp", [C, NPAD], fp16, kind="ExternalInput").ap()
    wgtq = nc.dram_tensor("wgtq", [C, NPAD], fp16, kind="ExternalInput").ap()
    fTd = nc.dram_tensor("fTd", [C, 3 * NG * C], fp16, kind="ExternalInput").ap()
    wgtq2 = nc.dram_tensor("wgtq2", [C, NPAD], fp16, kind="ExternalInput").ap()
    w1f = nc.dram_tensor("w1f", [C, L * C], fp16, kind="ExternalInput").ap()
    # conv1 weight-branch params: w1wq|w1wq2 = 384+128
    pkw = nc.dram_tensor("pkw", [C, 512], fp16, kind="ExternalInput").ap()
    # fp16 params packed: w2|coefT|bnd64|mask2 = 36+768+1152+64
    pk = nc.dram_tensor("pk", [C, 2020], fp16, kind="ExternalInput").ap()
    w1w = nc.dram_tensor("w1w", [CW, C], fp16, kind="ExternalInput").ap()
    pb = nc.dram_tensor("pb", [C, 3], fp32, kind="ExternalInput").ap()
    out = nc.dram_tensor("out", [C, BP], fp16, kind="ExternalOutput").ap()

    Tanh = mybir.ActivationFunctionType.Tanh
    Ident = mybir.ActivationFunctionType.Identity
    Copy = mybir.ActivationFunctionType.Copy
    MUL = mybir.AluOpType.mult
    ADD = mybir.AluOpType.add

    with tile.TileContext(nc) as tc:
        with (
            tc.tile_pool(name="const", bufs=1) as const,
            tc.tile_pool(name="big", bufs=1) as big,
        ):
            featp_sb = big.tile([C, NPAD], fp16)
            wgtp_sb = big.tile([C, NPAD], fp16)
            cuts = [0, 2404, 4808, 7212, NPAD]
            # startup: tiny "head" pieces first so the first conv row-tile's
            # operands land within ~2.5us (DMA pipeline latency floor)
            HD = 686  # cols covering feat/wgt rows 0..6 (row-tile 0 + halo)
            w1f_sb = const.tile([C, L * C], fp16)
            nc.sync.dma_start(w1f_sb[:, : 4 * C], w1f[:, : 4 * C])
            nc.sync.dma_start(featp_sb[:, :HD], featp[:, :HD])
            pkw_sb = const.tile([C, 512], fp16)
            nc.sync.dma_start(pkw_sb[:], pkw)
            w1wq_sb = pkw_sb[:, 0:384]
            w1wq2_sb = pkw_sb[:, 384:512]
            nc.sync.dma_start(w1f_sb[:, 4 * C :], w1f[:, 4 * C :])
            wgtq2_sb = big.tile([C, NPAD], fp16)
            nc.sync.dma_start(wgtp_sb[:, :HD], wgtq[:, :HD])
            nc.sync.dma_start(wgtq2_sb[:, :HD], wgtq2[:, :HD])
            pb_sb = const.tile([C, 3], fp32)
            nc.sync.dma_start(pb_sb[:], pb)
            b1_sb = pb_sb[:, 0:1]
            b3_sb = pb_sb[:, 1:2]
            b2_sb = pb_sb[:NBT, 2:3]
            w1w_sb = const.tile([CW, C], fp16)
            nc.sync.dma_start(w1w_sb[:], w1w)
            nc.sync.dma_start(featp_sb[:, HD : cuts[1]], featp[:, HD : cuts[1]])
            pk_sb = const.tile([C, 2020], fp16)
            nc.sync.dma_start(pk_sb[:, 804:], pk[:, 804:])
            nc.sync.dma_start(pk_sb[:, :804], pk[:, :804])
            w2_sb = pk_sb[:, 0:36]
            coefT_sb = pk_sb[:, 36:804]
            bnd64_sb = pk_sb[:, 804:1956]
            mask2_sb = pk_sb[:, 1956:2020]
            nc.sync.dma_start(wgtq2_sb[:, HD : cuts[1]], wgtq2[:, HD : cuts[1]])
            nc.sync.dma_start(wgtp_sb[:, HD : cuts[2]], wgtq[:, HD : cuts[2]])
            nc.sync.dma_start(
                featp_sb[:, cuts[1] : cuts[2]], featp[:, cuts[1] : cuts[2]]
            )
            nc.sync.dma_start(wgtq2_sb[:, cuts[1] :], wgtq2[:, cuts[1] :])
            nc.sync.dma_start(
                featp_sb[:, cuts[2] : cuts[3]], featp[:, cuts[2] : cuts[3]]
            )
            nc.sync.dma_start(wgtp_sb[:, cuts[2] :], wgtq[:, cuts[2] :])
            nc.sync.dma_start(
                featp_sb[:, cuts[3] :], featp[:, cuts[3] :]
            )
            identNBT = const.tile([NBT, NBT], fp16)
            make_identity(nc, identNBT[:])

            bsb = big.tile([NBT, BP], fp16)
            # zero only the border/tail cells conv2 never writes (full memset
            # would hold Pool for 8us before the first b write)
            nc.gpsimd.memset(bsb[:, : WP + 2], 0.0)
            edge = bsb[:, 97 : 97 + 97 * WP].rearrange("c (r w) -> c r w", w=WP)
            nc.gpsimd.memset(edge[:, :, 0:2], 0.0)
            nc.gpsimd.memset(bsb[:, 97 * WP :], 0.0)

            b3d = bsb[:, :NPAD].rearrange("c (r w) -> c r w", w=WP)
            f3 = featp_sb[:].rearrange("c (r w) -> c r w", w=WP)
            w3 = wgtp_sb[:].rearrange("c (r w) -> c r w", w=WP)
            wq2 = wgtq2_sb[:].rearrange("c (r w) -> c r w", w=WP)

            # ---- fused pipeline: conv rows (phase A) stream in between the
            # software-pipelined per-group stages of phase B.
            with (
                tc.tile_pool(name="hmp", bufs=3) as hmp,
                tc.tile_pool(name="ftp", bufs=3) as ftp,
                tc.tile_pool(name="svsp", bufs=3) as svsp,
                tc.tile_pool(name="fbp", bufs=2) as fbp,
                tc.tile_pool(name="dgp", bufs=2) as dgp,
                tc.tile_pool(name="bop", bufs=2) as bop,
                tc.tile_pool(name="orp", bufs=2) as orp,
                tc.tile_pool(name="psB2", bufs=1, space="PSUM") as psB2,
            ):
                fbS_r, dg_r, boS_r, pso_r, acc_r = {}, {}, {}, {}, {}
                svS_r, ft_r = {}, {}
                orow_bufs = {}

                def emit_arow_f(t):
                    r0 = t * RT
                    ps = psB2.tile([C, FT], fp32, tag="psA", bufs=1, name="ps")
                    for kk, (i, j) in enumerate(product(range(3), range(3))):
                        nc.tensor.matmul(
                            ps[:],
                            w1f_sb[:, (i * 3 + j) * C : (i * 3 + j + 1) * C],
                            f3[:, r0 + i : r0 + i + RT, j : j + W],
                            start=(kk == 0),
                            stop=False,
                        )
                    return ps

                def emit_arow_w(t, ps):
                    r0 = t * RT
                    for j in range(3):
                        nc.tensor.matmul(
                            ps[:],
                            w1wq_sb[:, j * C : (j + 1) * C],
                            w3[:, r0 : r0 + RT, j : j + W],
                            start=False,
                            stop=False,
                        )
                    nc.tensor.matmul(
                        ps[:],
                        w1wq2_sb,
                        wq2[:, r0 + 2 : r0 + 2 + RT, 0 : W],
                        start=False,
                        stop=False,
                    )
                    nc.tensor.matmul(
                        ps[:],
                        w1w_sb[:],
                        w3[:64, r0 + 2 : r0 + 2 + RT, 2 : 2 + W],
                        start=False,
                        stop=True,
                    )
                    hm = hmp.tile([C, FT], fp16, tag="hm")
                    nc.scalar.activation(hm[:], ps[:], Tanh, bias=b1_sb)
                    ps2 = psB2.tile([NBT, FT], fp32, tag="psB", bufs=1, name="ps2")
                    nc.tensor.matmul(ps2[:], w2_sb, hm[:], start=True, stop=True)
                    nc.scalar.activation(
                        b3d[:, r0 + 1 : r0 + 1 + RT, 1 : 1 + W],
                        ps2[:].rearrange("c (r w) -> c r w", w=W),
                        Tanh,
                        bias=b2_sb,
                    )

                def emit_sv(g):
                    # b rows (k-major: row = k*6+m) -> per-pixel scales in
                    # (k-parity*64+p, kpair*6+m) layout via double transpose
                    # + strided half copies
                    svp = psB2.tile([C, NBT], fp32, tag="svp", bufs=1,
                                    name="svp")
                    src = bsb[:, 1 + g * GP : 1 + (g + 1) * GP]
                    nc.tensor.matmul(svp[:GP, :], src, identNBT[:],
                                     start=True, stop=True)
                    nc.tensor.matmul(svp[GP:, :], src, identNBT[:],
                                     start=True, stop=True)
                    svS = svsp.tile([C, 18], fp32, tag="svS")
                    svS_r[g] = svS
                    lo = svp[:GP, :].rearrange("p (kp ki m) -> p kp ki m",
                                               ki=2, m=NB)
                    hi = svp[GP:, :].rearrange("p (kp ki m) -> p kp ki m",
                                               ki=2, m=NB)
                    dlo = svS[:GP, :].rearrange("p (kp m) -> p kp m", m=NB)
                    dhi = svS[GP:, :].rearrange("p (kp m) -> p kp m", m=NB)
                    nc.vector.tensor_copy(dlo, lo[:, :, 0, :])
                    nc.vector.tensor_copy(dhi, hi[:, :, 1, :])

                def emit_builds(g, dg, js):
                    svS = svS_r[g]
                    for j in js:
                        m, kp = divmod(j, 3)
                        sc = svS[:, kp * NB + m : kp * NB + m + 1]
                        dslice = dg[:, j * GP : (j + 1) * GP]
                        eng = _B18[j]
                        if eng == "D":
                            nc.vector.tensor_scalar(
                                dslice, mask2_sb, sc, None, MUL
                            )
                        elif eng == "P":
                            nc.gpsimd.tensor_scalar(
                                dslice, mask2_sb, sc, None, MUL
                            )
                        else:
                            nc.scalar.activation(dslice, mask2_sb, Copy,
                                                 scale=sc)

                # conv row-tile slots spread at cadence ~6.5 so conv filler
                # lasts until iteration ~138 (row 23 deadline is ~141)
                _fs = {int(_r * 6.5 + 0.5): _r + 2 for _r in range(NT - 2)}
                _ws = {k + 1: v for k, v in _fs.items()}

                for r in range(2):
                    emit_arow_w(r, emit_arow_f(r))
                # fTd chunks 0,1 + SV for groups 0,1
                for cch in range(2):
                    ft = ftp.tile([C, FCH8], fp16, tag="ft")
                    ft_r[cch] = ft
                    nc.sync.dma_start(ft[:], fTd[:, cch * FCH8 : (cch + 1) * FCH8])
                emit_sv(0)
                emit_sv(1)

                for i in range(NG + 3):
                    if i in _fs:
                        arow_ps = emit_arow_f(_fs[i])
                    elif i in _ws:
                        emit_arow_w(_ws[i], arow_ps)
                    # acc(i-2) psum -> SBUF (Pool)
                    if 0 <= i - 2 < NG:
                        boS = bop.tile([C, NB * GP], fp16, tag="boS")
                        boS_r[i - 2] = boS
                        acc = acc_r.pop(i - 2)
                        nc.vector.tensor_copy(boS[:, : 3 * GP], acc[:, : 3 * GP])
                        nc.scalar.copy(boS[:, 3 * GP :], acc[:, 3 * GP :])
                    # orow(i-3) + store
                    if 0 <= i - 3 < NG:
                        j = i - 3
                        g8 = j % OG
                        if g8 == 0:
                            orow_bufs[j] = orp.tile(
                                [C, OG * GP], fp16, tag="orow", name="orow_buf"
                            )
                        ob = orow_bufs[j - g8]
                        nc.scalar.activation(
                            ob[:, g8 * GP : (g8 + 1) * GP], pso_r.pop(j),
                            Ident, bias=b3_sb,
                        )
                        if g8 == OG - 1 or j == NG - 1:
                            t0 = j - g8
                            nc.sync.dma_start(
                                out[:, 1 + t0 * GP : 1 + t0 * GP + (g8 + 1) * GP],
                                ob[:, : (g8 + 1) * GP],
                            )
                            del orow_bufs[t0]
                    # stream next fTd chunk
                    if i % 8 == 0 and i // 8 + 2 < NCH:
                        cch = i // 8 + 2
                        ft = ftp.tile([C, FCH8], fp16, tag="ft")
                        ft_r[cch] = ft
                        nc.sync.dma_start(
                            ft[:], fTd[:, cch * FCH8 : (cch + 1) * FCH8]
                        )
                    # per-pixel scales two groups ahead
                    if i + 2 < NG:
                        emit_sv(i + 2)
                    # F(i), blockdiag builds(i), F evac(i)
                    if i < NG:
                        psf = psB2.tile([C, 3 * C], fp32, tag="psf", bufs=1,
                                        name="psf")
                        ft = ft_r[i // 8]
                        for kp in range(3):
                            for di in range(3):
                                nc.tensor.matmul(
                                    psf[:, kp * C : (kp + 1) * C],
                                    bnd64_sb[:, (kp * 3 + di) * C
                                             : (kp * 3 + di + 1) * C],
                                    ft[:, ((i % 8) * 3 + di) * C
                                       : ((i % 8) * 3 + di + 1) * C],
                                    start=(di == 0),
                                    stop=(di == 2),
                                )
                        dg = dgp.tile([C, 18 * GP], fp16, tag="dg")
                        dg_r[i] = dg
                        emit_builds(i, dg, range(9))
                        fbS = fbp.tile([C, 3 * C], fp16, tag="fbS")
                        nc.scalar.copy(fbS[:], psf[:])
                        fbS_r[i] = fbS
                        emit_builds(i, dg, range(9, 18))
                        if i % 8 == 7 or i == NG - 1:
                            ft_r.pop(i // 8)
                    # blockdiag matmuls (i-1): acc_m = sum_kp fbS_kp^T @ bd2
                    if 0 <= i - 1 < NG:
                        acc = psB2.tile([C, NB * GP], fp32, tag="accT", bufs=2,
                                        name="acc")
                        acc_r[i - 1] = acc
                        fbS = fbS_r.pop(i - 1)
                        dg = dg_r.pop(i - 1)
                        for m in range(NB):
                            for kp in range(3):
                                j = m * 3 + kp
                                nc.tensor.matmul(
                                    acc[:, m * GP : (m + 1) * GP],
                                    fbS[:, kp * C : (kp + 1) * C],
                                    dg[:, j * GP : (j + 1) * GP],
                                    start=(kp == 0),
                                    stop=(kp == 2),
                                )
                    # coef matmuls (i-2)
                    if 0 <= i - 2 < NG:
                        pso = psB2.tile([C, GP], fp32, tag="pso", bufs=2,
                                        name="pso")
                        pso_r[i - 2] = pso
                        boS = boS_r.pop(i - 2)
                        for m in range(NB):
                            nc.tensor.matmul(
                                pso[:],
                                coefT_sb[:, m * C : (m + 1) * C],
                                boS[:, m * GP : (m + 1) * GP],
                                start=(m == 0),
                                stop=(m == NB - 1),
                            )

    nc.compile()
    return nc


def _get_nc():
    if "nc" not in _CACHE:
        _CACHE["nc"] = build_nc()
    return _CACHE["nc"]


def _prep_maps(feat, weight, conv1_w, conv1_b, conv2_w, conv2_b, bases_buf, coef, bias):
    feat = np.asarray(feat, np.float32)
    weight = np.asarray(weight, np.float32)
    conv1_w = np.asarray(conv1_w, np.float32)
    conv2_w = np.asarray(conv2_w, np.float32)
    bases_buf = np.asarray(bases_buf, np.float32)
    coef = np.asarray(coef, np.float32)

    n = feat.shape[0]
    featp = np.zeros((n, C, HP, WP), np.float16)
    featp[:, :, 1 : H + 1, 1 : W + 1] = feat
    wgtp = np.zeros((n, CW, HP, WP), np.float16)
    wgtp[:, :, 1 : H + 1, 1 : W + 1] = weight

    # host-prepped transposed feature chunks per (group, di):
    # fTd[q, (g*3+di)*C + c] = fe[c, FOFF + g*GP + (di-1)*WP + q]
    fe = np.zeros((n, C, FEXT), np.float16)
    fe[:, :, FOFF : FOFF + NPAD] = featp.reshape(n, C, NPAD)
    fTdh = np.empty((n, 3 * NG, C, C), np.float16)
    for g in range(NG):
        for di in range(3):
            s0 = FOFF + g * GP + (di - 1) * WP
            fTdh[:, g * 3 + di] = fe[:, :, s0 : s0 + C].transpose(0, 2, 1)
    fTdh = np.ascontiguousarray(
        fTdh.transpose(0, 2, 1, 3).reshape(n, C, 3 * NG * C)
    )

    w1f = np.ascontiguousarray(
        conv1_w[:, :C].transpose(1, 2, 3, 0).reshape(C, L * C)
    ).astype(np.float16)
    w1w = np.ascontiguousarray(
        conv1_w[:, C:].transpose(1, 2, 3, 0).reshape(CW, L * C)
    ).astype(np.float16)
    # conv2 weights with k-major output-channel ordering (col = k*NB + m)
    w2h = np.ascontiguousarray(conv2_w[:, :, 0, 0].T).astype(np.float16)
    w2i = np.ascontiguousarray(
        w2h.reshape(C, NB, TEM).transpose(0, 2, 1).reshape(C, NBT)
    )
    # interleaved band blocks: bnd64[q, (kp*3+di)*C + ki*GP + p]
    #   = bases_buf[2*kp+ki, di*3 + (q-p)] for q-p in 0..2
    bndh = np.zeros((C, 3, 3, 2, GP), np.float32)
    for kp in range(3):
        for di in range(3):
            for ki in range(2):
                k = 2 * kp + ki
                for dj in range(3):
                    for p in range(GP):
                        bndh[p + dj, kp, di, ki, p] = bases_buf[k, di * 3 + dj]
    bndh = bndh.reshape(C, 9 * C).astype(np.float16)
    # mask2: ones at (p,p) and (GP+p, p)
    mask2 = np.zeros((C, GP), np.float16)
    for p in range(GP):
        mask2[p, p] = 1.0
        mask2[GP + p, p] = 1.0
    coefTh = np.ascontiguousarray(
        coef[:, :, 0, 0].reshape(C, C, NB).transpose(1, 2, 0).reshape(C, NB * C)
    ).astype(np.float16)
    b1h = np.asarray(conv1_b, np.float32).reshape(C, 1)
    b2h = np.asarray(conv2_b, np.float32).reshape(NB, TEM).T.reshape(NBT, 1)
    b3h = np.asarray(bias, np.float32).reshape(C, 1)

    wgtq = np.zeros((n, C, NPAD), np.float16)
    wgtq[:, :CW] = wgtp.reshape(n, CW, NPAD)
    wgtq[:, CW:, : NPAD - WP] = wgtp.reshape(n, CW, NPAD)[:, :, WP:]
    # paired weights: rows 0-63 = tap (0,j), rows 64-127 = tap (1,j)
    wgtq2 = np.zeros((n, C, NPAD), np.float16)
    wgtq2[:, :CW] = wgtp.reshape(n, CW, NPAD)
    wgtq2[:, CW:, : NPAD - 1] = wgtp.reshape(n, CW, NPAD)[:, :, 1:]
    w1wq2 = np.concatenate(
        [
            w1w.reshape(CW, 3, 3, C)[:, 2, 0],
            w1w.reshape(CW, 3, 3, C)[:, 2, 1],
        ],
        axis=0,
    ).reshape(C, C)
    w1wq = np.concatenate(
        [
            w1w.reshape(CW, 3, 3, C)[:, 0],
            w1w.reshape(CW, 3, 3, C)[:, 1],
        ],
        axis=0,
    ).reshape(C, 3 * C)
    pkw = np.concatenate([w1wq, w1wq2], axis=1)
    pk = np.concatenate([w2i, coefTh, bndh, mask2], axis=1)
    pb = np.zeros((C, 3), np.float32)
    pb[:, 0:1] = b1h
    pb[:, 1:2] = b3h
    pb[:NBT, 2:3] = b2h
    shared = {"w1f": w1f, "pkw": pkw, "pk": pk,
              "w1w": np.ascontiguousarray(w1w[:, 8 * C:]), "pb": pb}
    return [
        {
            "featp": featp[i].reshape(C, NPAD).astype(np.float16),
            "wgtq": wgtq[i], "wgtq2": wgtq2[i],
            "fTd": fTdh[i],
            **shared,
        }
        for i in range(n)
    ]


def kernel(feat, weight, conv1_w, conv1_b, conv2_w, conv2_b, bases_buf, coef, bias,
           **run_kwargs):
    in_maps = _prep_maps(
        feat, weight, conv1_w, conv1_b, conv2_w, conv2_b, bases_buf, coef, bias
    )
    res = run_bass_kernel_spmd(
        _get_nc(), in_maps, core_ids=list(range(len(in_maps))), **run_kwargs
    )
    outp = np.stack([r["out"] for r in res.results], 0).astype(np.float32)
    outp = outp[:, :, :NPAD].reshape(-1, C, HP, WP)[:, :, 1 : H + 1, 1 : W + 1]
    _CACHE["last_results"] = res
    return np.ascontiguousarray(outp)


# revision 24
# speedup vs baseline: 1.3419x; 1.0200x over previous
"""Trainium2 Bass kernel for the DCF (dynamic conv filter) module.

Sharding: pure data-parallel over batch N=8 across 8 NeuronCores (one image
per core); all parameters replicated.

Pipeline per core (one 128x96x96 image):
  A:  conv1 (3x3, 192->128) + tanh -> hmid;  conv2 (1x1, 128->36) + tanh -> b
  B:  per 64-pixel group g:
        - SV: double transpose of the b rows + strided half-copies give the
          per-pixel scales in (k-parity x 64px, k-pair*6+m) layout
        - F: fixed-basis convs of feat via k-pair-interleaved banded matmuls
          on host-prepped transposed feature chunks (fTd), 3 di accumulated;
          output partitions are (k-parity*64 + pixel)
        - acc_m = sum_kpair fbS_kpair^T @ blockdiag2(s) -- each matmul
          contracts 2 k's at once (k lives in the contraction dim), so the
          k-reduction costs half the rows of the per-(m,k) diag scheme
        - out_group = sum_m coef_m @ acc_m (+bias)

Block-diagonals are built as tensor_scalar(mask2 * s) (DVE 4x mode); acc and
pso are double-buffered in PSUM so evacuation copies never stall the PE."""

from itertools import product

import numpy as np

import concourse.bass as bass
import concourse.tile as tile
from concourse import bacc, mybir
from concourse.bass_utils import run_bass_kernel_spmd
from concourse.masks import make_identity

fp16 = mybir.dt.float16
fp32 = mybir.dt.float32

N_CORES = 8
C = 128
CW = 64
H = W = 96
HP = WP = 98
NPIX = H * W
NPAD = HP * WP  # 9604
NB = 6
TEM = 6
L = 9
NBT = NB * TEM  # 36
RT = 4
FT = RT * W  # 384
NT = H // RT  # 24
GP = 64           # output pixels per group
NG = 152          # groups (cover padded idx 1 .. 1+152*64 = 9729)
BP = 9732         # padded bsb/out length
FEXT = 10000      # extended (host-side) padded feat length for fTd windows
FOFF = 98         # fTd window base offset inside the extended buffer
OG = 8            # output groups per store
FCH8 = 8 * 3 * C  # fTd cols per streamed chunk (8 groups)
NCH = (NG + 7) // 8  # 19 chunks

# build engine mix for the 18 blockdiag builds per group: 12 DVE (86ns in 4x
# mode), 4 Pool, 2 Act
_B18 = ["D", "P", "D", "P", "D", "P", "D", "D", "D",
        "P", "P", "D", "D", "P", "D", "P", "D", "D"]

_CACHE = {}


def build_nc():
    nc = bacc.Bacc("TRN2", target_bir_lowering=False, debug=False)

    featp = nc.dram_tensor("featp", [C, NPAD], fp16, kind="ExternalInput").ap()
    wgtq = nc.dram_tensor("wgtq", [C, NPAD], fp16, kind="ExternalInput").ap()
    fTd = nc.dram_tensor("fTd", [C, 3 * NG * C], fp16, kind="ExternalInput").ap()
    wgtq2 = nc.dram_tensor("wgtq2", [C, NPAD], fp16, kind="ExternalInput").ap()
    w1f = nc.dram_tensor("w1f", [C, L * C], fp16, kind="ExternalInput").ap()
    # conv1 weight-branch params: w1wq|w1wq2 = 384+128
    pkw = nc.dram_tensor("pkw", [C, 512], fp16, kind="ExternalInput").ap()
    # fp16 params packed: w2|coefT|bnd64|mask2 = 36+768+1152+64
    pk = nc.dram_tensor("pk", [C, 2020], fp16, kind="ExternalInput").ap()
    w1w = nc.dram_tensor("w1w", [CW, C], fp16, kind="ExternalInput").ap()
    pb = nc.dram_tensor("pb", [C, 3], fp32, kind="ExternalInput").ap()
    out = nc.dram_tensor("out", [C, BP], fp16, kind="ExternalOutput").ap()

    Tanh = mybir.ActivationFunctionType.Tanh
    Ident = mybir.ActivationFunctionType.Identity
    Copy = mybir.ActivationFunctionType.Copy
    MUL = mybir.AluOpType.mult
    ADD = mybir.AluOpType.add

    with tile.TileContext(nc) as tc:
        with (
            tc.tile_pool(name="const", bufs=1) as const,
            tc.tile_pool(name="big", bufs=1) as big,
        ):
            featp_sb = big.tile([C, NPAD], fp16)
            wgtp_sb = big.tile([C, NPAD], fp16)
            cuts = [0, 2404, 4808, 7212, NPAD]
            # startup: tiny "head" pieces first so the first conv row-tile's
            # operands land within ~2.5us (DMA pipeline latency floor)
            HD = 686  # cols covering feat/wgt rows 0..6 (row-tile 0 + halo)
            w1f_sb = const.tile([C, L * C], fp16)
            nc.sync.dma_start(w1f_sb[:, : 4 * C], w1f[:, : 4 * C])
            nc.sync.dma_start(featp_sb[:, :HD], featp[:, :HD])
            nc.sync.dma_start(w1f_sb[:, 4 * C :], w1f[:, 4 * C :])
            pkw_sb = const.tile([C, 512], fp16)
            nc.sync.dma_start(pkw_sb[:], pkw)
            w1wq_sb = pkw_sb[:, 0:384]
            w1wq2_sb = pkw_sb[:, 384:512]
            wgtq2_sb = big.tile([C, NPAD], fp16)
            nc.sync.dma_start(wgtp_sb[:, :HD], wgtq[:, :HD])
            nc.sync.dma_start(wgtq2_sb[:, :HD], wgtq2[:, :HD])
            pb_sb = const.tile([C, 3], fp32)
            nc.sync.dma_start(pb_sb[:], pb)
            b1_sb = pb_sb[:, 0:1]
            b3_sb = pb_sb[:, 1:2]
            b2_sb = pb_sb[:NBT, 2:3]
            w1w_sb = const.tile([CW, C], fp16)
            nc.sync.dma_start(w1w_sb[:], w1w)
            nc.sync.dma_start(featp_sb[:, HD : cuts[1]], featp[:, HD : cuts[1]])
            pk_sb = const.tile([C, 2020], fp16)
            nc.sync.dma_start(pk_sb[:, 804:], pk[:, 804:])
            nc.sync.dma_start(pk_sb[:, :804], pk[:, :804])
            w2_sb = pk_sb[:, 0:36]
            coefT_sb = pk_sb[:, 36:804]
            bnd64_sb = pk_sb[:, 804:1956]
            mask2_sb = pk_sb[:, 1956:2020]
            nc.sync.dma_start(wgtq2_sb[:, HD : cuts[1]], wgtq2[:, HD : cuts[1]])
            nc.sync.dma_start(wgtp_sb[:, HD : cuts[2]], wgtq[:, HD : cuts[2]])
            nc.sync.dma_start(
                featp_sb[:, cuts[1] : cuts[2]], featp[:, cuts[1] : cuts[2]]
            )
            nc.sync.dma_start(wgtq2_sb[:, cuts[1] :], wgtq2[:, cuts[1] :])
            nc.sync.dma_start(
                featp_sb[:, cuts[2] : cuts[3]], featp[:, cuts[2] : cuts[3]]
            )
            nc.sync.dma_start(wgtp_sb[:, cuts[2] :], wgtq[:, cuts[2] :])
            nc.sync.dma_start(
                featp_sb[:, cuts[3] :], featp[:, cuts[3] :]
            )
            identNBT = const.tile([NBT, NBT], fp16)
            make_identity(nc, identNBT[:])

            bsb = big.tile([NBT, BP], fp16)
            # zero only the border/tail cells conv2 never writes (full memset
            # would hold Pool for 8us before the first b write)
            nc.gpsimd.memset(bsb[:, : WP + 2], 0.0)
            edge = bsb[:, 97 : 97 + 97 * WP].rearrange("c (r w) -> c r w", w=WP)
            nc.gpsimd.memset(edge[:, :, 0:2], 0.0)
            nc.gpsimd.memset(bsb[:, 97 * WP :], 0.0)

            b3d = bsb[:, :NPAD].rearrange("c (r w) -> c r w", w=WP)
            f3 = featp_sb[:].rearrange("c (r w) -> c r w", w=WP)
            w3 = wgtp_sb[:].rearrange("c (r w) -> c r w", w=WP)
            wq2 = wgtq2_sb[:].rearrange("c (r w) -> c r w", w=WP)

            # ---- fused pipeline: conv rows (phase A) stream in between the
            # software-pipelined per-group stages of phase B.
            with (
                tc.tile_pool(name="hmp", bufs=3) as hmp,
                tc.tile_pool(name="ftp", bufs=3) as ftp,
                tc.tile_pool(name="svsp", bufs=4) as svsp,
                tc.tile_pool(name="fbp", bufs=3) as fbp,
                tc.tile_pool(name="dgp", bufs=3) as dgp,
                tc.tile_pool(name="bop", bufs=3) as bop,
                tc.tile_pool(name="orp", bufs=2) as orp,
                tc.tile_pool(name="psB2", bufs=1, space="PSUM") as psB2,
            ):
                fbS_r, dg_r, boS_r, pso_r, acc_r = {}, {}, {}, {}, {}
                hm_r = {}
                svS_r, ft_r = {}, {}
                orow_bufs = {}

                def emit_arow_f(t):
                    r0 = t * RT
                    ps = psB2.tile([C, FT], fp32, tag="psA", bufs=1, name="ps")
                    for kk, (i, j) in enumerate(product(range(3), range(3))):
                        nc.tensor.matmul(
                            ps[:],
                            w1f_sb[:, (i * 3 + j) * C : (i * 3 + j + 1) * C],
                            f3[:, r0 + i : r0 + i + RT, j : j + W],
                            start=(kk == 0),
                            stop=False,
                        )
                    return ps

                def emit_arow_w(t, ps):
                    r0 = t * RT
                    for j in range(3):
                        nc.tensor.matmul(
                            ps[:],
                            w1wq_sb[:, j * C : (j + 1) * C],
                            w3[:, r0 : r0 + RT, j : j + W],
                            start=False,
                            stop=False,
                        )
                    nc.tensor.matmul(
                        ps[:],
                        w1wq2_sb,
                        wq2[:, r0 + 2 : r0 + 2 + RT, 0 : W],
                        start=False,
                        stop=False,
                    )
                    nc.tensor.matmul(
                        ps[:],
                        w1w_sb[:],
                        w3[:64, r0 + 2 : r0 + 2 + RT, 2 : 2 + W],
                        start=False,
                        stop=True,
                    )
                    hm = hmp.tile([C, FT], fp16, tag="hm")
                    nc.scalar.activation(hm[:], ps[:], Tanh, bias=b1_sb)
                    hm_r[t] = hm

                def emit_arow_b(t):
                    r0 = t * RT
                    hm = hm_r.pop(t)
                    ps2 = psB2.tile([NBT, FT], fp32, tag="psB", bufs=1, name="ps2")
                    nc.tensor.matmul(ps2[:], w2_sb, hm[:], start=True, stop=True)
                    nc.scalar.activation(
                        b3d[:, r0 + 1 : r0 + 1 + RT, 1 : 1 + W],
                        ps2[:].rearrange("c (r w) -> c r w", w=W),
                        Tanh,
                        bias=b2_sb,
                    )

                def emit_sv(g):
                    # b rows (k-major: row = k*6+m) -> per-pixel scales in
                    # (k-parity*64+p, kpair*6+m) layout via double transpose
                    # + strided half copies
                    svp = psB2.tile([C, NBT], fp32, tag="svp", bufs=1,
                                    name="svp")
                    src = bsb[:, 1 + g * GP : 1 + (g + 1) * GP]
                    nc.tensor.matmul(svp[:GP, :], src, identNBT[:],
                                     start=True, stop=True)
                    nc.tensor.matmul(svp[GP:, :], src, identNBT[:],
                                     start=True, stop=True)
                    svS = svsp.tile([C, 18], fp32, tag="svS")
                    svS_r[g] = svS
                    lo = svp[:GP, :].rearrange("p (kp ki m) -> p kp ki m",
                                               ki=2, m=NB)
                    hi = svp[GP:, :].rearrange("p (kp ki m) -> p kp ki m",
                                               ki=2, m=NB)
                    dlo = svS[:GP, :].rearrange("p (kp m) -> p kp m", m=NB)
                    dhi = svS[GP:, :].rearrange("p (kp m) -> p kp m", m=NB)
                    nc.vector.tensor_copy(dlo, lo[:, :, 0, :])
                    nc.vector.tensor_copy(dhi, hi[:, :, 1, :])

                def emit_builds(g, dg, js):
                    svS = svS_r[g]
                    for j in js:
                        m, kp = divmod(j, 3)
                        sc = svS[:, kp * NB + m : kp * NB + m + 1]
                        dslice = dg[:, j * GP : (j + 1) * GP]
                        eng = _B18[j]
                        if eng == "D":
                            nc.vector.tensor_scalar(
                                dslice, mask2_sb, sc, None, MUL
                            )
                        elif eng == "P":
                            nc.gpsimd.tensor_scalar(
                                dslice, mask2_sb, sc, None, MUL
                            )
                        else:
                            nc.scalar.activation(dslice, mask2_sb, Copy,
                                                 scale=sc)

                # conv row-tile slots spread at cadence ~6.5 so conv filler
                # lasts until iteration ~138 (row 23 deadline is ~141)
                _fs = {int(_r * 6.5 + 0.5): _r + 2 for _r in range(NT - 2)}
                _ws = {k + 1: v for k, v in _fs.items()}
                _bs = {k + 2: v for k, v in _fs.items()}

                for r in range(2):
                    emit_arow_w(r, emit_arow_f(r))
                    emit_arow_b(r)
                # fTd chunks 0,1 + SV for groups 0,1
                for cch in range(2):
                    ft = ftp.tile([C, FCH8], fp16, tag="ft")
                    ft_r[cch] = ft
                    nc.sync.dma_start(ft[:], fTd[:, cch * FCH8 : (cch + 1) * FCH8])
                emit_sv(0)
                emit_sv(1)

                for i in range(NG + 3):
                    if i in _fs:
                        arow_ps = emit_arow_f(_fs[i])
                    elif i in _ws:
                        emit_arow_w(_ws[i], arow_ps)
                    elif i in _bs:
                        emit_arow_b(_bs[i])
                    # acc(i-2) psum -> SBUF (Pool)
                    if 0 <= i - 2 < NG:
                        boS = bop.tile([C, NB * GP], fp16, tag="boS")
                        boS_r[i - 2] = boS
                        acc = acc_r.pop(i - 2)
                        nc.vector.tensor_copy(boS[:, : 3 * GP], acc[:, : 3 * GP])
                        nc.scalar.copy(boS[:, 3 * GP :], acc[:, 3 * GP :])
                    # orow(i-3) + store
                    if 0 <= i - 3 < NG:
                        j = i - 3
                        g8 = j % OG
                        if g8 == 0:
                            orow_bufs[j] = orp.tile(
                                [C, OG * GP], fp16, tag="orow", name="orow_buf"
                            )
                        ob = orow_bufs[j - g8]
                        nc.scalar.activation(
                            ob[:, g8 * GP : (g8 + 1) * GP], pso_r.pop(j),
                            Ident, bias=b3_sb,
                        )
                        if g8 == OG - 1 or j == NG - 1:
                            t0 = j - g8
                            nc.sync.dma_start(
                                out[:, 1 + t0 * GP : 1 + t0 * GP + (g8 + 1) * GP],
                                ob[:, : (g8 + 1) * GP],
                            )
                            del orow_bufs[t0]
                    # stream next fTd chunk
                    if i % 8 == 0 and i // 8 + 2 < NCH:
                        cch = i // 8 + 2
                        ft = ftp.tile([C, FCH8], fp16, tag="ft")
                        ft_r[cch] = ft
                        nc.sync.dma_start(
                            ft[:], fTd[:, cch * FCH8 : (cch + 1) * FCH8]
                        )
                    # per-pixel scales two groups ahead
                    if i + 2 < NG:
                        emit_sv(i + 2)
                    # F(i), blockdiag builds(i), F evac(i)
                    if i < NG:
                        psf = psB2.tile([C, 3 * C], fp32, tag="psf", bufs=1,
                                        name="psf")
                        ft = ft_r[i // 8]
                        for kp in range(3):
                            for di in range(3):
                                nc.tensor.matmul(
                                    psf[:, kp * C : (kp + 1) * C],
                                    bnd64_sb[:, (kp * 3 + di) * C
                                             : (kp * 3 + di + 1) * C],
                                    ft[:, ((i % 8) * 3 + di) * C
                                       : ((i % 8) * 3 + di + 1) * C],
                                    start=(di == 0),
                                    stop=(di == 2),
                                )
                        dg = dgp.tile([C, 18 * GP], fp16, tag="dg")
                        dg_r[i] = dg
                        emit_builds(i, dg, range(9))
                        fbS = fbp.tile([C, 3 * C], fp16, tag="fbS")
                        nc.scalar.copy(fbS[:], psf[:])
                        fbS_r[i] = fbS
                        emit_builds(i, dg, range(9, 18))
                        if i % 8 == 7 or i == NG - 1:
                            ft_r.pop(i // 8)
                    # blockdiag matmuls (i-1): acc_m = sum_kp fbS_kp^T @ bd2
                    if 0 <= i - 1 < NG:
                        acc = psB2.tile([C, NB * GP], fp32, tag="accT", bufs=2,
                                        name="acc")
                        acc_r[i - 1] = acc
                        fbS = fbS_r.pop(i - 1)
                        dg = dg_r.pop(i - 1)
                        for m in range(NB):
                            for kp in range(3):
                                j = m * 3 + kp
                                nc.tensor.matmul(
                                    acc[:, m * GP : (m + 1) * GP],
                                    fbS[:, kp * C : (kp + 1) * C],
                                    dg[:, j * GP : (j + 1) * GP],
                                    start=(kp == 0),
                                    stop=(kp == 2),
                                )
                    # coef matmuls (i-2)
                    if 0 <= i - 2 < NG:
                        pso = psB2.tile([C, GP], fp32, tag="pso", bufs=2,
                                        name="pso")
                        pso_r[i - 2] = pso
                        boS = boS_r.pop(i - 2)
                        for m in range(NB):
                            nc.tensor.matmul(
                                pso[:],
                                coefT_sb[:, m * C : (m + 1) * C],
                                boS[:, m * GP : (m + 1) * GP],
                                start=(m == 0),
                                stop=(m == NB - 1),
                            )

    nc.compile()
    return nc


def _get_nc():
    if "nc" not in _CACHE:
        _CACHE["nc"] = build_nc()
    return _CACHE["nc"]


def _prep_maps(feat, weight, conv1_w, conv1_b, conv2_w, conv2_b, bases_buf, coef, bias):
    feat = np.asarray(feat, np.float32)
    weight = np.asarray(weight, np.float32)
    conv1_w = np.asarray(conv1_w, np.float32)
    conv2_w = np.asarray(conv2_w, np.float32)
    bases_buf = np.asarray(bases_buf, np.float32)
    coef = np.asarray(coef, np.float32)

    n = feat.shape[0]
    featp = np.zeros((n, C, HP, WP), np.float16)
    featp[:, :, 1 : H + 1, 1 : W + 1] = feat
    wgtp = np.zeros((n, CW, HP, WP), np.float16)
    wgtp[:, :, 1 : H + 1, 1 : W + 1] = weight

    # host-prepped transposed feature chunks per (group, di):
    # fTd[q, (g*3+di)*C + c] = fe[c, FOFF + g*GP + (di-1)*WP + q]
    fe = np.zeros((n, C, FEXT), np.float16)
    fe[:, :, FOFF : FOFF + NPAD] = featp.reshape(n, C, NPAD)
    fTdh = np.empty((n, 3 * NG, C, C), np.float16)
    for g in range(NG):
        for di in range(3):
            s0 = FOFF + g * GP + (di - 1) * WP
            fTdh[:, g * 3 + di] = fe[:, :, s0 : s0 + C].transpose(0, 2, 1)
    fTdh = np.ascontiguousarray(
        fTdh.transpose(0, 2, 1, 3).reshape(n, C, 3 * NG * C)
    )

    w1f = np.ascontiguousarray(
        conv1_w[:, :C].transpose(1, 2, 3, 0).reshape(C, L * C)
    ).astype(np.float16)
    w1w = np.ascontiguousarray(
        conv1_w[:, C:].transpose(1, 2, 3, 0).reshape(CW, L * C)
    ).astype(np.float16)
    # conv2 weights with k-major output-channel ordering (col = k*NB + m)
    w2h = np.ascontiguousarray(conv2_w[:, :, 0, 0].T).astype(np.float16)
    w2i = np.ascontiguousarray(
        w2h.reshape(C, NB, TEM).transpose(0, 2, 1).reshape(C, NBT)
    )
    # interleaved band blocks: bnd64[q, (kp*3+di)*C + ki*GP + p]
    #   = bases_buf[2*kp+ki, di*3 + (q-p)] for q-p in 0..2
    bndh = np.zeros((C, 3, 3, 2, GP), np.float32)
    for kp in range(3):
        for di in range(3):
            for ki in range(2):
                k = 2 * kp + ki
                for dj in range(3):
                    for p in range(GP):
                        bndh[p + dj, kp, di, ki, p] = bases_buf[k, di * 3 + dj]
    bndh = bndh.reshape(C, 9 * C).astype(np.float16)
    # mask2: ones at (p,p) and (GP+p, p)
    mask2 = np.zeros((C, GP), np.float16)
    for p in range(GP):
        mask2[p, p] = 1.0
        mask2[GP + p, p] = 1.0
    coefTh = np.ascontiguousarray(
        coef[:, :, 0, 0].reshape(C, C, NB).transpose(1, 2, 0).reshape(C, NB * C)
    ).astype(np.float16)
    b1h = np.asarray(conv1_b, np.float32).reshape(C, 1)
    b2h = np.asarray(conv2_b, np.float32).reshape(NB, TEM).T.reshape(NBT, 1)
    b3h = np.asarray(bias, np.float32).reshape(C, 1)

    wgtq = np.zeros((n, C, NPAD), np.float16)
    wgtq[:, :CW] = wgtp.reshape(n, CW, NPAD)
    wgtq[:, CW:, : NPAD - WP] = wgtp.reshape(n, CW, NPAD)[:, :, WP:]
    # paired weights: rows 0-63 = tap (0,j), rows 64-127 = tap (1,j)
    wgtq2 = np.zeros((n, C, NPAD), np.float16)
    wgtq2[:, :CW] = wgtp.reshape(n, CW, NPAD)
    wgtq2[:, CW:, : NPAD - 1] = wgtp.reshape(n, CW, NPAD)[:, :, 1:]
    w1wq2 = np.concatenate(
        [
            w1w.reshape(CW, 3, 3, C)[:, 2, 0],
            w1w.reshape(CW, 3, 3, C)[:, 2, 1],
        ],
        axis=0,
    ).reshape(C, C)
    w1wq = np.concatenate(
        [
            w1w.reshape(CW, 3, 3, C)[:, 0],
            w1w.reshape(CW, 3, 3, C)[:, 1],
        ],
        axis=0,
    ).reshape(C, 3 * C)
    pkw = np.concatenate([w1wq, w1wq2], axis=1)
    pk = np.concatenate([w2i, coefTh, bndh, mask2], axis=1)
    pb = np.zeros((C, 3), np.float32)
    pb[:, 0:1] = b1h
    pb[:, 1:2] = b3h
    pb[:NBT, 2:3] = b2h
    shared = {"w1f": w1f, "pkw": pkw, "pk": pk,
              "w1w": np.ascontiguousarray(w1w[:, 8 * C:]), "pb": pb}
    return [
        {
            "featp": featp[i].reshape(C, NPAD).astype(np.float16),
            "wgtq": wgtq[i], "wgtq2": wgtq2[i],
            "fTd": fTdh[i],
            **shared,
        }
        for i in range(n)
    ]


def kernel(feat, weight, conv1_w, conv1_b, conv2_w, conv2_b, bases_buf, coef, bias,
           **run_kwargs):
    in_maps = _prep_maps(
        feat, weight, conv1_w, conv1_b, conv2_w, conv2_b, bases_buf, coef, bias
    )
    res = run_bass_kernel_spmd(
        _get_nc(), in_maps, core_ids=list(range(len(in_maps))), **run_kwargs
    )
    outp = np.stack([r["out"] for r in res.results], 0).astype(np.float32)
    outp = outp[:, :, :NPAD].reshape(-1, C, HP, WP)[:, :, 1 : H + 1, 1 : W + 1]
    _CACHE["last_results"] = res
    return np.ascontiguousarray(outp)


# revision 25
# speedup vs baseline: 1.3740x; 1.0239x over previous
"""Trainium2 Bass kernel for the DCF (dynamic conv filter) module.

Sharding: pure data-parallel over batch N=8 across 8 NeuronCores (one image
per core); all parameters replicated.

Pipeline per core (one 128x96x96 image):
  A:  conv1 (3x3, 192->128) + tanh -> hmid;  conv2 (1x1, 128->36) + tanh -> b
  B:  per 64-pixel group g:
        - SV: double transpose of the b rows + strided half-copies give the
          per-pixel scales in (k-parity x 64px, k-pair*6+m) layout
        - F: fixed-basis convs of feat via k-pair-interleaved banded matmuls
          on host-prepped transposed feature chunks (fTd), 3 di accumulated;
          output partitions are (k-parity*64 + pixel)
        - acc_m = sum_kpair fbS_kpair^T @ blockdiag2(s) -- each matmul
          contracts 2 k's at once (k lives in the contraction dim), so the
          k-reduction costs half the rows of the per-(m,k) diag scheme
        - out_group = sum_m coef_m @ acc_m (+bias)

Block-diagonals are built as tensor_scalar(mask2 * s) (DVE 4x mode); acc and
pso are double-buffered in PSUM so evacuation copies never stall the PE."""

from itertools import product

import numpy as np

import concourse.bass as bass
import concourse.tile as tile
from concourse import bacc, mybir
from concourse.bass_utils import run_bass_kernel_spmd
from concourse.masks import make_identity

fp16 = mybir.dt.float16
fp32 = mybir.dt.float32

N_CORES = 8
C = 128
CW = 64
H = W = 96
HP = WP = 98
NPIX = H * W
NPAD = HP * WP  # 9604
NB = 6
TEM = 6
L = 9
NBT = NB * TEM  # 36
RT = 4
FT = RT * W  # 384
NT = H // RT  # 24
GP = 64           # output pixels per group
NG = 152          # groups (cover padded idx 1 .. 1+152*64 = 9729)
BP = 9732         # padded bsb/out length
FEXT = 10000      # extended (host-side) padded feat length for fTd windows
FOFF = 98         # fTd window base offset inside the extended buffer
OG = 8            # output groups per store
FCH8 = 8 * 2 * C  # fTd cols per streamed chunk (8 groups)
NCH = (NG + 7) // 8  # 19 chunks

# build engine mix for the 18 blockdiag builds per group: 12 DVE (86ns in 4x
# mode), 4 Pool, 2 Act
_B18 = ["D", "P", "D", "P", "D", "P", "D", "D", "D",
        "P", "P", "D", "D", "P", "D", "P", "D", "D"]

_CACHE = {}


def build_nc():
    nc = bacc.Bacc("TRN2", target_bir_lowering=False, debug=False)

    featp = nc.dram_tensor("featp", [C, NPAD], fp16, kind="ExternalInput").ap()
    wgtq = nc.dram_tensor("wgtq", [C, NPAD], fp16, kind="ExternalInput").ap()
    fTd = nc.dram_tensor("fTd", [C, 2 * NG * C], fp16, kind="ExternalInput").ap()
    wgtq2 = nc.dram_tensor("wgtq2", [C, NPAD], fp16, kind="ExternalInput").ap()
    w1f = nc.dram_tensor("w1f", [C, L * C], fp16, kind="ExternalInput").ap()
    # conv1 weight-branch params: w1wq|w1wq2 = 384+128
    pkw = nc.dram_tensor("pkw", [C, 512], fp16, kind="ExternalInput").ap()
    # fp16 params packed: w2|coefT|bnd2|mask2 = 36+768+768+64
    pk = nc.dram_tensor("pk", [C, 1636], fp16, kind="ExternalInput").ap()
    w1w = nc.dram_tensor("w1w", [CW, C], fp16, kind="ExternalInput").ap()
    pb = nc.dram_tensor("pb", [C, 3], fp32, kind="ExternalInput").ap()
    out = nc.dram_tensor("out", [C, BP], fp16, kind="ExternalOutput").ap()

    Tanh = mybir.ActivationFunctionType.Tanh
    Ident = mybir.ActivationFunctionType.Identity
    Copy = mybir.ActivationFunctionType.Copy
    MUL = mybir.AluOpType.mult
    ADD = mybir.AluOpType.add

    with tile.TileContext(nc) as tc:
        with (
            tc.tile_pool(name="const", bufs=1) as const,
            tc.tile_pool(name="big", bufs=1) as big,
        ):
            featp_sb = big.tile([C, NPAD], fp16)
            wgtp_sb = big.tile([C, NPAD], fp16)
            cuts = [0, 2404, 4808, 7212, NPAD]
            # startup: tiny "head" pieces first so the first conv row-tile's
            # operands land within ~2.5us (DMA pipeline latency floor)
            HD = 686  # cols covering feat/wgt rows 0..6 (row-tile 0 + halo)
            w1f_sb = const.tile([C, L * C], fp16)
            nc.sync.dma_start(w1f_sb[:, : 4 * C], w1f[:, : 4 * C])
            nc.sync.dma_start(featp_sb[:, :HD], featp[:, :HD])
            nc.sync.dma_start(w1f_sb[:, 4 * C :], w1f[:, 4 * C :])
            pkw_sb = const.tile([C, 512], fp16)
            nc.sync.dma_start(pkw_sb[:], pkw)
            w1wq_sb = pkw_sb[:, 0:384]
            w1wq2_sb = pkw_sb[:, 384:512]
            wgtq2_sb = big.tile([C, NPAD], fp16)
            nc.sync.dma_start(wgtp_sb[:, :HD], wgtq[:, :HD])
            nc.sync.dma_start(wgtq2_sb[:, :HD], wgtq2[:, :HD])
            pb_sb = const.tile([C, 3], fp32)
            nc.sync.dma_start(pb_sb[:], pb)
            b1_sb = pb_sb[:, 0:1]
            b3_sb = pb_sb[:, 1:2]
            b2_sb = pb_sb[:NBT, 2:3]
            w1w_sb = const.tile([CW, C], fp16)
            nc.sync.dma_start(w1w_sb[:], w1w)
            nc.sync.dma_start(featp_sb[:, HD : cuts[1]], featp[:, HD : cuts[1]])
            pk_sb = const.tile([C, 1636], fp16)
            nc.sync.dma_start(pk_sb[:, 804:], pk[:, 804:])
            nc.sync.dma_start(pk_sb[:, :804], pk[:, :804])
            w2_sb = pk_sb[:, 0:36]
            coefT_sb = pk_sb[:, 36:804]
            bnd2_sb = pk_sb[:, 804:1572]
            mask2_sb = pk_sb[:, 1572:1636]
            nc.sync.dma_start(wgtq2_sb[:, HD : cuts[1]], wgtq2[:, HD : cuts[1]])
            nc.sync.dma_start(wgtp_sb[:, HD : cuts[2]], wgtq[:, HD : cuts[2]])
            nc.sync.dma_start(
                featp_sb[:, cuts[1] : cuts[2]], featp[:, cuts[1] : cuts[2]]
            )
            nc.sync.dma_start(wgtq2_sb[:, cuts[1] :], wgtq2[:, cuts[1] :])
            nc.sync.dma_start(
                featp_sb[:, cuts[2] : cuts[3]], featp[:, cuts[2] : cuts[3]]
            )
            nc.sync.dma_start(wgtp_sb[:, cuts[2] :], wgtq[:, cuts[2] :])
            nc.sync.dma_start(
                featp_sb[:, cuts[3] :], featp[:, cuts[3] :]
            )
            identNBT = const.tile([NBT, NBT], fp16)
            make_identity(nc, identNBT[:])

            bsb = big.tile([NBT, BP], fp16)
            # zero only the border/tail cells conv2 never writes (full memset
            # would hold Pool for 8us before the first b write)
            nc.gpsimd.memset(bsb[:, : WP + 2], 0.0)
            edge = bsb[:, 97 : 97 + 97 * WP].rearrange("c (r w) -> c r w", w=WP)
            nc.gpsimd.memset(edge[:, :, 0:2], 0.0)
            nc.gpsimd.memset(bsb[:, 97 * WP :], 0.0)

            b3d = bsb[:, :NPAD].rearrange("c (r w) -> c r w", w=WP)
            f3 = featp_sb[:].rearrange("c (r w) -> c r w", w=WP)
            w3 = wgtp_sb[:].rearrange("c (r w) -> c r w", w=WP)
            wq2 = wgtq2_sb[:].rearrange("c (r w) -> c r w", w=WP)

            # ---- fused pipeline: conv rows (phase A) stream in between the
            # software-pipelined per-group stages of phase B.
            with (
                tc.tile_pool(name="hmp", bufs=3) as hmp,
                tc.tile_pool(name="ftp", bufs=3) as ftp,
                tc.tile_pool(name="svsp", bufs=4) as svsp,
                tc.tile_pool(name="fbp", bufs=3) as fbp,
                tc.tile_pool(name="dgp", bufs=3) as dgp,
                tc.tile_pool(name="bop", bufs=3) as bop,
                tc.tile_pool(name="orp", bufs=2) as orp,
                tc.tile_pool(name="psB2", bufs=1, space="PSUM") as psB2,
            ):
                fbS_r, dg_r, boS_r, pso_r, acc_r = {}, {}, {}, {}, {}
                hm_r = {}
                svS_r, ft_r = {}, {}
                orow_bufs = {}

                def emit_arow_f(t):
                    r0 = t * RT
                    ps = psB2.tile([C, FT], fp32, tag="psA", bufs=1, name="ps")
                    for kk, (i, j) in enumerate(product(range(3), range(3))):
                        nc.tensor.matmul(
                            ps[:],
                            w1f_sb[:, (i * 3 + j) * C : (i * 3 + j + 1) * C],
                            f3[:, r0 + i : r0 + i + RT, j : j + W],
                            start=(kk == 0),
                            stop=False,
                        )
                    return ps

                def emit_arow_w(t, ps):
                    r0 = t * RT
                    for j in range(3):
                        nc.tensor.matmul(
                            ps[:],
                            w1wq_sb[:, j * C : (j + 1) * C],
                            w3[:, r0 : r0 + RT, j : j + W],
                            start=False,
                            stop=False,
                        )
                    nc.tensor.matmul(
                        ps[:],
                        w1wq2_sb,
                        wq2[:, r0 + 2 : r0 + 2 + RT, 0 : W],
                        start=False,
                        stop=False,
                    )
                    nc.tensor.matmul(
                        ps[:],
                        w1w_sb[:],
                        w3[:64, r0 + 2 : r0 + 2 + RT, 2 : 2 + W],
                        start=False,
                        stop=True,
                    )
                    hm = hmp.tile([C, FT], fp16, tag="hm")
                    nc.scalar.activation(hm[:], ps[:], Tanh, bias=b1_sb)
                    hm_r[t] = hm

                def emit_arow_b(t):
                    r0 = t * RT
                    hm = hm_r.pop(t)
                    ps2 = psB2.tile([NBT, FT], fp32, tag="psB", bufs=1, name="ps2")
                    nc.tensor.matmul(ps2[:], w2_sb, hm[:], start=True, stop=True)
                    nc.scalar.activation(
                        b3d[:, r0 + 1 : r0 + 1 + RT, 1 : 1 + W],
                        ps2[:].rearrange("c (r w) -> c r w", w=W),
                        Tanh,
                        bias=b2_sb,
                    )

                def emit_sv(g):
                    # b rows (k-major: row = k*6+m) -> per-pixel scales in
                    # (k-parity*64+p, kpair*6+m) layout via double transpose
                    # + strided half copies
                    svp = psB2.tile([C, NBT], fp32, tag="svp", bufs=1,
                                    name="svp")
                    src = bsb[:, 1 + g * GP : 1 + (g + 1) * GP]
                    nc.tensor.matmul(svp[:GP, :], src, identNBT[:],
                                     start=True, stop=True)
                    nc.tensor.matmul(svp[GP:, :], src, identNBT[:],
                                     start=True, stop=True)
                    svS = svsp.tile([C, 18], fp32, tag="svS")
                    svS_r[g] = svS
                    lo = svp[:GP, :].rearrange("p (kp ki m) -> p kp ki m",
                                               ki=2, m=NB)
                    hi = svp[GP:, :].rearrange("p (kp ki m) -> p kp ki m",
                                               ki=2, m=NB)
                    dlo = svS[:GP, :].rearrange("p (kp m) -> p kp m", m=NB)
                    dhi = svS[GP:, :].rearrange("p (kp m) -> p kp m", m=NB)
                    nc.vector.tensor_copy(dlo, lo[:, :, 0, :])
                    nc.vector.tensor_copy(dhi, hi[:, :, 1, :])

                def emit_builds(g, dg, js):
                    svS = svS_r[g]
                    for j in js:
                        m, kp = divmod(j, 3)
                        sc = svS[:, kp * NB + m : kp * NB + m + 1]
                        dslice = dg[:, j * GP : (j + 1) * GP]
                        eng = _B18[j]
                        if eng == "D":
                            nc.vector.tensor_scalar(
                                dslice, mask2_sb, sc, None, MUL
                            )
                        elif eng == "P":
                            nc.gpsimd.tensor_scalar(
                                dslice, mask2_sb, sc, None, MUL
                            )
                        else:
                            nc.scalar.activation(dslice, mask2_sb, Copy,
                                                 scale=sc)

                # conv row-tile slots spread at cadence ~6.5 so conv filler
                # lasts until iteration ~138 (row 23 deadline is ~141)
                _fs = {int(_r * 6.5 + 0.5): _r + 2 for _r in range(NT - 2)}
                _ws = {k + 1: v for k, v in _fs.items()}
                _bs = {k + 2: v for k, v in _fs.items()}

                for r in range(2):
                    emit_arow_w(r, emit_arow_f(r))
                    emit_arow_b(r)
                # fTd chunks 0,1 + SV for groups 0,1
                for cch in range(2):
                    ft = ftp.tile([C, FCH8], fp16, tag="ft")
                    ft_r[cch] = ft
                    nc.sync.dma_start(ft[:], fTd[:, cch * FCH8 : (cch + 1) * FCH8])
                emit_sv(0)
                emit_sv(1)

                for i in range(NG + 3):
                    if i in _fs:
                        arow_ps = emit_arow_f(_fs[i])
                    elif i in _ws:
                        emit_arow_w(_ws[i], arow_ps)
                    elif i in _bs:
                        emit_arow_b(_bs[i])
                    # acc(i-2) psum -> SBUF (Pool)
                    if 0 <= i - 2 < NG:
                        boS = bop.tile([C, NB * GP], fp16, tag="boS")
                        boS_r[i - 2] = boS
                        acc = acc_r.pop(i - 2)
                        nc.vector.tensor_copy(boS[:, : 3 * GP], acc[:, : 3 * GP])
                        nc.scalar.copy(boS[:, 3 * GP :], acc[:, 3 * GP :])
                    # orow(i-3) + store
                    if 0 <= i - 3 < NG:
                        j = i - 3
                        g8 = j % OG
                        if g8 == 0:
                            orow_bufs[j] = orp.tile(
                                [C, OG * GP], fp16, tag="orow", name="orow_buf"
                            )
                        ob = orow_bufs[j - g8]
                        nc.scalar.activation(
                            ob[:, g8 * GP : (g8 + 1) * GP], pso_r.pop(j),
                            Ident, bias=b3_sb,
                        )
                        if g8 == OG - 1 or j == NG - 1:
                            t0 = j - g8
                            nc.sync.dma_start(
                                out[:, 1 + t0 * GP : 1 + t0 * GP + (g8 + 1) * GP],
                                ob[:, : (g8 + 1) * GP],
                            )
                            del orow_bufs[t0]
                    # stream next fTd chunk
                    if i % 8 == 0 and i // 8 + 2 < NCH:
                        cch = i // 8 + 2
                        ft = ftp.tile([C, FCH8], fp16, tag="ft")
                        ft_r[cch] = ft
                        nc.sync.dma_start(
                            ft[:], fTd[:, cch * FCH8 : (cch + 1) * FCH8]
                        )
                    # per-pixel scales two groups ahead
                    if i + 2 < NG:
                        emit_sv(i + 2)
                    # F(i), blockdiag builds(i), F evac(i)
                    if i < NG:
                        psf = psB2.tile([C, 3 * C], fp32, tag="psf", bufs=1,
                                        name="psf")
                        ft = ft_r[i // 8]
                        for kp in range(3):
                            for ci in range(2):
                                nc.tensor.matmul(
                                    psf[:, kp * C : (kp + 1) * C],
                                    bnd2_sb[:, (kp * 2 + ci) * C
                                            : (kp * 2 + ci + 1) * C],
                                    ft[:, ((i % 8) * 2 + ci) * C
                                       : ((i % 8) * 2 + ci + 1) * C],
                                    start=(ci == 0),
                                    stop=(ci == 1),
                                )
                        dg = dgp.tile([C, 18 * GP], fp16, tag="dg")
                        dg_r[i] = dg
                        emit_builds(i, dg, range(9))
                        fbS = fbp.tile([C, 3 * C], fp16, tag="fbS")
                        nc.scalar.copy(fbS[:], psf[:])
                        fbS_r[i] = fbS
                        emit_builds(i, dg, range(9, 18))
                        if i % 8 == 7 or i == NG - 1:
                            ft_r.pop(i // 8)
                    # blockdiag matmuls (i-1): acc_m = sum_kp fbS_kp^T @ bd2
                    if 0 <= i - 1 < NG:
                        acc = psB2.tile([C, NB * GP], fp32, tag="accT", bufs=2,
                                        name="acc")
                        acc_r[i - 1] = acc
                        fbS = fbS_r.pop(i - 1)
                        dg = dg_r.pop(i - 1)
                        for m in range(NB):
                            for kp in range(3):
                                j = m * 3 + kp
                                nc.tensor.matmul(
                                    acc[:, m * GP : (m + 1) * GP],
                                    fbS[:, kp * C : (kp + 1) * C],
                                    dg[:, j * GP : (j + 1) * GP],
                                    start=(kp == 0),
                                    stop=(kp == 2),
                                )
                    # coef matmuls (i-2)
                    if 0 <= i - 2 < NG:
                        pso = psB2.tile([C, GP], fp32, tag="pso", bufs=2,
                                        name="pso")
                        pso_r[i - 2] = pso
                        boS = boS_r.pop(i - 2)
                        for m in range(NB):
                            nc.tensor.matmul(
                                pso[:],
                                coefT_sb[:, m * C : (m + 1) * C],
                                boS[:, m * GP : (m + 1) * GP],
                                start=(m == 0),
                                stop=(m == NB - 1),
                            )

    nc.compile()
    return nc


def _get_nc():
    if "nc" not in _CACHE:
        _CACHE["nc"] = build_nc()
    return _CACHE["nc"]


def _prep_maps(feat, weight, conv1_w, conv1_b, conv2_w, conv2_b, bases_buf, coef, bias):
    feat = np.asarray(feat, np.float32)
    weight = np.asarray(weight, np.float32)
    conv1_w = np.asarray(conv1_w, np.float32)
    conv2_w = np.asarray(conv2_w, np.float32)
    bases_buf = np.asarray(bases_buf, np.float32)
    coef = np.asarray(coef, np.float32)

    n = feat.shape[0]
    featp = np.zeros((n, C, HP, WP), np.float16)
    featp[:, :, 1 : H + 1, 1 : W + 1] = feat
    wgtp = np.zeros((n, CW, HP, WP), np.float16)
    wgtp[:, :, 1 : H + 1, 1 : W + 1] = weight

    # host-prepped transposed feature chunks: the 3 disjoint di input
    # windows (66 px each) packed into 2 chunks of 128 rows per group
    # fTd[q, (g*2+ci)*C + c] = fe[c, FOFF + 1 + g*GP + RELS[ci][q]]
    rels1 = list(range(-99, -33)) + list(range(-1, 61))
    rels2 = list(range(61, 65)) + list(range(97, 163))
    rels2 = rels2 + [163] * (C - len(rels2))
    RELS = [np.array(rels1), np.array(rels2)]
    fe = np.zeros((n, C, FEXT), np.float16)
    fe[:, :, FOFF : FOFF + NPAD] = featp.reshape(n, C, NPAD)
    fTdh = np.empty((n, 2 * NG, C, C), np.float16)
    for g in range(NG):
        for ci in range(2):
            idx = FOFF + 1 + g * GP + RELS[ci]
            fTdh[:, g * 2 + ci] = fe[:, :, idx].transpose(0, 2, 1)
    fTdh = np.ascontiguousarray(
        fTdh.transpose(0, 2, 1, 3).reshape(n, C, 2 * NG * C)
    )

    w1f = np.ascontiguousarray(
        conv1_w[:, :C].transpose(1, 2, 3, 0).reshape(C, L * C)
    ).astype(np.float16)
    w1w = np.ascontiguousarray(
        conv1_w[:, C:].transpose(1, 2, 3, 0).reshape(CW, L * C)
    ).astype(np.float16)
    # conv2 weights with k-major output-channel ordering (col = k*NB + m)
    w2h = np.ascontiguousarray(conv2_w[:, :, 0, 0].T).astype(np.float16)
    w2i = np.ascontiguousarray(
        w2h.reshape(C, NB, TEM).transpose(0, 2, 1).reshape(C, NBT)
    )
    # packed band blocks: bnd2[q, (kp*2+ci)*C + ki*GP + p] accumulates
    # bases_buf[2*kp+ki, di*3+dj] where chunk ci row q holds input offset
    # rel = p + (di-1)*WP + dj - 1
    pos = []
    for rels in RELS:
        d = {}
        for q, r in enumerate(rels.tolist()):
            if r not in d:
                d[r] = q
        pos.append(d)
    bndh = np.zeros((C, 3, 2, 2, GP), np.float32)
    for kp in range(3):
        for ki in range(2):
            k = 2 * kp + ki
            for di in range(3):
                for dj in range(3):
                    for p in range(GP):
                        rel = p + (di - 1) * WP + dj - 1
                        ci = 0 if rel in pos[0] else 1
                        bndh[pos[ci][rel], kp, ci, ki, p] += \
                            bases_buf[k, di * 3 + dj]
    bndh = bndh.reshape(C, 6 * C).astype(np.float16)
    # mask2: ones at (p,p) and (GP+p, p)
    mask2 = np.zeros((C, GP), np.float16)
    for p in range(GP):
        mask2[p, p] = 1.0
        mask2[GP + p, p] = 1.0
    coefTh = np.ascontiguousarray(
        coef[:, :, 0, 0].reshape(C, C, NB).transpose(1, 2, 0).reshape(C, NB * C)
    ).astype(np.float16)
    b1h = np.asarray(conv1_b, np.float32).reshape(C, 1)
    b2h = np.asarray(conv2_b, np.float32).reshape(NB, TEM).T.reshape(NBT, 1)
    b3h = np.asarray(bias, np.float32).reshape(C, 1)

    wgtq = np.zeros((n, C, NPAD), np.float16)
    wgtq[:, :CW] = wgtp.reshape(n, CW, NPAD)
    wgtq[:, CW:, : NPAD - WP] = wgtp.reshape(n, CW, NPAD)[:, :, WP:]
    # paired weights: rows 0-63 = tap (0,j), rows 64-127 = tap (1,j)
    wgtq2 = np.zeros((n, C, NPAD), np.float16)
    wgtq2[:, :CW] = wgtp.reshape(n, CW, NPAD)
    wgtq2[:, CW:, : NPAD - 1] = wgtp.reshape(n, CW, NPAD)[:, :, 1:]
    w1wq2 = np.concatenate(
        [
            w1w.reshape(CW, 3, 3, C)[:, 2, 0],
            w1w.reshape(CW, 3, 3, C)[:, 2, 1],
        ],
        axis=0,
    ).reshape(C, C)
    w1wq = np.concatenate(
        [
            w1w.reshape(CW, 3, 3, C)[:, 0],
            w1w.reshape(CW, 3, 3, C)[:, 1],
        ],
        axis=0,
    ).reshape(C, 3 * C)
    pkw = np.concatenate([w1wq, w1wq2], axis=1)
    pk = np.concatenate([w2i, coefTh, bndh, mask2], axis=1)
    pb = np.zeros((C, 3), np.float32)
    pb[:, 0:1] = b1h
    pb[:, 1:2] = b3h
    pb[:NBT, 2:3] = b2h
    shared = {"w1f": w1f, "pkw": pkw, "pk": pk,
              "w1w": np.ascontiguousarray(w1w[:, 8 * C:]), "pb": pb}
    return [
        {
            "featp": featp[i].reshape(C, NPAD).astype(np.float16),
            "wgtq": wgtq[i], "wgtq2": wgtq2[i],
            "fTd": fTdh[i],
            **shared,
        }
        for i in range(n)
    ]


def kernel(feat, weight, conv1_w, conv1_b, conv2_w, conv2_b, bases_buf, coef, bias,
           **run_kwargs):
    in_maps = _prep_maps(
        feat, weight, conv1_w, conv1_b, conv2_w, conv2_b, bases_buf, coef, bias
    )
    res = run_bass_kernel_spmd(
        _get_nc(), in_maps, core_ids=list(range(len(in_maps))), **run_kwargs
    )
    outp = np.stack([r["out"] for r in res.results], 0).astype(np.float32)
    outp = outp[:, :, :NPAD].reshape(-1, C, HP, WP)[:, :, 1 : H + 1, 1 : W + 1]
    _CACHE["last_results"] = res
    return np.ascontiguousarray(outp)


# revision 26
# speedup vs baseline: 1.3907x; 1.0121x over previous
"""Trainium2 Bass kernel for the DCF (dynamic conv filter) module.

Sharding: pure data-parallel over batch N=8 across 8 NeuronCores (one image
per core); all parameters replicated.

Pipeline per core (one 128x96x96 image):
  A:  conv1 (3x3, 192->128) + tanh -> hmid;  conv2 (1x1, 128->36) + tanh -> b
  B:  per 64-pixel group g:
        - SV: double transpose of the b rows + strided half-copies give the
          per-pixel scales in (k-parity x 64px, k-pair*6+m) layout
        - F: fixed-basis convs of feat via k-pair-interleaved banded matmuls
          on host-prepped transposed feature chunks (fTd), 3 di accumulated;
          output partitions are (k-parity*64 + pixel)
        - acc_m = sum_kpair fbS_kpair^T @ blockdiag2(s) -- each matmul
          contracts 2 k's at once (k lives in the contraction dim), so the
          k-reduction costs half the rows of the per-(m,k) diag scheme
        - out_group = sum_m coef_m @ acc_m (+bias)

Block-diagonals are built as tensor_scalar(mask2 * s) (DVE 4x mode); acc and
pso are double-buffered in PSUM so evacuation copies never stall the PE."""

from itertools import product

import numpy as np

import concourse.bass as bass
import concourse.tile as tile
from concourse import bacc, mybir
from concourse.bass_utils import run_bass_kernel_spmd
from concourse.masks import make_identity

fp16 = mybir.dt.float16
fp32 = mybir.dt.float32

N_CORES = 8
C = 128
CW = 64
H = W = 96
HP = WP = 98
NPIX = H * W
NPAD = HP * WP  # 9604
NB = 6
TEM = 6
L = 9
NBT = NB * TEM  # 36
RT = 4
FT = RT * W  # 384
NT = H // RT  # 24
GP = 64           # output pixels per group
NG = 152          # groups (cover padded idx 1 .. 1+152*64 = 9729)
BP = 9732         # padded bsb/out length
FEXT = 10000      # extended (host-side) padded feat length for fTd windows
FOFF = 98         # fTd window base offset inside the extended buffer
OG = 8            # output groups per store
FCH8 = 8 * 2 * C  # fTd cols per streamed chunk (8 groups)
NCH = (NG + 7) // 8  # 19 chunks

# build engine mix for the 18 blockdiag builds per group: 12 DVE (86ns in 4x
# mode), 4 Pool, 2 Act
_B18 = ["D", "P", "D", "P", "D", "P", "D", "D", "D",
        "P", "P", "D", "D", "P", "D", "P", "D", "D"]

_CACHE = {}


def build_nc():
    nc = bacc.Bacc("TRN2", target_bir_lowering=False, debug=False)

    featp = nc.dram_tensor("featp", [C, NPAD], fp16, kind="ExternalInput").ap()
    wgtq = nc.dram_tensor("wgtq", [C, NPAD], fp16, kind="ExternalInput").ap()
    fTd = nc.dram_tensor("fTd", [C, 2 * NG * C], fp16, kind="ExternalInput").ap()
    wgtq2 = nc.dram_tensor("wgtq2", [C, NPAD], fp16, kind="ExternalInput").ap()
    w1f = nc.dram_tensor("w1f", [C, L * C], fp16, kind="ExternalInput").ap()
    # conv1 weight-branch params: w1wq|w1wq2 = 384+128
    pkw = nc.dram_tensor("pkw", [C, 512], fp16, kind="ExternalInput").ap()
    # fp16 params packed: w2|coefT|bnd2|mask2|P1|P2 = 36+768+768+64+18+18
    pk = nc.dram_tensor("pk", [C, 1672], fp16, kind="ExternalInput").ap()
    w1w = nc.dram_tensor("w1w", [CW, C], fp16, kind="ExternalInput").ap()
    pb = nc.dram_tensor("pb", [C, 3], fp32, kind="ExternalInput").ap()
    out = nc.dram_tensor("out", [C, BP], fp16, kind="ExternalOutput").ap()

    Tanh = mybir.ActivationFunctionType.Tanh
    Ident = mybir.ActivationFunctionType.Identity
    Copy = mybir.ActivationFunctionType.Copy
    MUL = mybir.AluOpType.mult
    ADD = mybir.AluOpType.add

    with tile.TileContext(nc) as tc:
        with (
            tc.tile_pool(name="const", bufs=1) as const,
            tc.tile_pool(name="big", bufs=1) as big,
        ):
            featp_sb = big.tile([C, NPAD], fp16)
            wgtp_sb = big.tile([C, NPAD], fp16)
            cuts = [0, 2404, 4808, 7212, NPAD]
            # startup: tiny "head" pieces first so the first conv row-tile's
            # operands land within ~2.5us (DMA pipeline latency floor)
            HD = 686  # cols covering feat/wgt rows 0..6 (row-tile 0 + halo)
            w1f_sb = const.tile([C, L * C], fp16)
            nc.sync.dma_start(w1f_sb[:, : 4 * C], w1f[:, : 4 * C])
            nc.sync.dma_start(featp_sb[:, :HD], featp[:, :HD])
            nc.sync.dma_start(w1f_sb[:, 4 * C :], w1f[:, 4 * C :])
            pkw_sb = const.tile([C, 512], fp16)
            nc.sync.dma_start(pkw_sb[:], pkw)
            w1wq_sb = pkw_sb[:, 0:384]
            w1wq2_sb = pkw_sb[:, 384:512]
            wgtq2_sb = big.tile([C, NPAD], fp16)
            nc.sync.dma_start(wgtp_sb[:, :HD], wgtq[:, :HD])
            nc.sync.dma_start(wgtq2_sb[:, :HD], wgtq2[:, :HD])
            pb_sb = const.tile([C, 3], fp32)
            nc.sync.dma_start(pb_sb[:], pb)
            b1_sb = pb_sb[:, 0:1]
            b3_sb = pb_sb[:, 1:2]
            b2_sb = pb_sb[:NBT, 2:3]
            w1w_sb = const.tile([CW, C], fp16)
            nc.sync.dma_start(w1w_sb[:], w1w)
            nc.sync.dma_start(featp_sb[:, HD : cuts[1]], featp[:, HD : cuts[1]])
            pk_sb = const.tile([C, 1672], fp16)
            nc.sync.dma_start(pk_sb[:, 804:], pk[:, 804:])
            nc.sync.dma_start(pk_sb[:, :804], pk[:, :804])
            w2_sb = pk_sb[:, 0:36]
            coefT_sb = pk_sb[:, 36:804]
            bnd2_sb = pk_sb[:, 804:1572]
            mask2_sb = pk_sb[:, 1572:1636]
            p1_sb = pk_sb[:NBT, 1636:1654]
            p2_sb = pk_sb[:NBT, 1654:1672]
            nc.sync.dma_start(wgtq2_sb[:, HD : cuts[1]], wgtq2[:, HD : cuts[1]])
            nc.sync.dma_start(wgtp_sb[:, HD : cuts[2]], wgtq[:, HD : cuts[2]])
            nc.sync.dma_start(
                featp_sb[:, cuts[1] : cuts[2]], featp[:, cuts[1] : cuts[2]]
            )
            nc.sync.dma_start(wgtq2_sb[:, cuts[1] :], wgtq2[:, cuts[1] :])
            nc.sync.dma_start(
                featp_sb[:, cuts[2] : cuts[3]], featp[:, cuts[2] : cuts[3]]
            )
            nc.sync.dma_start(wgtp_sb[:, cuts[2] :], wgtq[:, cuts[2] :])
            nc.sync.dma_start(
                featp_sb[:, cuts[3] :], featp[:, cuts[3] :]
            )
            bsb = big.tile([NBT, BP], fp16)
            # zero only the border/tail cells conv2 never writes (full memset
            # would hold Pool for 8us before the first b write)
            nc.gpsimd.memset(bsb[:, : WP + 2], 0.0)
            edge = bsb[:, 97 : 97 + 97 * WP].rearrange("c (r w) -> c r w", w=WP)
            nc.gpsimd.memset(edge[:, :, 0:2], 0.0)
            nc.gpsimd.memset(bsb[:, 97 * WP :], 0.0)

            b3d = bsb[:, :NPAD].rearrange("c (r w) -> c r w", w=WP)
            f3 = featp_sb[:].rearrange("c (r w) -> c r w", w=WP)
            w3 = wgtp_sb[:].rearrange("c (r w) -> c r w", w=WP)
            wq2 = wgtq2_sb[:].rearrange("c (r w) -> c r w", w=WP)

            # ---- fused pipeline: conv rows (phase A) stream in between the
            # software-pipelined per-group stages of phase B.
            with (
                tc.tile_pool(name="hmp", bufs=3) as hmp,
                tc.tile_pool(name="ftp", bufs=3) as ftp,
                tc.tile_pool(name="svsp", bufs=4) as svsp,
                tc.tile_pool(name="fbp", bufs=3) as fbp,
                tc.tile_pool(name="dgp", bufs=3) as dgp,
                tc.tile_pool(name="bop", bufs=3) as bop,
                tc.tile_pool(name="orp", bufs=2) as orp,
                tc.tile_pool(name="psB2", bufs=1, space="PSUM") as psB2,
            ):
                fbS_r, dg_r, boS_r, pso_r, acc_r = {}, {}, {}, {}, {}
                hm_r = {}
                svS_r, ft_r = {}, {}
                orow_bufs = {}

                def emit_arow_f(t):
                    r0 = t * RT
                    ps = psB2.tile([C, FT], fp32, tag="psA", bufs=1, name="ps")
                    for kk, (i, j) in enumerate(product(range(3), range(3))):
                        nc.tensor.matmul(
                            ps[:],
                            w1f_sb[:, (i * 3 + j) * C : (i * 3 + j + 1) * C],
                            f3[:, r0 + i : r0 + i + RT, j : j + W],
                            start=(kk == 0),
                            stop=False,
                        )
                    return ps

                def emit_arow_w(t, ps):
                    r0 = t * RT
                    for j in range(3):
                        nc.tensor.matmul(
                            ps[:],
                            w1wq_sb[:, j * C : (j + 1) * C],
                            w3[:, r0 : r0 + RT, j : j + W],
                            start=False,
                            stop=False,
                        )
                    nc.tensor.matmul(
                        ps[:],
                        w1wq2_sb,
                        wq2[:, r0 + 2 : r0 + 2 + RT, 0 : W],
                        start=False,
                        stop=False,
                    )
                    nc.tensor.matmul(
                        ps[:],
                        w1w_sb[:],
                        w3[:64, r0 + 2 : r0 + 2 + RT, 2 : 2 + W],
                        start=False,
                        stop=True,
                    )
                    hm = hmp.tile([C, FT], fp16, tag="hm")
                    nc.scalar.activation(hm[:], ps[:], Tanh, bias=b1_sb)
                    hm_r[t] = hm

                def emit_arow_b(t):
                    r0 = t * RT
                    hm = hm_r.pop(t)
                    ps2 = psB2.tile([NBT, FT], fp32, tag="psB", bufs=1, name="ps2")
                    nc.tensor.matmul(ps2[:], w2_sb, hm[:], start=True, stop=True)
                    nc.scalar.activation(
                        b3d[:, r0 + 1 : r0 + 1 + RT, 1 : 1 + W],
                        ps2[:].rearrange("c (r w) -> c r w", w=W),
                        Tanh,
                        bias=b2_sb,
                    )

                def emit_sv(g):
                    # b rows (k-major: row = k*6+m) -> per-pixel scales in
                    # (k-parity*64+p, kpair*6+m) layout: the two transposes
                    # use column-selector matrices (even/odd k) so svp lands
                    # pre-interleaved; one contiguous copy evacuates it
                    svp = psB2.tile([C, 18], fp32, tag="svp", bufs=1,
                                    name="svp")
                    src = bsb[:, 1 + g * GP : 1 + (g + 1) * GP]
                    nc.tensor.matmul(svp[:GP, :], src, p1_sb,
                                     start=True, stop=True)
                    nc.tensor.matmul(svp[GP:, :], src, p2_sb,
                                     start=True, stop=True)
                    svS = svsp.tile([C, 18], fp32, tag="svS")
                    svS_r[g] = svS
                    nc.vector.tensor_copy(svS[:], svp[:])

                def emit_builds(g, dg, js):
                    svS = svS_r[g]
                    for j in js:
                        m, kp = divmod(j, 3)
                        sc = svS[:, kp * NB + m : kp * NB + m + 1]
                        dslice = dg[:, j * GP : (j + 1) * GP]
                        eng = _B18[j]
                        if eng == "D":
                            nc.vector.tensor_scalar(
                                dslice, mask2_sb, sc, None, MUL
                            )
                        elif eng == "P":
                            nc.gpsimd.tensor_scalar(
                                dslice, mask2_sb, sc, None, MUL
                            )
                        else:
                            nc.scalar.activation(dslice, mask2_sb, Copy,
                                                 scale=sc)

                # conv row-tile slots spread at cadence ~6.5 so conv filler
                # lasts until iteration ~138 (row 23 deadline is ~141)
                _fs = {int(_r * 6.5 + 0.5): _r + 2 for _r in range(NT - 2)}
                _ws = {k + 1: v for k, v in _fs.items()}
                _bs = {k + 2: v for k, v in _fs.items()}

                for r in range(2):
                    emit_arow_w(r, emit_arow_f(r))
                    emit_arow_b(r)
                # fTd chunks 0,1 + SV for groups 0,1
                for cch in range(2):
                    ft = ftp.tile([C, FCH8], fp16, tag="ft")
                    ft_r[cch] = ft
                    nc.sync.dma_start(ft[:], fTd[:, cch * FCH8 : (cch + 1) * FCH8])
                emit_sv(0)
                emit_sv(1)

                for i in range(NG + 3):
                    if i in _fs:
                        arow_ps = emit_arow_f(_fs[i])
                    elif i in _ws:
                        emit_arow_w(_ws[i], arow_ps)
                    elif i in _bs:
                        emit_arow_b(_bs[i])
                    # acc(i-2) psum -> SBUF (Pool)
                    if 0 <= i - 2 < NG:
                        boS = bop.tile([C, NB * GP], fp16, tag="boS")
                        boS_r[i - 2] = boS
                        acc = acc_r.pop(i - 2)
                        nc.vector.tensor_copy(boS[:, : 3 * GP], acc[:, : 3 * GP])
                        nc.scalar.copy(boS[:, 3 * GP :], acc[:, 3 * GP :])
                    # orow(i-3) + store
                    if 0 <= i - 3 < NG:
                        j = i - 3
                        g8 = j % OG
                        if g8 == 0:
                            orow_bufs[j] = orp.tile(
                                [C, OG * GP], fp16, tag="orow", name="orow_buf"
                            )
                        ob = orow_bufs[j - g8]
                        nc.scalar.activation(
                            ob[:, g8 * GP : (g8 + 1) * GP], pso_r.pop(j),
                            Ident, bias=b3_sb,
                        )
                        if g8 == OG - 1 or j == NG - 1:
                            t0 = j - g8
                            nc.sync.dma_start(
                                out[:, 1 + t0 * GP : 1 + t0 * GP + (g8 + 1) * GP],
                                ob[:, : (g8 + 1) * GP],
                            )
                            del orow_bufs[t0]
                    # stream next fTd chunk
                    if i % 8 == 0 and i // 8 + 2 < NCH:
                        cch = i // 8 + 2
                        ft = ftp.tile([C, FCH8], fp16, tag="ft")
                        ft_r[cch] = ft
                        nc.sync.dma_start(
                            ft[:], fTd[:, cch * FCH8 : (cch + 1) * FCH8]
                        )
                    # per-pixel scales two groups ahead
                    if i + 2 < NG:
                        emit_sv(i + 2)
                    # F(i), blockdiag builds(i), F evac(i)
                    if i < NG:
                        psf = psB2.tile([C, 3 * C], fp32, tag="psf", bufs=1,
                                        name="psf")
                        ft = ft_r[i // 8]
                        for kp in range(3):
                            for ci in range(2):
                                nc.tensor.matmul(
                                    psf[:, kp * C : (kp + 1) * C],
                                    bnd2_sb[:, (kp * 2 + ci) * C
                                            : (kp * 2 + ci + 1) * C],
                                    ft[:, ((i % 8) * 2 + ci) * C
                                       : ((i % 8) * 2 + ci + 1) * C],
                                    start=(ci == 0),
                                    stop=(ci == 1),
                                )
                        dg = dgp.tile([C, 18 * GP], fp16, tag="dg")
                        dg_r[i] = dg
                        emit_builds(i, dg, range(9))
                        fbS = fbp.tile([C, 3 * C], fp16, tag="fbS")
                        nc.scalar.copy(fbS[:], psf[:])
                        fbS_r[i] = fbS
                        emit_builds(i, dg, range(9, 18))
                        if i % 8 == 7 or i == NG - 1:
                            ft_r.pop(i // 8)
                    # blockdiag matmuls (i-1): acc_m = sum_kp fbS_kp^T @ bd2
                    if 0 <= i - 1 < NG:
                        acc = psB2.tile([C, NB * GP], fp32, tag="accT", bufs=2,
                                        name="acc")
                        acc_r[i - 1] = acc
                        fbS = fbS_r.pop(i - 1)
                        dg = dg_r.pop(i - 1)
                        for m in range(NB):
                            for kp in range(3):
                                j = m * 3 + kp
                                nc.tensor.matmul(
                                    acc[:, m * GP : (m + 1) * GP],
                                    fbS[:, kp * C : (kp + 1) * C],
                                    dg[:, j * GP : (j + 1) * GP],
                                    start=(kp == 0),
                                    stop=(kp == 2),
                                )
                    # coef matmuls (i-2)
                    if 0 <= i - 2 < NG:
                        pso = psB2.tile([C, GP], fp32, tag="pso", bufs=2,
                                        name="pso")
                        pso_r[i - 2] = pso
                        boS = boS_r.pop(i - 2)
                        for m in range(NB):
                            nc.tensor.matmul(
                                pso[:],
                                coefT_sb[:, m * C : (m + 1) * C],
                                boS[:, m * GP : (m + 1) * GP],
                                start=(m == 0),
                                stop=(m == NB - 1),
                            )

    nc.compile()
    return nc


def _get_nc():
    if "nc" not in _CACHE:
        _CACHE["nc"] = build_nc()
    return _CACHE["nc"]


def _prep_maps(feat, weight, conv1_w, conv1_b, conv2_w, conv2_b, bases_buf, coef, bias):
    feat = np.asarray(feat, np.float32)
    weight = np.asarray(weight, np.float32)
    conv1_w = np.asarray(conv1_w, np.float32)
    conv2_w = np.asarray(conv2_w, np.float32)
    bases_buf = np.asarray(bases_buf, np.float32)
    coef = np.asarray(coef, np.float32)

    n = feat.shape[0]
    featp = np.zeros((n, C, HP, WP), np.float16)
    featp[:, :, 1 : H + 1, 1 : W + 1] = feat
    wgtp = np.zeros((n, CW, HP, WP), np.float16)
    wgtp[:, :, 1 : H + 1, 1 : W + 1] = weight

    # host-prepped transposed feature chunks: the 3 disjoint di input
    # windows (66 px each) packed into 2 chunks of 128 rows per group
    # fTd[q, (g*2+ci)*C + c] = fe[c, FOFF + 1 + g*GP + RELS[ci][q]]
    rels1 = list(range(-99, -33)) + list(range(-1, 61))
    rels2 = list(range(61, 65)) + list(range(97, 163))
    rels2 = rels2 + [163] * (C - len(rels2))
    RELS = [np.array(rels1), np.array(rels2)]
    fe = np.zeros((n, C, FEXT), np.float16)
    fe[:, :, FOFF : FOFF + NPAD] = featp.reshape(n, C, NPAD)
    fTdh = np.empty((n, 2 * NG, C, C), np.float16)
    for g in range(NG):
        for ci in range(2):
            idx = FOFF + 1 + g * GP + RELS[ci]
            fTdh[:, g * 2 + ci] = fe[:, :, idx].transpose(0, 2, 1)
    fTdh = np.ascontiguousarray(
        fTdh.transpose(0, 2, 1, 3).reshape(n, C, 2 * NG * C)
    )

    w1f = np.ascontiguousarray(
        conv1_w[:, :C].transpose(1, 2, 3, 0).reshape(C, L * C)
    ).astype(np.float16)
    w1w = np.ascontiguousarray(
        conv1_w[:, C:].transpose(1, 2, 3, 0).reshape(CW, L * C)
    ).astype(np.float16)
    # conv2 weights with k-major output-channel ordering (col = k*NB + m)
    w2h = np.ascontiguousarray(conv2_w[:, :, 0, 0].T).astype(np.float16)
    w2i = np.ascontiguousarray(
        w2h.reshape(C, NB, TEM).transpose(0, 2, 1).reshape(C, NBT)
    )
    # packed band blocks: bnd2[q, (kp*2+ci)*C + ki*GP + p] accumulates
    # bases_buf[2*kp+ki, di*3+dj] where chunk ci row q holds input offset
    # rel = p + (di-1)*WP + dj - 1
    pos = []
    for rels in RELS:
        d = {}
        for q, r in enumerate(rels.tolist()):
            if r not in d:
                d[r] = q
        pos.append(d)
    bndh = np.zeros((C, 3, 2, 2, GP), np.float32)
    for kp in range(3):
        for ki in range(2):
            k = 2 * kp + ki
            for di in range(3):
                for dj in range(3):
                    for p in range(GP):
                        rel = p + (di - 1) * WP + dj - 1
                        ci = 0 if rel in pos[0] else 1
                        bndh[pos[ci][rel], kp, ci, ki, p] += \
                            bases_buf[k, di * 3 + dj]
    bndh = bndh.reshape(C, 6 * C).astype(np.float16)
    # mask2: ones at (p,p) and (GP+p, p)
    mask2 = np.zeros((C, GP), np.float16)
    for p in range(GP):
        mask2[p, p] = 1.0
        mask2[GP + p, p] = 1.0
    coefTh = np.ascontiguousarray(
        coef[:, :, 0, 0].reshape(C, C, NB).transpose(1, 2, 0).reshape(C, NB * C)
    ).astype(np.float16)
    b1h = np.asarray(conv1_b, np.float32).reshape(C, 1)
    b2h = np.asarray(conv2_b, np.float32).reshape(NB, TEM).T.reshape(NBT, 1)
    b3h = np.asarray(bias, np.float32).reshape(C, 1)

    wgtq = np.zeros((n, C, NPAD), np.float16)
    wgtq[:, :CW] = wgtp.reshape(n, CW, NPAD)
    wgtq[:, CW:, : NPAD - WP] = wgtp.reshape(n, CW, NPAD)[:, :, WP:]
    # paired weights: rows 0-63 = tap (0,j), rows 64-127 = tap (1,j)
    wgtq2 = np.zeros((n, C, NPAD), np.float16)
    wgtq2[:, :CW] = wgtp.reshape(n, CW, NPAD)
    wgtq2[:, CW:, : NPAD - 1] = wgtp.reshape(n, CW, NPAD)[:, :, 1:]
    w1wq2 = np.concatenate(
        [
            w1w.reshape(CW, 3, 3, C)[:, 2, 0],
            w1w.reshape(CW, 3, 3, C)[:, 2, 1],
        ],
        axis=0,
    ).reshape(C, C)
    w1wq = np.concatenate(
        [
            w1w.reshape(CW, 3, 3, C)[:, 0],
            w1w.reshape(CW, 3, 3, C)[:, 1],
        ],
        axis=0,
    ).reshape(C, 3 * C)
    pkw = np.concatenate([w1wq, w1wq2], axis=1)
    # transpose column selectors: P1 even-k cols, P2 odd-k cols
    p1 = np.zeros((C, 18), np.float16)
    p2 = np.zeros((C, 18), np.float16)
    for kp in range(3):
        for m in range(NB):
            p1[kp * 12 + m, kp * NB + m] = 1.0
            p2[kp * 12 + NB + m, kp * NB + m] = 1.0
    pk = np.concatenate([w2i, coefTh, bndh, mask2, p1, p2], axis=1)
    pb = np.zeros((C, 3), np.float32)
    pb[:, 0:1] = b1h
    pb[:, 1:2] = b3h
    pb[:NBT, 2:3] = b2h
    shared = {"w1f": w1f, "pkw": pkw, "pk": pk,
              "w1w": np.ascontiguousarray(w1w[:, 8 * C:]), "pb": pb}
    return [
        {
            "featp": featp[i].reshape(C, NPAD).astype(np.float16),
            "wgtq": wgtq[i], "wgtq2": wgtq2[i],
            "fTd": fTdh[i],
            **shared,
        }
        for i in range(n)
    ]


def kernel(feat, weight, conv1_w, conv1_b, conv2_w, conv2_b, bases_buf, coef, bias,
           **run_kwargs):
    in_maps = _prep_maps(
        feat, weight, conv1_w, conv1_b, conv2_w, conv2_b, bases_buf, coef, bias
    )
    res = run_bass_kernel_spmd(
        _get_nc(), in_maps, core_ids=list(range(len(in_maps))), **run_kwargs
    )
    outp = np.stack([r["out"] for r in res.results], 0).astype(np.float32)
    outp = outp[:, :, :NPAD].reshape(-1, C, HP, WP)[:, :, 1 : H + 1, 1 : W + 1]
    _CACHE["last_results"] = res
    return np.ascontiguousarray(outp)


# revision 27
# speedup vs baseline: 1.4716x; 1.0581x over previous
"""Trainium2 Bass kernel for the DCF (dynamic conv filter) module.

Sharding: pure data-parallel over batch N=8 across 8 NeuronCores (one image
per core); all parameters replicated.

Pipeline per core (one 128x96x96 image):
  A:  conv1 (3x3, 192->128) + tanh -> hmid;  conv2 (1x1, 128->36) + tanh -> b
  B:  per 64-pixel group g:
        - SV: double transpose of the b rows + strided half-copies give the
          per-pixel scales in (k-parity x 64px, k-pair*6+m) layout
        - F: fixed-basis convs of feat via k-pair-interleaved banded matmuls
          on host-prepped transposed feature chunks (fTd), 3 di accumulated;
          output partitions are (k-parity*64 + pixel)
        - acc_m = sum_kpair fbS_kpair^T @ blockdiag2(s) -- each matmul
          contracts 2 k's at once (k lives in the contraction dim), so the
          k-reduction costs half the rows of the per-(m,k) diag scheme
        - out_group = sum_m coef_m @ acc_m (+bias)

Block-diagonals are built as tensor_scalar(mask2 * s) (DVE 4x mode); acc and
pso are double-buffered in PSUM so evacuation copies never stall the PE."""

from itertools import product

import numpy as np

import concourse.bass as bass
import concourse.tile as tile
from concourse import bacc, mybir
from concourse.bass_utils import run_bass_kernel_spmd
from concourse.masks import make_identity

fp16 = mybir.dt.float16
fp32 = mybir.dt.float32

N_CORES = 8
C = 128
CW = 64
H = W = 96
HP = WP = 98
NPIX = H * W
NPAD = HP * WP  # 9604
NB = 6
TEM = 6
L = 9
NBT = NB * TEM  # 36
RT = 4
FT = RT * W  # 384
NT = H // RT  # 24
GP = 64           # output pixels per group
NG = 152          # groups (cover padded idx 1 .. 1+152*64 = 9729)
BP = 9732         # padded bsb/out length
FEXT = 10000      # extended (host-side) padded feat length for fTd windows
FOFF = 98         # fTd window base offset inside the extended buffer
OG = 8            # output groups per store
FCH8 = 8 * 2 * C  # fTd cols per streamed chunk (8 groups)
NCH = (NG + 7) // 8  # 19 chunks

# build engine mix for the 18 blockdiag builds per group: 12 DVE (86ns in 4x
# mode), 4 Pool, 2 Act
_B18 = ["D", "P", "D", "P", "D", "P", "D", "D", "D",
        "P", "P", "D", "D", "P", "D", "P", "D", "D"]

_CACHE = {}


def build_nc():
    nc = bacc.Bacc("TRN2", target_bir_lowering=False, debug=False)

    featp = nc.dram_tensor("featp", [C, NPAD], fp16, kind="ExternalInput").ap()
    wgtq = nc.dram_tensor("wgtq", [C, NPAD], fp16, kind="ExternalInput").ap()
    fTd = nc.dram_tensor("fTd", [C, 2 * NG * C], fp16, kind="ExternalInput").ap()
    wgtq2 = nc.dram_tensor("wgtq2", [C, NPAD], fp16, kind="ExternalInput").ap()
    w1f = nc.dram_tensor("w1f", [C, L * C], fp16, kind="ExternalInput").ap()
    # conv1 weight-branch params: w1wq|w1wq2 = 384+128
    pkw = nc.dram_tensor("pkw", [C, 512], fp16, kind="ExternalInput").ap()
    # fp16 params packed: w2|coefT|bnd2|mask2|P1|P2 = 36+768+768+64+18+18
    pk = nc.dram_tensor("pk", [C, 1672], fp16, kind="ExternalInput").ap()
    w1w = nc.dram_tensor("w1w", [CW, C], fp16, kind="ExternalInput").ap()
    pb = nc.dram_tensor("pb", [C, 3], fp32, kind="ExternalInput").ap()
    out = nc.dram_tensor("out", [C, BP], fp16, kind="ExternalOutput").ap()

    Tanh = mybir.ActivationFunctionType.Tanh
    Ident = mybir.ActivationFunctionType.Identity
    Copy = mybir.ActivationFunctionType.Copy
    MUL = mybir.AluOpType.mult
    ADD = mybir.AluOpType.add

    with tile.TileContext(nc) as tc:
        with (
            tc.tile_pool(name="const", bufs=1) as const,
            tc.tile_pool(name="big", bufs=1) as big,
        ):
            featp_sb = big.tile([C, NPAD], fp16)
            wgtp_sb = big.tile([C, NPAD], fp16)
            cuts = [0, 2404, 4808, 7212, NPAD]
            # startup: tiny "head" pieces first so the first conv row-tile's
            # operands land within ~2.5us (DMA pipeline latency floor)
            HD = 686  # cols covering feat/wgt rows 0..6 (row-tile 0 + halo)
            w1f_sb = const.tile([C, L * C], fp16)
            nc.sync.dma_start(w1f_sb[:, : 4 * C], w1f[:, : 4 * C])
            nc.sync.dma_start(featp_sb[:, :HD], featp[:, :HD])
            nc.sync.dma_start(w1f_sb[:, 4 * C :], w1f[:, 4 * C :])
            pkw_sb = const.tile([C, 512], fp16)
            nc.sync.dma_start(pkw_sb[:], pkw)
            w1wq_sb = pkw_sb[:, 0:384]
            w1wq2_sb = pkw_sb[:, 384:512]
            wgtq2_sb = big.tile([C, NPAD], fp16)
            nc.sync.dma_start(wgtp_sb[:, :HD], wgtq[:, :HD])
            nc.sync.dma_start(wgtq2_sb[:, :HD], wgtq2[:, :HD])
            pb_sb = const.tile([C, 3], fp32)
            nc.sync.dma_start(pb_sb[:], pb)
            b1_sb = pb_sb[:, 0:1]
            b3_sb = pb_sb[:, 1:2]
            b2_sb = pb_sb[:NBT, 2:3]
            w1w_sb = const.tile([CW, C], fp16)
            nc.sync.dma_start(w1w_sb[:], w1w)
            nc.sync.dma_start(featp_sb[:, HD : cuts[1]], featp[:, HD : cuts[1]])
            pk_sb = const.tile([C, 1672], fp16)
            nc.sync.dma_start(pk_sb[:, 804:], pk[:, 804:])
            nc.sync.dma_start(pk_sb[:, :804], pk[:, :804])
            ft0_sb = big.tile([C, FCH8], fp16)
            nc.sync.dma_start(ft0_sb[:], fTd[:, :FCH8])
            w2_sb = pk_sb[:, 0:36]
            coefT_sb = pk_sb[:, 36:804]
            bnd2_sb = pk_sb[:, 804:1572]
            mask2_sb = pk_sb[:, 1572:1636]
            p1_sb = pk_sb[:NBT, 1636:1654]
            p2_sb = pk_sb[:NBT, 1654:1672]
            nc.sync.dma_start(wgtq2_sb[:, HD : cuts[1]], wgtq2[:, HD : cuts[1]])
            ft1_sb = big.tile([C, FCH8], fp16)
            nc.sync.dma_start(ft1_sb[:], fTd[:, FCH8 : 2 * FCH8])
            nc.sync.dma_start(wgtp_sb[:, HD : cuts[2]], wgtq[:, HD : cuts[2]])
            nc.sync.dma_start(
                featp_sb[:, cuts[1] : cuts[2]], featp[:, cuts[1] : cuts[2]]
            )
            nc.sync.dma_start(wgtq2_sb[:, cuts[1] :], wgtq2[:, cuts[1] :])
            nc.sync.dma_start(
                featp_sb[:, cuts[2] : cuts[3]], featp[:, cuts[2] : cuts[3]]
            )
            nc.sync.dma_start(wgtp_sb[:, cuts[2] :], wgtq[:, cuts[2] :])
            nc.sync.dma_start(
                featp_sb[:, cuts[3] :], featp[:, cuts[3] :]
            )
            bsb = big.tile([NBT, BP], fp16)
            # zero only the border/tail cells conv2 never writes (full memset
            # would hold Pool for 8us before the first b write)
            nc.gpsimd.memset(bsb[:, : WP + 2], 0.0)
            edge = bsb[:, 97 : 97 + 97 * WP].rearrange("c (r w) -> c r w", w=WP)
            nc.gpsimd.memset(edge[:, :, 0:2], 0.0)
            nc.gpsimd.memset(bsb[:, 97 * WP :], 0.0)

            b3d = bsb[:, :NPAD].rearrange("c (r w) -> c r w", w=WP)
            f3 = featp_sb[:].rearrange("c (r w) -> c r w", w=WP)
            w3 = wgtp_sb[:].rearrange("c (r w) -> c r w", w=WP)
            wq2 = wgtq2_sb[:].rearrange("c (r w) -> c r w", w=WP)

            # ---- fused pipeline: conv rows (phase A) stream in between the
            # software-pipelined per-group stages of phase B.
            with (
                tc.tile_pool(name="hmp", bufs=3) as hmp,
                tc.tile_pool(name="ftp", bufs=3) as ftp,
                tc.tile_pool(name="svsp", bufs=4) as svsp,
                tc.tile_pool(name="fbp", bufs=3) as fbp,
                tc.tile_pool(name="dgp", bufs=3) as dgp,
                tc.tile_pool(name="bop", bufs=3) as bop,
                tc.tile_pool(name="orp", bufs=2) as orp,
                tc.tile_pool(name="psB2", bufs=1, space="PSUM") as psB2,
            ):
                fbS_r, dg_r, boS_r, pso_r, acc_r = {}, {}, {}, {}, {}
                hm_r = {}
                svS_r, ft_r = {}, {}
                orow_bufs = {}

                def emit_arow_f(t):
                    r0 = t * RT
                    ps = psB2.tile([C, FT], fp32, tag="psA", bufs=1, name="ps")
                    for kk, (i, j) in enumerate(product(range(3), range(3))):
                        nc.tensor.matmul(
                            ps[:],
                            w1f_sb[:, (i * 3 + j) * C : (i * 3 + j + 1) * C],
                            f3[:, r0 + i : r0 + i + RT, j : j + W],
                            start=(kk == 0),
                            stop=False,
                        )
                    return ps

                def emit_arow_w(t, ps):
                    r0 = t * RT
                    for j in range(3):
                        nc.tensor.matmul(
                            ps[:],
                            w1wq_sb[:, j * C : (j + 1) * C],
                            w3[:, r0 : r0 + RT, j : j + W],
                            start=False,
                            stop=False,
                        )
                    nc.tensor.matmul(
                        ps[:],
                        w1wq2_sb,
                        wq2[:, r0 + 2 : r0 + 2 + RT, 0 : W],
                        start=False,
                        stop=False,
                    )
                    nc.tensor.matmul(
                        ps[:],
                        w1w_sb[:],
                        w3[:64, r0 + 2 : r0 + 2 + RT, 2 : 2 + W],
                        start=False,
                        stop=True,
                    )
                    hm = hmp.tile([C, FT], fp16, tag="hm")
                    nc.scalar.activation(hm[:], ps[:], Tanh, bias=b1_sb)
                    hm_r[t] = hm

                def emit_arow_b(t):
                    r0 = t * RT
                    hm = hm_r.pop(t)
                    ps2 = psB2.tile([NBT, FT], fp32, tag="psB", bufs=1, name="ps2")
                    nc.tensor.matmul(ps2[:], w2_sb, hm[:], start=True, stop=True)
                    nc.scalar.activation(
                        b3d[:, r0 + 1 : r0 + 1 + RT, 1 : 1 + W],
                        ps2[:].rearrange("c (r w) -> c r w", w=W),
                        Tanh,
                        bias=b2_sb,
                    )

                def emit_sv(g):
                    # b rows (k-major: row = k*6+m) -> per-pixel scales in
                    # (k-parity*64+p, kpair*6+m) layout: the two transposes
                    # use column-selector matrices (even/odd k) so svp lands
                    # pre-interleaved; one contiguous copy evacuates it
                    svp = psB2.tile([C, 18], fp32, tag="svp", bufs=1,
                                    name="svp")
                    src = bsb[:, 1 + g * GP : 1 + (g + 1) * GP]
                    nc.tensor.matmul(svp[:GP, :], src, p1_sb,
                                     start=True, stop=True)
                    nc.tensor.matmul(svp[GP:, :], src, p2_sb,
                                     start=True, stop=True)
                    svS = svsp.tile([C, 18], fp32, tag="svS")
                    svS_r[g] = svS
                    nc.vector.tensor_copy(svS[:], svp[:])

                def emit_builds(g, dg, js):
                    svS = svS_r[g]
                    for j in js:
                        m, kp = divmod(j, 3)
                        sc = svS[:, kp * NB + m : kp * NB + m + 1]
                        dslice = dg[:, j * GP : (j + 1) * GP]
                        eng = _B18[j]
                        if eng == "D":
                            nc.vector.tensor_scalar(
                                dslice, mask2_sb, sc, None, MUL
                            )
                        elif eng == "P":
                            nc.gpsimd.tensor_scalar(
                                dslice, mask2_sb, sc, None, MUL
                            )
                        else:
                            nc.scalar.activation(dslice, mask2_sb, Copy,
                                                 scale=sc)

                # conv row-tile slots spread at cadence ~6.5 so conv filler
                # lasts until iteration ~138 (row 23 deadline is ~141)
                _fs = {int(_r * 6.5 + 0.5): _r + 2 for _r in range(NT - 2)}
                _ws = {k + 1: v for k, v in _fs.items()}
                _bs = {k + 2: v for k, v in _fs.items()}

                for r in range(2):
                    emit_arow_w(r, emit_arow_f(r))
                    emit_arow_b(r)
                # fTd chunks 0,1 were DMA'd in the startup sequence
                ft_r[0] = ft0_sb
                ft_r[1] = ft1_sb
                emit_sv(0)
                emit_sv(1)

                for i in range(NG + 3):
                    if i in _fs:
                        arow_ps = emit_arow_f(_fs[i])
                    elif i in _ws:
                        emit_arow_w(_ws[i], arow_ps)
                    elif i in _bs:
                        emit_arow_b(_bs[i])
                    # acc(i-2) psum -> SBUF (Pool)
                    if 0 <= i - 2 < NG:
                        boS = bop.tile([C, NB * GP], fp16, tag="boS")
                        boS_r[i - 2] = boS
                        acc = acc_r.pop(i - 2)
                        nc.vector.tensor_copy(boS[:, : 3 * GP], acc[:, : 3 * GP])
                        nc.scalar.copy(boS[:, 3 * GP :], acc[:, 3 * GP :])
                    # orow(i-3) + store
                    if 0 <= i - 3 < NG:
                        j = i - 3
                        g8 = j % OG
                        if g8 == 0:
                            orow_bufs[j] = orp.tile(
                                [C, OG * GP], fp16, tag="orow", name="orow_buf"
                            )
                        ob = orow_bufs[j - g8]
                        nc.scalar.activation(
                            ob[:, g8 * GP : (g8 + 1) * GP], pso_r.pop(j),
                            Ident, bias=b3_sb,
                        )
                        if g8 == OG - 1 or j == NG - 1:
                            t0 = j - g8
                            nc.sync.dma_start(
                                out[:, 1 + t0 * GP : 1 + t0 * GP + (g8 + 1) * GP],
                                ob[:, : (g8 + 1) * GP],
                            )
                            del orow_bufs[t0]
                    # stream next fTd chunk
                    if i % 8 == 0 and i // 8 + 2 < NCH:
                        cch = i // 8 + 2
                        ft = ftp.tile([C, FCH8], fp16, tag="ft")
                        ft_r[cch] = ft
                        nc.sync.dma_start(
                            ft[:], fTd[:, cch * FCH8 : (cch + 1) * FCH8]
                        )
                    # per-pixel scales two groups ahead
                    if i + 2 < NG:
                        emit_sv(i + 2)
                    # F(i), blockdiag builds(i), F evac(i)
                    if i < NG:
                        psf = psB2.tile([C, 3 * C], fp32, tag="psf", bufs=1,
                                        name="psf")
                        ft = ft_r[i // 8]
                        for kp in range(3):
                            for ci in range(2):
                                nc.tensor.matmul(
                                    psf[:, kp * C : (kp + 1) * C],
                                    bnd2_sb[:, (kp * 2 + ci) * C
                                            : (kp * 2 + ci + 1) * C],
                                    ft[:, ((i % 8) * 2 + ci) * C
                                       : ((i % 8) * 2 + ci + 1) * C],
                                    start=(ci == 0),
                                    stop=(ci == 1),
                                )
                        dg = dgp.tile([C, 18 * GP], fp16, tag="dg")
                        dg_r[i] = dg
                        emit_builds(i, dg, range(9))
                        fbS = fbp.tile([C, 3 * C], fp16, tag="fbS")
                        nc.scalar.copy(fbS[:], psf[:])
                        fbS_r[i] = fbS
                        emit_builds(i, dg, range(9, 18))
                        if i % 8 == 7 or i == NG - 1:
                            ft_r.pop(i // 8)
                    # blockdiag matmuls (i-1): acc_m = sum_kp fbS_kp^T @ bd2
                    if 0 <= i - 1 < NG:
                        acc = psB2.tile([C, NB * GP], fp32, tag="accT", bufs=2,
                                        name="acc")
                        acc_r[i - 1] = acc
                        fbS = fbS_r.pop(i - 1)
                        dg = dg_r.pop(i - 1)
                        for m in range(NB):
                            for kp in range(3):
                                j = m * 3 + kp
                                nc.tensor.matmul(
                                    acc[:, m * GP : (m + 1) * GP],
                                    fbS[:, kp * C : (kp + 1) * C],
                                    dg[:, j * GP : (j + 1) * GP],
                                    start=(kp == 0),
                                    stop=(kp == 2),
                                )
                    # coef matmuls (i-2)
                    if 0 <= i - 2 < NG:
                        pso = psB2.tile([C, GP], fp32, tag="pso", bufs=2,
                                        name="pso")
                        pso_r[i - 2] = pso
                        boS = boS_r.pop(i - 2)
                        for m in range(NB):
                            nc.tensor.matmul(
                                pso[:],
                                coefT_sb[:, m * C : (m + 1) * C],
                                boS[:, m * GP : (m + 1) * GP],
                                start=(m == 0),
                                stop=(m == NB - 1),
                            )

    nc.compile()
    return nc


def _get_nc():
    if "nc" not in _CACHE:
        _CACHE["nc"] = build_nc()
    return _CACHE["nc"]


def _prep_maps(feat, weight, conv1_w, conv1_b, conv2_w, conv2_b, bases_buf, coef, bias):
    feat = np.asarray(feat, np.float32)
    weight = np.asarray(weight, np.float32)
    conv1_w = np.asarray(conv1_w, np.float32)
    conv2_w = np.asarray(conv2_w, np.float32)
    bases_buf = np.asarray(bases_buf, np.float32)
    coef = np.asarray(coef, np.float32)

    n = feat.shape[0]
    featp = np.zeros((n, C, HP, WP), np.float16)
    featp[:, :, 1 : H + 1, 1 : W + 1] = feat
    wgtp = np.zeros((n, CW, HP, WP), np.float16)
    wgtp[:, :, 1 : H + 1, 1 : W + 1] = weight

    # host-prepped transposed feature chunks: the 3 disjoint di input
    # windows (66 px each) packed into 2 chunks of 128 rows per group
    # fTd[q, (g*2+ci)*C + c] = fe[c, FOFF + 1 + g*GP + RELS[ci][q]]
    rels1 = list(range(-99, -33)) + list(range(-1, 61))
    rels2 = list(range(61, 65)) + list(range(97, 163))
    rels2 = rels2 + [163] * (C - len(rels2))
    RELS = [np.array(rels1), np.array(rels2)]
    fe = np.zeros((n, C, FEXT), np.float16)
    fe[:, :, FOFF : FOFF + NPAD] = featp.reshape(n, C, NPAD)
    fTdh = np.empty((n, 2 * NG, C, C), np.float16)
    for g in range(NG):
        for ci in range(2):
            idx = FOFF + 1 + g * GP + RELS[ci]
            fTdh[:, g * 2 + ci] = fe[:, :, idx].transpose(0, 2, 1)
    fTdh = np.ascontiguousarray(
        fTdh.transpose(0, 2, 1, 3).reshape(n, C, 2 * NG * C)
    )

    w1f = np.ascontiguousarray(
        conv1_w[:, :C].transpose(1, 2, 3, 0).reshape(C, L * C)
    ).astype(np.float16)
    w1w = np.ascontiguousarray(
        conv1_w[:, C:].transpose(1, 2, 3, 0).reshape(CW, L * C)
    ).astype(np.float16)
    # conv2 weights with k-major output-channel ordering (col = k*NB + m)
    w2h = np.ascontiguousarray(conv2_w[:, :, 0, 0].T).astype(np.float16)
    w2i = np.ascontiguousarray(
        w2h.reshape(C, NB, TEM).transpose(0, 2, 1).reshape(C, NBT)
    )
    # packed band blocks: bnd2[q, (kp*2+ci)*C + ki*GP + p] accumulates
    # bases_buf[2*kp+ki, di*3+dj] where chunk ci row q holds input offset
    # rel = p + (di-1)*WP + dj - 1
    pos = []
    for rels in RELS:
        d = {}
        for q, r in enumerate(rels.tolist()):
            if r not in d:
                d[r] = q
        pos.append(d)
    bndh = np.zeros((C, 3, 2, 2, GP), np.float32)
    for kp in range(3):
        for ki in range(2):
            k = 2 * kp + ki
            for di in range(3):
                for dj in range(3):
                    for p in range(GP):
                        rel = p + (di - 1) * WP + dj - 1
                        ci = 0 if rel in pos[0] else 1
                        bndh[pos[ci][rel], kp, ci, ki, p] += \
                            bases_buf[k, di * 3 + dj]
    bndh = bndh.reshape(C, 6 * C).astype(np.float16)
    # mask2: ones at (p,p) and (GP+p, p)
    mask2 = np.zeros((C, GP), np.float16)
    for p in range(GP):
        mask2[p, p] = 1.0
        mask2[GP + p, p] = 1.0
    coefTh = np.ascontiguousarray(
        coef[:, :, 0, 0].reshape(C, C, NB).transpose(1, 2, 0).reshape(C, NB * C)
    ).astype(np.float16)
    b1h = np.asarray(conv1_b, np.float32).reshape(C, 1)
    b2h = np.asarray(conv2_b, np.float32).reshape(NB, TEM).T.reshape(NBT, 1)
    b3h = np.asarray(bias, np.float32).reshape(C, 1)

    wgtq = np.zeros((n, C, NPAD), np.float16)
    wgtq[:, :CW] = wgtp.reshape(n, CW, NPAD)
    wgtq[:, CW:, : NPAD - WP] = wgtp.reshape(n, CW, NPAD)[:, :, WP:]
    # paired weights: rows 0-63 = tap (0,j), rows 64-127 = tap (1,j)
    wgtq2 = np.zeros((n, C, NPAD), np.float16)
    wgtq2[:, :CW] = wgtp.reshape(n, CW, NPAD)
    wgtq2[:, CW:, : NPAD - 1] = wgtp.reshape(n, CW, NPAD)[:, :, 1:]
    w1wq2 = np.concatenate(
        [
            w1w.reshape(CW, 3, 3, C)[:, 2, 0],
            w1w.reshape(CW, 3, 3, C)[:, 2, 1],
        ],
        axis=0,
    ).reshape(C, C)
    w1wq = np.concatenate(
        [
            w1w.reshape(CW, 3, 3, C)[:, 0],
            w1w.reshape(CW, 3, 3, C)[:, 1],
        ],
        axis=0,
    ).reshape(C, 3 * C)
    pkw = np.concatenate([w1wq, w1wq2], axis=1)
    # transpose column selectors: P1 even-k cols, P2 odd-k cols
    p1 = np.zeros((C, 18), np.float16)
    p2 = np.zeros((C, 18), np.float16)
    for kp in range(3):
        for m in range(NB):
            p1[kp * 12 + m, kp * NB + m] = 1.0
            p2[kp * 12 + NB + m, kp * NB + m] = 1.0
    pk = np.concatenate([w2i, coefTh, bndh, mask2, p1, p2], axis=1)
    pb = np.zeros((C, 3), np.float32)
    pb[:, 0:1] = b1h
    pb[:, 1:2] = b3h
    pb[:NBT, 2:3] = b2h
    shared = {"w1f": w1f, "pkw": pkw, "pk": pk,
              "w1w": np.ascontiguousarray(w1w[:, 8 * C:]), "pb": pb}
    return [
        {
            "featp": featp[i].reshape(C, NPAD).astype(np.float16),
            "wgtq": wgtq[i], "wgtq2": wgtq2[i],
            "fTd": fTdh[i],
            **shared,
        }
        for i in range(n)
    ]


def kernel(feat, weight, conv1_w, conv1_b, conv2_w, conv2_b, bases_buf, coef, bias,
           **run_kwargs):
    in_maps = _prep_maps(
        feat, weight, conv1_w, conv1_b, conv2_w, conv2_b, bases_buf, coef, bias
    )
    res = run_bass_kernel_spmd(
        _get_nc(), in_maps, core_ids=list(range(len(in_maps))), **run_kwargs
    )
    outp = np.stack([r["out"] for r in res.results], 0).astype(np.float32)
    outp = outp[:, :, :NPAD].reshape(-1, C, HP, WP)[:, :, 1 : H + 1, 1 : W + 1]
    _CACHE["last_results"] = res
    return np.ascontiguousarray(outp)


# revision 28
# speedup vs baseline: 1.4718x; 1.0001x over previous
"""Trainium2 Bass kernel for the DCF (dynamic conv filter) module.

Sharding: pure data-parallel over batch N=8 across 8 NeuronCores (one image
per core); all parameters replicated.

Pipeline per core (one 128x96x96 image):
  A:  conv1 (3x3, 192->128) + tanh -> hmid;  conv2 (1x1, 128->36) + tanh -> b
  B:  per 64-pixel group g:
        - SV: double transpose of the b rows + strided half-copies give the
          per-pixel scales in (k-parity x 64px, k-pair*6+m) layout
        - F: fixed-basis convs of feat via k-pair-interleaved banded matmuls
          on host-prepped transposed feature chunks (fTd), 3 di accumulated;
          output partitions are (k-parity*64 + pixel)
        - acc_m = sum_kpair fbS_kpair^T @ blockdiag2(s) -- each matmul
          contracts 2 k's at once (k lives in the contraction dim), so the
          k-reduction costs half the rows of the per-(m,k) diag scheme
        - out_group = sum_m coef_m @ acc_m (+bias)

Block-diagonals are built as tensor_scalar(mask2 * s) (DVE 4x mode); acc and
pso are double-buffered in PSUM so evacuation copies never stall the PE."""

from itertools import product

import numpy as np

import concourse.bass as bass
import concourse.tile as tile
from concourse import bacc, mybir
from concourse.bass_utils import run_bass_kernel_spmd
from concourse.masks import make_identity

fp16 = mybir.dt.float16
fp32 = mybir.dt.float32

N_CORES = 8
C = 128
CW = 64
H = W = 96
HP = WP = 98
NPIX = H * W
NPAD = HP * WP  # 9604
NB = 6
TEM = 6
L = 9
NBT = NB * TEM  # 36
RT = 4
FT = RT * W  # 384
NT = H // RT  # 24
GP = 64           # output pixels per group
NG = 152          # groups (cover padded idx 1 .. 1+152*64 = 9729)
BP = 9732         # padded bsb/out length
FEXT = 10000      # extended (host-side) padded feat length for fTd windows
FOFF = 98         # fTd window base offset inside the extended buffer
OG = 8            # output groups per store
FCH8 = 8 * 2 * C  # fTd cols per streamed chunk (8 groups)
NCH = (NG + 7) // 8  # 19 chunks

# build engine mix for the 18 blockdiag builds per group: 12 DVE (86ns in 4x
# mode), 4 Pool, 2 Act
_B18 = ["D", "P", "D", "P", "D", "P", "D", "D", "D",
        "P", "P", "D", "D", "P", "D", "P", "D", "D"]

_CACHE = {}


def build_nc():
    nc = bacc.Bacc("TRN2", target_bir_lowering=False, debug=False)

    featp = nc.dram_tensor("featp", [C, NPAD], fp16, kind="ExternalInput").ap()
    wgtq = nc.dram_tensor("wgtq", [C, NPAD], fp16, kind="ExternalInput").ap()
    fTd = nc.dram_tensor("fTd", [C, 2 * NG * C], fp16, kind="ExternalInput").ap()
    wgtq2 = nc.dram_tensor("wgtq2", [C, NPAD], fp16, kind="ExternalInput").ap()
    w1f = nc.dram_tensor("w1f", [C, L * C], fp16, kind="ExternalInput").ap()
    # conv1 weight-branch params: w1wq|w1wq2 = 384+128
    pkw = nc.dram_tensor("pkw", [C, 512], fp16, kind="ExternalInput").ap()
    # fp16 params packed: w2|coefT|bnd2|mask2|P1|P2 = 36+768+768+64+18+18
    pk = nc.dram_tensor("pk", [C, 1672], fp16, kind="ExternalInput").ap()
    w1w = nc.dram_tensor("w1w", [CW, C], fp16, kind="ExternalInput").ap()
    pb = nc.dram_tensor("pb", [C, 3], fp32, kind="ExternalInput").ap()
    out = nc.dram_tensor("out", [C, BP], fp16, kind="ExternalOutput").ap()

    Tanh = mybir.ActivationFunctionType.Tanh
    Ident = mybir.ActivationFunctionType.Identity
    Copy = mybir.ActivationFunctionType.Copy
    MUL = mybir.AluOpType.mult
    ADD = mybir.AluOpType.add

    with tile.TileContext(nc) as tc:
        with (
            tc.tile_pool(name="const", bufs=1) as const,
            tc.tile_pool(name="big", bufs=1) as big,
        ):
            featp_sb = big.tile([C, NPAD], fp16)
            wgtp_sb = big.tile([C, NPAD], fp16)
            cuts = [0, 2404, 4808, 7212, NPAD]
            # startup: tiny "head" pieces first so the first conv row-tile's
            # operands land within ~2.5us (DMA pipeline latency floor)
            HD = 686  # cols covering feat/wgt rows 0..6 (row-tile 0 + halo)
            w1f_sb = const.tile([C, L * C], fp16)
            nc.sync.dma_start(w1f_sb[:, : 4 * C], w1f[:, : 4 * C])
            nc.sync.dma_start(featp_sb[:, :HD], featp[:, :HD])
            nc.sync.dma_start(w1f_sb[:, 4 * C :], w1f[:, 4 * C :])
            pkw_sb = const.tile([C, 512], fp16)
            nc.sync.dma_start(pkw_sb[:], pkw)
            w1wq_sb = pkw_sb[:, 0:384]
            w1wq2_sb = pkw_sb[:, 384:512]
            wgtq2_sb = big.tile([C, NPAD], fp16)
            nc.sync.dma_start(wgtp_sb[:, :HD], wgtq[:, :HD])
            nc.sync.dma_start(wgtq2_sb[:, :HD], wgtq2[:, :HD])
            pb_sb = const.tile([C, 3], fp32)
            nc.sync.dma_start(pb_sb[:], pb)
            b1_sb = pb_sb[:, 0:1]
            b3_sb = pb_sb[:, 1:2]
            b2_sb = pb_sb[:NBT, 2:3]
            w1w_sb = const.tile([CW, C], fp16)
            nc.sync.dma_start(w1w_sb[:], w1w)
            nc.sync.dma_start(featp_sb[:, HD : cuts[1]], featp[:, HD : cuts[1]])
            pk_sb = const.tile([C, 1672], fp16)
            nc.sync.dma_start(pk_sb[:, 804:], pk[:, 804:])
            nc.sync.dma_start(pk_sb[:, :804], pk[:, :804])
            ft0_sb = big.tile([C, FCH8], fp16)
            nc.sync.dma_start(ft0_sb[:], fTd[:, :FCH8])
            w2_sb = pk_sb[:, 0:36]
            coefT_sb = pk_sb[:, 36:804]
            bnd2_sb = pk_sb[:, 804:1572]
            mask2_sb = pk_sb[:, 1572:1636]
            p1_sb = pk_sb[:NBT, 1636:1654]
            p2_sb = pk_sb[:NBT, 1654:1672]
            nc.sync.dma_start(wgtq2_sb[:, HD : cuts[1]], wgtq2[:, HD : cuts[1]])
            ft1_sb = big.tile([C, FCH8], fp16)
            nc.sync.dma_start(ft1_sb[:], fTd[:, FCH8 : 2 * FCH8])
            nc.sync.dma_start(wgtp_sb[:, HD : cuts[2]], wgtq[:, HD : cuts[2]])
            nc.sync.dma_start(
                featp_sb[:, cuts[1] : cuts[2]], featp[:, cuts[1] : cuts[2]]
            )
            nc.sync.dma_start(wgtq2_sb[:, cuts[1] :], wgtq2[:, cuts[1] :])
            nc.sync.dma_start(
                featp_sb[:, cuts[2] : cuts[3]], featp[:, cuts[2] : cuts[3]]
            )
            nc.sync.dma_start(wgtp_sb[:, cuts[2] :], wgtq[:, cuts[2] :])
            nc.sync.dma_start(
                featp_sb[:, cuts[3] :], featp[:, cuts[3] :]
            )
            bsb = big.tile([NBT, BP], fp16)
            # zero only the border/tail cells conv2 never writes (full memset
            # would hold Pool for 8us before the first b write)
            nc.gpsimd.memset(bsb[:, : WP + 2], 0.0)
            edge = bsb[:, 97 : 97 + 97 * WP].rearrange("c (r w) -> c r w", w=WP)
            nc.gpsimd.memset(edge[:, :, 0:2], 0.0)
            nc.gpsimd.memset(bsb[:, 97 * WP :], 0.0)

            b3d = bsb[:, :NPAD].rearrange("c (r w) -> c r w", w=WP)
            f3 = featp_sb[:].rearrange("c (r w) -> c r w", w=WP)
            w3 = wgtp_sb[:].rearrange("c (r w) -> c r w", w=WP)
            wq2 = wgtq2_sb[:].rearrange("c (r w) -> c r w", w=WP)

            # ---- fused pipeline: conv rows (phase A) stream in between the
            # software-pipelined per-group stages of phase B.
            with (
                tc.tile_pool(name="hmp", bufs=3) as hmp,
                tc.tile_pool(name="ftp", bufs=3) as ftp,
                tc.tile_pool(name="svsp", bufs=4) as svsp,
                tc.tile_pool(name="fbp", bufs=3) as fbp,
                tc.tile_pool(name="dgp", bufs=3) as dgp,
                tc.tile_pool(name="bop", bufs=3) as bop,
                tc.tile_pool(name="orp", bufs=2) as orp,
                tc.tile_pool(name="psB2", bufs=1, space="PSUM") as psB2,
            ):
                fbS_r, dg_r, boS_r, pso_r, acc_r = {}, {}, {}, {}, {}
                hm_r = {}
                svS_r, ft_r = {}, {}
                orow_bufs = {}

                def emit_arow_f(t):
                    r0 = t * RT
                    ps = psB2.tile([C, FT], fp32, tag="psA", bufs=1, name="ps")
                    for kk, (i, j) in enumerate(product(range(3), range(3))):
                        nc.tensor.matmul(
                            ps[:],
                            w1f_sb[:, (i * 3 + j) * C : (i * 3 + j + 1) * C],
                            f3[:, r0 + i : r0 + i + RT, j : j + W],
                            start=(kk == 0),
                            stop=False,
                        )
                    return ps

                def emit_arow_w(t, ps):
                    r0 = t * RT
                    for j in range(3):
                        nc.tensor.matmul(
                            ps[:],
                            w1wq_sb[:, j * C : (j + 1) * C],
                            w3[:, r0 : r0 + RT, j : j + W],
                            start=False,
                            stop=False,
                        )
                    nc.tensor.matmul(
                        ps[:],
                        w1wq2_sb,
                        wq2[:, r0 + 2 : r0 + 2 + RT, 0 : W],
                        start=False,
                        stop=False,
                    )
                    nc.tensor.matmul(
                        ps[:],
                        w1w_sb[:],
                        w3[:64, r0 + 2 : r0 + 2 + RT, 2 : 2 + W],
                        start=False,
                        stop=True,
                    )
                    hm = hmp.tile([C, FT], fp16, tag="hm")
                    nc.scalar.activation(hm[:], ps[:], Tanh, bias=b1_sb)
                    hm_r[t] = hm

                def emit_arow_b(t):
                    r0 = t * RT
                    hm = hm_r.pop(t)
                    ps2 = psB2.tile([NBT, FT], fp32, tag="psB", bufs=1, name="ps2")
                    nc.tensor.matmul(ps2[:], w2_sb, hm[:], start=True, stop=True)
                    nc.scalar.activation(
                        b3d[:, r0 + 1 : r0 + 1 + RT, 1 : 1 + W],
                        ps2[:].rearrange("c (r w) -> c r w", w=W),
                        Tanh,
                        bias=b2_sb,
                    )

                def emit_sv(g):
                    # b rows (k-major: row = k*6+m) -> per-pixel scales in
                    # (k-parity*64+p, kpair*6+m) layout: the two transposes
                    # use column-selector matrices (even/odd k) so svp lands
                    # pre-interleaved; one contiguous copy evacuates it
                    svp = psB2.tile([C, 18], fp32, tag="svp", bufs=1,
                                    name="svp")
                    src = bsb[:, 1 + g * GP : 1 + (g + 1) * GP]
                    nc.tensor.matmul(svp[:GP, :], src, p1_sb,
                                     start=True, stop=True)
                    nc.tensor.matmul(svp[GP:, :], src, p2_sb,
                                     start=True, stop=True)
                    svS = svsp.tile([C, 18], fp32, tag="svS")
                    svS_r[g] = svS
                    nc.vector.tensor_copy(svS[:], svp[:])

                def emit_builds(g, dg, js):
                    svS = svS_r[g]
                    for j in js:
                        m, kp = divmod(j, 3)
                        sc = svS[:, kp * NB + m : kp * NB + m + 1]
                        dslice = dg[:, j * GP : (j + 1) * GP]
                        eng = _B18[j]
                        if eng == "D":
                            nc.vector.tensor_scalar(
                                dslice, mask2_sb, sc, None, MUL
                            )
                        elif eng == "P":
                            nc.gpsimd.tensor_scalar(
                                dslice, mask2_sb, sc, None, MUL
                            )
                        else:
                            nc.scalar.activation(dslice, mask2_sb, Copy,
                                                 scale=sc)

                # conv row-tile slots spread at cadence ~6.5 so conv filler
                # lasts until iteration ~138 (row 23 deadline is ~141)
                _fs = {int(_r * 6.5 + 0.5): _r + 2 for _r in range(NT - 2)}
                _ws = {k + 1: v for k, v in _fs.items()}
                _bs = {k + 2: v for k, v in _fs.items()}

                for r in range(2):
                    emit_arow_w(r, emit_arow_f(r))
                    emit_arow_b(r)
                # fTd chunks 0,1 were DMA'd in the startup sequence
                ft_r[0] = ft0_sb
                ft_r[1] = ft1_sb
                emit_sv(0)
                emit_sv(1)

                for i in range(NG + 3):
                    if i in _fs:
                        arow_ps = emit_arow_f(_fs[i])
                    elif i in _ws:
                        emit_arow_w(_ws[i], arow_ps)
                    elif i in _bs:
                        emit_arow_b(_bs[i])
                    # acc(i-2) psum -> SBUF (Pool)
                    if 0 <= i - 2 < NG:
                        boS = bop.tile([C, NB * GP], fp16, tag="boS")
                        boS_r[i - 2] = boS
                        acc = acc_r.pop(i - 2)
                        nc.vector.tensor_copy(boS[:, : 3 * GP], acc[:, : 3 * GP])
                        nc.scalar.copy(boS[:, 3 * GP :], acc[:, 3 * GP :])
                    # orow(i-3) + store
                    if 0 <= i - 3 < NG:
                        j = i - 3
                        g8 = j % OG
                        if g8 == 0:
                            orow_bufs[j] = orp.tile(
                                [C, OG * GP], fp16, tag="orow", name="orow_buf"
                            )
                        ob = orow_bufs[j - g8]
                        nc.scalar.activation(
                            ob[:, g8 * GP : (g8 + 1) * GP], pso_r.pop(j),
                            Ident, bias=b3_sb,
                        )
                        if g8 == OG - 1 or j == NG - 1:
                            t0 = j - g8
                            st0 = 4 * GP if j == NG - 1 else 0
                            nc.sync.dma_start(
                                out[:, 1 + t0 * GP + st0
                                    : 1 + t0 * GP + (g8 + 1) * GP],
                                ob[:, st0 : (g8 + 1) * GP],
                            )
                            del orow_bufs[t0]
                        elif j == NG - 5 and g8 == 3:
                            t0 = j - g8
                            nc.sync.dma_start(
                                out[:, 1 + t0 * GP : 1 + t0 * GP + 4 * GP],
                                ob[:, : 4 * GP],
                            )
                    # stream next fTd chunk
                    if i % 8 == 0 and i // 8 + 2 < NCH:
                        cch = i // 8 + 2
                        ft = ftp.tile([C, FCH8], fp16, tag="ft")
                        ft_r[cch] = ft
                        nc.sync.dma_start(
                            ft[:], fTd[:, cch * FCH8 : (cch + 1) * FCH8]
                        )
                    # per-pixel scales two groups ahead
                    if i + 2 < NG:
                        emit_sv(i + 2)
                    # F(i), blockdiag builds(i), F evac(i)
                    if i < NG:
                        psf = psB2.tile([C, 3 * C], fp32, tag="psf", bufs=1,
                                        name="psf")
                        ft = ft_r[i // 8]
                        for kp in range(3):
                            for ci in range(2):
                                nc.tensor.matmul(
                                    psf[:, kp * C : (kp + 1) * C],
                                    bnd2_sb[:, (kp * 2 + ci) * C
                                            : (kp * 2 + ci + 1) * C],
                                    ft[:, ((i % 8) * 2 + ci) * C
                                       : ((i % 8) * 2 + ci + 1) * C],
                                    start=(ci == 0),
                                    stop=(ci == 1),
                                )
                        dg = dgp.tile([C, 18 * GP], fp16, tag="dg")
                        dg_r[i] = dg
                        emit_builds(i, dg, range(9))
                        fbS = fbp.tile([C, 3 * C], fp16, tag="fbS")
                        nc.scalar.copy(fbS[:], psf[:])
                        fbS_r[i] = fbS
                        emit_builds(i, dg, range(9, 18))
                        if i % 8 == 7 or i == NG - 1:
                            ft_r.pop(i // 8)
                    # blockdiag matmuls (i-1): acc_m = sum_kp fbS_kp^T @ bd2
                    if 0 <= i - 1 < NG:
                        acc = psB2.tile([C, NB * GP], fp32, tag="accT", bufs=2,
                                        name="acc")
                        acc_r[i - 1] = acc
                        fbS = fbS_r.pop(i - 1)
                        dg = dg_r.pop(i - 1)
                        for m in range(NB):
                            for kp in range(3):
                                j = m * 3 + kp
                                nc.tensor.matmul(
                                    acc[:, m * GP : (m + 1) * GP],
                                    fbS[:, kp * C : (kp + 1) * C],
                                    dg[:, j * GP : (j + 1) * GP],
                                    start=(kp == 0),
                                    stop=(kp == 2),
                                )
                    # coef matmuls (i-2)
                    if 0 <= i - 2 < NG:
                        pso = psB2.tile([C, GP], fp32, tag="pso", bufs=2,
                                        name="pso")
                        pso_r[i - 2] = pso
                        boS = boS_r.pop(i - 2)
                        for m in range(NB):
                            nc.tensor.matmul(
                                pso[:],
                                coefT_sb[:, m * C : (m + 1) * C],
                                boS[:, m * GP : (m + 1) * GP],
                                start=(m == 0),
                                stop=(m == NB - 1),
                            )

    nc.compile()
    return nc


def _get_nc():
    if "nc" not in _CACHE:
        _CACHE["nc"] = build_nc()
    return _CACHE["nc"]


def _prep_maps(feat, weight, conv1_w, conv1_b, conv2_w, conv2_b, bases_buf, coef, bias):
    feat = np.asarray(feat, np.float32)
    weight = np.asarray(weight, np.float32)
    conv1_w = np.asarray(conv1_w, np.float32)
    conv2_w = np.asarray(conv2_w, np.float32)
    bases_buf = np.asarray(bases_buf, np.float32)
    coef = np.asarray(coef, np.float32)

    n = feat.shape[0]
    featp = np.zeros((n, C, HP, WP), np.float16)
    featp[:, :, 1 : H + 1, 1 : W + 1] = feat
    wgtp = np.zeros((n, CW, HP, WP), np.float16)
    wgtp[:, :, 1 : H + 1, 1 : W + 1] = weight

    # host-prepped transposed feature chunks: the 3 disjoint di input
    # windows (66 px each) packed into 2 chunks of 128 rows per group
    # fTd[q, (g*2+ci)*C + c] = fe[c, FOFF + 1 + g*GP + RELS[ci][q]]
    rels1 = list(range(-99, -33)) + list(range(-1, 61))
    rels2 = list(range(61, 65)) + list(range(97, 163))
    rels2 = rels2 + [163] * (C - len(rels2))
    RELS = [np.array(rels1), np.array(rels2)]
    fe = np.zeros((n, C, FEXT), np.float16)
    fe[:, :, FOFF : FOFF + NPAD] = featp.reshape(n, C, NPAD)
    fTdh = np.empty((n, 2 * NG, C, C), np.float16)
    for g in range(NG):
        for ci in range(2):
            idx = FOFF + 1 + g * GP + RELS[ci]
            fTdh[:, g * 2 + ci] = fe[:, :, idx].transpose(0, 2, 1)
    fTdh = np.ascontiguousarray(
        fTdh.transpose(0, 2, 1, 3).reshape(n, C, 2 * NG * C)
    )

    w1f = np.ascontiguousarray(
        conv1_w[:, :C].transpose(1, 2, 3, 0).reshape(C, L * C)
    ).astype(np.float16)
    w1w = np.ascontiguousarray(
        conv1_w[:, C:].transpose(1, 2, 3, 0).reshape(CW, L * C)
    ).astype(np.float16)
    # conv2 weights with k-major output-channel ordering (col = k*NB + m)
    w2h = np.ascontiguousarray(conv2_w[:, :, 0, 0].T).astype(np.float16)
    w2i = np.ascontiguousarray(
        w2h.reshape(C, NB, TEM).transpose(0, 2, 1).reshape(C, NBT)
    )
    # packed band blocks: bnd2[q, (kp*2+ci)*C + ki*GP + p] accumulates
    # bases_buf[2*kp+ki, di*3+dj] where chunk ci row q holds input offset
    # rel = p + (di-1)*WP + dj - 1
    pos = []
    for rels in RELS:
        d = {}
        for q, r in enumerate(rels.tolist()):
            if r not in d:
                d[r] = q
        pos.append(d)
    bndh = np.zeros((C, 3, 2, 2, GP), np.float32)
    for kp in range(3):
        for ki in range(2):
            k = 2 * kp + ki
            for di in range(3):
                for dj in range(3):
                    for p in range(GP):
                        rel = p + (di - 1) * WP + dj - 1
                        ci = 0 if rel in pos[0] else 1
                        bndh[pos[ci][rel], kp, ci, ki, p] += \
                            bases_buf[k, di * 3 + dj]
    bndh = bndh.reshape(C, 6 * C).astype(np.float16)
    # mask2: ones at (p,p) and (GP+p, p)
    mask2 = np.zeros((C, GP), np.float16)
    for p in range(GP):
        mask2[p, p] = 1.0
        mask2[GP + p, p] = 1.0
    coefTh = np.ascontiguousarray(
        coef[:, :, 0, 0].reshape(C, C, NB).transpose(1, 2, 0).reshape(C, NB * C)
    ).astype(np.float16)
    b1h = np.asarray(conv1_b, np.float32).reshape(C, 1)
    b2h = np.asarray(conv2_b, np.float32).reshape(NB, TEM).T.reshape(NBT, 1)
    b3h = np.asarray(bias, np.float32).reshape(C, 1)

    wgtq = np.zeros((n, C, NPAD), np.float16)
    wgtq[:, :CW] = wgtp.reshape(n, CW, NPAD)
    wgtq[:, CW:, : NPAD - WP] = wgtp.reshape(n, CW, NPAD)[:, :, WP:]
    # paired weights: rows 0-63 = tap (0,j), rows 64-127 = tap (1,j)
    wgtq2 = np.zeros((n, C, NPAD), np.float16)
    wgtq2[:, :CW] = wgtp.reshape(n, CW, NPAD)
    wgtq2[:, CW:, : NPAD - 1] = wgtp.reshape(n, CW, NPAD)[:, :, 1:]
    w1wq2 = np.concatenate(
        [
            w1w.reshape(CW, 3, 3, C)[:, 2, 0],
            w1w.reshape(CW, 3, 3, C)[:, 2, 1],
        ],
        axis=0,
    ).reshape(C, C)
    w1wq = np.concatenate(
        [
            w1w.reshape(CW, 3, 3, C)[:, 0],
            w1w.reshape(CW, 3, 3, C)[:, 1],
        ],
        axis=0,
    ).reshape(C, 3 * C)
    pkw = np.concatenate([w1wq, w1wq2], axis=1)
    # transpose column selectors: P1 even-k cols, P2 odd-k cols
    p1 = np.zeros((C, 18), np.float16)
    p2 = np.zeros((C, 18), np.float16)
    for kp in range(3):
        for m in range(NB):
            p1[kp * 12 + m, kp * NB + m] = 1.0
            p2[kp * 12 + NB + m, kp * NB + m] = 1.0
    pk = np.concatenate([w2i, coefTh, bndh, mask2, p1, p2], axis=1)
    pb = np.zeros((C, 3), np.float32)
    pb[:, 0:1] = b1h
    pb[:, 1:2] = b3h
    pb[:NBT, 2:3] = b2h
    shared = {"w1f": w1f, "pkw": pkw, "pk": pk,
              "w1w": np.ascontiguousarray(w1w[:, 8 * C:]), "pb": pb}
    return [
        {
            "featp": featp[i].reshape(C, NPAD).astype(np.float16),
            "wgtq": wgtq[i], "wgtq2": wgtq2[i],
            "fTd": fTdh[i],
            **shared,
        }
        for i in range(n)
    ]


def kernel(feat, weight, conv1_w, conv1_b, conv2_w, conv2_b, bases_buf, coef, bias,
           **run_kwargs):
    in_maps = _prep_maps(
        feat, weight, conv1_w, conv1_b, conv2_w, conv2_b, bases_buf, coef, bias
    )
    res = run_bass_kernel_spmd(
        _get_nc(), in_maps, core_ids=list(range(len(in_maps))), **run_kwargs
    )
    outp = np.stack([r["out"] for r in res.results], 0).astype(np.float32)
    outp = outp[:, :, :NPAD].reshape(-1, C, HP, WP)[:, :, 1 : H + 1, 1 : W + 1]
    _CACHE["last_results"] = res
    return np.ascontiguousarray(outp)


# revision 29
# speedup vs baseline: 1.5090x; 1.0253x over previous
"""Trainium2 Bass kernel for the DCF (dynamic conv filter) module.

Sharding: pure data-parallel over batch N=8 across 8 NeuronCores (one image
per core); all parameters replicated.

Pipeline per core (one 128x96x96 image):
  A:  conv1 (3x3, 192->128) + tanh -> hmid;  conv2 (1x1, 128->36) + tanh -> b
  B:  per 64-pixel group g:
        - SV: double transpose of the b rows + strided half-copies give the
          per-pixel scales in (k-parity x 64px, k-pair*6+m) layout
        - F: fixed-basis convs of feat via k-pair-interleaved banded matmuls
          on host-prepped transposed feature chunks (fTd), 3 di accumulated;
          output partitions are (k-parity*64 + pixel)
        - acc_m = sum_kpair fbS_kpair^T @ blockdiag2(s) -- each matmul
          contracts 2 k's at once (k lives in the contraction dim), so the
          k-reduction costs half the rows of the per-(m,k) diag scheme
        - out_group = sum_m coef_m @ acc_m (+bias)

Block-diagonals are built as tensor_scalar(mask2 * s) (DVE 4x mode); acc and
pso are double-buffered in PSUM so evacuation copies never stall the PE."""

from itertools import product

import numpy as np

import concourse.bass as bass
import concourse.tile as tile
from concourse import bacc, mybir
from concourse.bass_utils import run_bass_kernel_spmd
from concourse.masks import make_identity

fp16 = mybir.dt.float16
fp32 = mybir.dt.float32

N_CORES = 8
C = 128
CW = 64
H = W = 96
HP = WP = 98
NPIX = H * W
NPAD = HP * WP  # 9604
NB = 6
TEM = 6
L = 9
NBT = NB * TEM  # 36
RT = 4
FT = RT * W  # 384
NT = H // RT  # 24
GP = 64           # output pixels per group
NG = 152          # groups (cover padded idx 1 .. 1+152*64 = 9729)
BP = 9732         # padded bsb/out length
FEXT = 10000      # extended (host-side) padded feat length for fTd windows
FOFF = 98         # fTd window base offset inside the extended buffer
OG = 8            # output groups per store
FCH8 = 8 * 2 * C  # fTd cols per streamed chunk (8 groups)
NCH = (NG + 7) // 8  # 19 chunks

# build engine mix for the 18 blockdiag builds per group: 12 DVE (86ns in 4x
# mode), 4 Pool, 2 Act
_B18 = ["D", "P", "D", "P", "D", "P", "D", "D", "D",
        "P", "P", "D", "D", "P", "D", "P", "D", "D"]

_CACHE = {}


def build_nc():
    nc = bacc.Bacc("TRN2", target_bir_lowering=False, debug=False)

    featp = nc.dram_tensor("featp", [C, NPAD], fp16, kind="ExternalInput").ap()
    wgtq = nc.dram_tensor("wgtq", [C, NPAD], fp16, kind="ExternalInput").ap()
    fTd = nc.dram_tensor("fTd", [C, 2 * NG * C], fp16, kind="ExternalInput").ap()
    wgtq2 = nc.dram_tensor("wgtq2", [C, NPAD], fp16, kind="ExternalInput").ap()
    w1f = nc.dram_tensor("w1f", [C, L * C], fp16, kind="ExternalInput").ap()
    # conv1 weight-branch params: w1wq|w1wq2 = 384+128
    pkw = nc.dram_tensor("pkw", [C, 512], fp16, kind="ExternalInput").ap()
    # fp16 params packed: w2|coefT|bnd2|mask2|P1|P2 = 36+768+768+64+18+18
    pk = nc.dram_tensor("pk", [C, 1672], fp16, kind="ExternalInput").ap()
    w1w = nc.dram_tensor("w1w", [CW, C], fp16, kind="ExternalInput").ap()
    pb = nc.dram_tensor("pb", [C, 3], fp32, kind="ExternalInput").ap()
    out = nc.dram_tensor("out", [C, BP], fp16, kind="ExternalOutput").ap()

    Tanh = mybir.ActivationFunctionType.Tanh
    Ident = mybir.ActivationFunctionType.Identity
    Copy = mybir.ActivationFunctionType.Copy
    MUL = mybir.AluOpType.mult
    ADD = mybir.AluOpType.add

    with tile.TileContext(nc) as tc:
        with (
            tc.tile_pool(name="const", bufs=1) as const,
            tc.tile_pool(name="big", bufs=1) as big,
        ):
            featp_sb = big.tile([C, NPAD], fp16)
            wgtp_sb = big.tile([C, NPAD], fp16)
            cuts = [0, 2404, 4808, 7212, NPAD]
            # startup: tiny "head" pieces first so the first conv row-tile's
            # operands land within ~2.5us (DMA pipeline latency floor)
            HD = 686  # cols covering feat/wgt rows 0..6 (row-tile 0 + halo)
            w1f_sb = const.tile([C, L * C], fp16)
            nc.sync.dma_start(w1f_sb[:, : 4 * C], w1f[:, : 4 * C])
            nc.sync.dma_start(featp_sb[:, :HD], featp[:, :HD])
            nc.sync.dma_start(w1f_sb[:, 4 * C :], w1f[:, 4 * C :])
            pkw_sb = const.tile([C, 512], fp16)
            nc.sync.dma_start(pkw_sb[:], pkw)
            w1wq_sb = pkw_sb[:, 0:384]
            w1wq2_sb = pkw_sb[:, 384:512]
            wgtq2_sb = big.tile([C, NPAD], fp16)
            nc.sync.dma_start(wgtp_sb[:, :HD], wgtq[:, :HD])
            nc.sync.dma_start(wgtq2_sb[:, :HD], wgtq2[:, :HD])
            pb_sb = const.tile([C, 3], fp32)
            nc.sync.dma_start(pb_sb[:], pb)
            b1_sb = pb_sb[:, 0:1]
            b3_sb = pb_sb[:, 1:2]
            b2_sb = pb_sb[:NBT, 2:3]
            w1w_sb = const.tile([CW, C], fp16)
            nc.sync.dma_start(w1w_sb[:], w1w)
            nc.sync.dma_start(featp_sb[:, HD : cuts[1]], featp[:, HD : cuts[1]])
            pk_sb = const.tile([C, 1672], fp16)
            nc.sync.dma_start(pk_sb[:, 804:], pk[:, 804:])
            nc.sync.dma_start(pk_sb[:, :804], pk[:, :804])
            ft0_sb = big.tile([C, FCH8], fp16)
            nc.sync.dma_start(ft0_sb[:], fTd[:, :FCH8])
            w2_sb = pk_sb[:, 0:36]
            coefT_sb = pk_sb[:, 36:804]
            bnd2_sb = pk_sb[:, 804:1572]
            mask2_sb = pk_sb[:, 1572:1636]
            p1_sb = pk_sb[:NBT, 1636:1654]
            p2_sb = pk_sb[:NBT, 1654:1672]
            nc.sync.dma_start(wgtq2_sb[:, HD : cuts[1]], wgtq2[:, HD : cuts[1]])
            ft1_sb = big.tile([C, FCH8], fp16)
            nc.sync.dma_start(ft1_sb[:], fTd[:, FCH8 : 2 * FCH8])
            nc.sync.dma_start(wgtp_sb[:, HD : cuts[2]], wgtq[:, HD : cuts[2]])
            nc.sync.dma_start(
                featp_sb[:, cuts[1] : cuts[2]], featp[:, cuts[1] : cuts[2]]
            )
            nc.sync.dma_start(wgtq2_sb[:, cuts[1] :], wgtq2[:, cuts[1] :])
            nc.sync.dma_start(
                featp_sb[:, cuts[2] : cuts[3]], featp[:, cuts[2] : cuts[3]]
            )
            nc.sync.dma_start(wgtp_sb[:, cuts[2] :], wgtq[:, cuts[2] :])
            nc.sync.dma_start(
                featp_sb[:, cuts[3] :], featp[:, cuts[3] :]
            )
            bsb = big.tile([NBT, BP], fp16)
            # zero only the border/tail cells conv2 never writes (full memset
            # would hold Pool for 8us before the first b write)
            nc.gpsimd.memset(bsb[:, : WP + 2], 0.0)
            edge = bsb[:, 97 : 97 + 97 * WP].rearrange("c (r w) -> c r w", w=WP)
            nc.gpsimd.memset(edge[:, :, 0:2], 0.0)
            nc.gpsimd.memset(bsb[:, 97 * WP :], 0.0)

            b3d = bsb[:, :NPAD].rearrange("c (r w) -> c r w", w=WP)
            f3 = featp_sb[:].rearrange("c (r w) -> c r w", w=WP)
            w3 = wgtp_sb[:].rearrange("c (r w) -> c r w", w=WP)
            wq2 = wgtq2_sb[:].rearrange("c (r w) -> c r w", w=WP)

            # ---- fused pipeline: conv rows (phase A) stream in between the
            # software-pipelined per-group stages of phase B.
            with (
                tc.tile_pool(name="hmp", bufs=3) as hmp,
                tc.tile_pool(name="ftp", bufs=3) as ftp,
                tc.tile_pool(name="svsp", bufs=4) as svsp,
                tc.tile_pool(name="fbp", bufs=3) as fbp,
                tc.tile_pool(name="dgp", bufs=3) as dgp,
                tc.tile_pool(name="bop", bufs=3) as bop,
                tc.tile_pool(name="orp", bufs=2) as orp,
                tc.tile_pool(name="psB2", bufs=1, space="PSUM") as psB2,
            ):
                fbS_r, dg_r, boS_r, pso_r, acc_r = {}, {}, {}, {}, {}
                hm_r = {}
                svS_r, ft_r = {}, {}
                orow_bufs = {}

                def emit_arow_f(t):
                    r0 = t * RT
                    ps = psB2.tile([C, FT], fp32, tag="psA", bufs=1, name="ps")
                    for kk, (i, j) in enumerate(product(range(3), range(3))):
                        nc.tensor.matmul(
                            ps[:],
                            w1f_sb[:, (i * 3 + j) * C : (i * 3 + j + 1) * C],
                            f3[:, r0 + i : r0 + i + RT, j : j + W],
                            start=(kk == 0),
                            stop=False,
                        )
                    return ps

                def emit_arow_w(t, ps):
                    r0 = t * RT
                    for j in range(3):
                        nc.tensor.matmul(
                            ps[:],
                            w1wq_sb[:, j * C : (j + 1) * C],
                            w3[:, r0 : r0 + RT, j : j + W],
                            start=False,
                            stop=False,
                        )
                    nc.tensor.matmul(
                        ps[:],
                        w1wq2_sb,
                        wq2[:, r0 + 2 : r0 + 2 + RT, 0 : W],
                        start=False,
                        stop=False,
                    )
                    nc.tensor.matmul(
                        ps[:],
                        w1w_sb[:],
                        w3[:64, r0 + 2 : r0 + 2 + RT, 2 : 2 + W],
                        start=False,
                        stop=True,
                    )
                    hm = hmp.tile([C, FT], fp16, tag="hm")
                    nc.scalar.activation(hm[:], ps[:], Tanh, bias=b1_sb)
                    hm_r[t] = hm

                def emit_arow_b(t):
                    r0 = t * RT
                    hm = hm_r.pop(t)
                    ps2 = psB2.tile([NBT, FT], fp32, tag="psB", bufs=1, name="ps2")
                    nc.tensor.matmul(ps2[:], w2_sb, hm[:], start=True, stop=True)
                    nc.scalar.activation(
                        b3d[:, r0 + 1 : r0 + 1 + RT, 1 : 1 + W],
                        ps2[:].rearrange("c (r w) -> c r w", w=W),
                        Tanh,
                        bias=b2_sb,
                    )

                def emit_sv(g):
                    # b rows (k-major: row = k*6+m) -> per-pixel scales in
                    # (k-parity*64+p, kpair*6+m) layout: the two transposes
                    # use column-selector matrices (even/odd k) so svp lands
                    # pre-interleaved; one contiguous copy evacuates it
                    svp = psB2.tile([C, 18], fp32, tag="svp", bufs=1,
                                    name="svp")
                    src = bsb[:, 1 + g * GP : 1 + (g + 1) * GP]
                    nc.tensor.matmul(svp[:GP, :], src, p1_sb,
                                     start=True, stop=True)
                    nc.tensor.matmul(svp[GP:, :], src, p2_sb,
                                     start=True, stop=True)
                    svS = svsp.tile([C, 18], fp32, tag="svS")
                    svS_r[g] = svS
                    nc.vector.tensor_copy(svS[:], svp[:])

                def emit_builds(g, dg, js):
                    svS = svS_r[g]
                    for j in js:
                        m, kp = divmod(j, 3)
                        sc = svS[:, kp * NB + m : kp * NB + m + 1]
                        dslice = dg[:, j * GP : (j + 1) * GP]
                        eng = _B18[j]
                        if eng == "D":
                            nc.vector.tensor_scalar(
                                dslice, mask2_sb, sc, None, MUL
                            )
                        elif eng == "P":
                            nc.gpsimd.tensor_scalar(
                                dslice, mask2_sb, sc, None, MUL
                            )
                        else:
                            nc.scalar.activation(dslice, mask2_sb, Copy,
                                                 scale=sc)

                # conv row-tile slots spread at cadence ~6.5 so conv filler
                # lasts until iteration ~138 (row 23 deadline is ~141)
                _fs = {int(_r * 6.5 + 0.5): _r + 2 for _r in range(NT - 2)}
                _ws = {k + 1: v for k, v in _fs.items()}
                _bs = {k + 2: v for k, v in _fs.items()}

                for r in range(2):
                    emit_arow_w(r, emit_arow_f(r))
                    emit_arow_b(r)
                # fTd chunks 0,1 were DMA'd in the startup sequence
                ft_r[0] = ft0_sb
                ft_r[1] = ft1_sb
                emit_sv(0)
                emit_sv(1)

                for i in range(NG + 3):
                    if i in _fs:
                        arow_ps = emit_arow_f(_fs[i])
                    elif i in _ws:
                        emit_arow_w(_ws[i], arow_ps)
                    elif i in _bs:
                        emit_arow_b(_bs[i])
                    # acc(i-2) psum -> SBUF (Pool)
                    if 0 <= i - 2 < NG:
                        boS = bop.tile([C, NB * GP], fp16, tag="boS")
                        boS_r[i - 2] = boS
                        acc = acc_r.pop(i - 2)
                        nc.vector.tensor_copy(boS[:, : 3 * GP], acc[:, : 3 * GP])
                        nc.scalar.copy(boS[:, 3 * GP :], acc[:, 3 * GP :])
                    # orow(i-3) + store
                    if 0 <= i - 3 < NG:
                        j = i - 3
                        g8 = j % OG
                        if g8 == 0:
                            orow_bufs[j] = orp.tile(
                                [C, OG * GP], fp16, tag="orow", name="orow_buf"
                            )
                        ob = orow_bufs[j - g8]
                        nc.scalar.activation(
                            ob[:, g8 * GP : (g8 + 1) * GP], pso_r.pop(j),
                            Ident, bias=b3_sb,
                        )
                        if g8 == OG - 1 or j == NG - 1:
                            t0 = j - g8
                            st0 = 4 * GP if j == NG - 1 else 0
                            nc.sync.dma_start(
                                out[:, 1 + t0 * GP + st0
                                    : 1 + t0 * GP + (g8 + 1) * GP],
                                ob[:, st0 : (g8 + 1) * GP],
                            )
                            del orow_bufs[t0]
                        elif j == NG - 5 and g8 == 3:
                            t0 = j - g8
                            nc.sync.dma_start(
                                out[:, 1 + t0 * GP : 1 + t0 * GP + 4 * GP],
                                ob[:, : 4 * GP],
                            )
                    # stream next fTd chunk
                    if i % 8 == 0 and i // 8 + 2 < NCH:
                        cch = i // 8 + 2
                        ft = ftp.tile([C, FCH8], fp16, tag="ft")
                        ft_r[cch] = ft
                        nc.sync.dma_start(
                            ft[:], fTd[:, cch * FCH8 : (cch + 1) * FCH8]
                        )
                    # per-pixel scales two groups ahead
                    if i + 2 < NG:
                        emit_sv(i + 2)
                    # F(i), blockdiag builds(i), F evac(i)
                    if i < NG:
                        psf = psB2.tile([C, 3 * C], fp32, tag="psf", bufs=2,
                                        name="psf")
                        ft = ft_r[i // 8]
                        for kp in range(3):
                            for ci in range(2):
                                nc.tensor.matmul(
                                    psf[:, kp * C : (kp + 1) * C],
                                    bnd2_sb[:, (kp * 2 + ci) * C
                                            : (kp * 2 + ci + 1) * C],
                                    ft[:, ((i % 8) * 2 + ci) * C
                                       : ((i % 8) * 2 + ci + 1) * C],
                                    start=(ci == 0),
                                    stop=(ci == 1),
                                )
                        dg = dgp.tile([C, 18 * GP], fp16, tag="dg")
                        dg_r[i] = dg
                        emit_builds(i, dg, range(9))
                        fbS = fbp.tile([C, 3 * C], fp16, tag="fbS")
                        nc.scalar.copy(fbS[:], psf[:])
                        fbS_r[i] = fbS
                        emit_builds(i, dg, range(9, 18))
                        if i % 8 == 7 or i == NG - 1:
                            ft_r.pop(i // 8)
                    # blockdiag matmuls (i-1): acc_m = sum_kp fbS_kp^T @ bd2
                    if 0 <= i - 1 < NG:
                        acc = psB2.tile([C, NB * GP], fp32, tag="accT", bufs=2,
                                        name="acc")
                        acc_r[i - 1] = acc
                        fbS = fbS_r.pop(i - 1)
                        dg = dg_r.pop(i - 1)
                        for m in range(NB):
                            for kp in range(3):
                                j = m * 3 + kp
                                nc.tensor.matmul(
                                    acc[:, m * GP : (m + 1) * GP],
                                    fbS[:, kp * C : (kp + 1) * C],
                                    dg[:, j * GP : (j + 1) * GP],
                                    start=(kp == 0),
                                    stop=(kp == 2),
                                )
                    # coef matmuls (i-2)
                    if 0 <= i - 2 < NG:
                        pso = psB2.tile([C, GP], fp32, tag="pso", bufs=1,
                                        name="pso")
                        pso_r[i - 2] = pso
                        boS = boS_r.pop(i - 2)
                        for m in range(NB):
                            nc.tensor.matmul(
                                pso[:],
                                coefT_sb[:, m * C : (m + 1) * C],
                                boS[:, m * GP : (m + 1) * GP],
                                start=(m == 0),
                                stop=(m == NB - 1),
                            )

    nc.compile()
    return nc


def _get_nc():
    if "nc" not in _CACHE:
        _CACHE["nc"] = build_nc()
    return _CACHE["nc"]


def _prep_maps(feat, weight, conv1_w, conv1_b, conv2_w, conv2_b, bases_buf, coef, bias):
    feat = np.asarray(feat, np.float32)
    weight = np.asarray(weight, np.float32)
    conv1_w = np.asarray(conv1_w, np.float32)
    conv2_w = np.asarray(conv2_w, np.float32)
    bases_buf = np.asarray(bases_buf, np.float32)
    coef = np.asarray(coef, np.float32)

    n = feat.shape[0]
    featp = np.zeros((n, C, HP, WP), np.float16)
    featp[:, :, 1 : H + 1, 1 : W + 1] = feat
    wgtp = np.zeros((n, CW, HP, WP), np.float16)
    wgtp[:, :, 1 : H + 1, 1 : W + 1] = weight

    # host-prepped transposed feature chunks: the 3 disjoint di input
    # windows (66 px each) packed into 2 chunks of 128 rows per group
    # fTd[q, (g*2+ci)*C + c] = fe[c, FOFF + 1 + g*GP + RELS[ci][q]]
    rels1 = list(range(-99, -33)) + list(range(-1, 61))
    rels2 = list(range(61, 65)) + list(range(97, 163))
    rels2 = rels2 + [163] * (C - len(rels2))
    RELS = [np.array(rels1), np.array(rels2)]
    fe = np.zeros((n, C, FEXT), np.float16)
    fe[:, :, FOFF : FOFF + NPAD] = featp.reshape(n, C, NPAD)
    fTdh = np.empty((n, 2 * NG, C, C), np.float16)
    for g in range(NG):
        for ci in range(2):
            idx = FOFF + 1 + g * GP + RELS[ci]
            fTdh[:, g * 2 + ci] = fe[:, :, idx].transpose(0, 2, 1)
    fTdh = np.ascontiguousarray(
        fTdh.transpose(0, 2, 1, 3).reshape(n, C, 2 * NG * C)
    )

    w1f = np.ascontiguousarray(
        conv1_w[:, :C].transpose(1, 2, 3, 0).reshape(C, L * C)
    ).astype(np.float16)
    w1w = np.ascontiguousarray(
        conv1_w[:, C:].transpose(1, 2, 3, 0).reshape(CW, L * C)
    ).astype(np.float16)
    # conv2 weights with k-major output-channel ordering (col = k*NB + m)
    w2h = np.ascontiguousarray(conv2_w[:, :, 0, 0].T).astype(np.float16)
    w2i = np.ascontiguousarray(
        w2h.reshape(C, NB, TEM).transpose(0, 2, 1).reshape(C, NBT)
    )
    # packed band blocks: bnd2[q, (kp*2+ci)*C + ki*GP + p] accumulates
    # bases_buf[2*kp+ki, di*3+dj] where chunk ci row q holds input offset
    # rel = p + (di-1)*WP + dj - 1
    pos = []
    for rels in RELS:
        d = {}
        for q, r in enumerate(rels.tolist()):
            if r not in d:
                d[r] = q
        pos.append(d)
    bndh = np.zeros((C, 3, 2, 2, GP), np.float32)
    for kp in range(3):
        for ki in range(2):
            k = 2 * kp + ki
            for di in range(3):
                for dj in range(3):
                    for p in range(GP):
                        rel = p + (di - 1) * WP + dj - 1
                        ci = 0 if rel in pos[0] else 1
                        bndh[pos[ci][rel], kp, ci, ki, p] += \
                            bases_buf[k, di * 3 + dj]
    bndh = bndh.reshape(C, 6 * C).astype(np.float16)
    # mask2: ones at (p,p) and (GP+p, p)
    mask2 = np.zeros((C, GP), np.float16)
    for p in range(GP):
        mask2[p, p] = 1.0
        mask2[GP + p, p] = 1.0
    coefTh = np.ascontiguousarray(
        coef[:, :, 0, 0].reshape(C, C, NB).transpose(1, 2, 0).reshape(C, NB * C)
    ).astype(np.float16)
    b1h = np.asarray(conv1_b, np.float32).reshape(C, 1)
    b2h = np.asarray(conv2_b, np.float32).reshape(NB, TEM).T.reshape(NBT, 1)
    b3h = np.asarray(bias, np.float32).reshape(C, 1)

    wgtq = np.zeros((n, C, NPAD), np.float16)
    wgtq[:, :CW] = wgtp.reshape(n, CW, NPAD)
    wgtq[:, CW:, : NPAD - WP] = wgtp.reshape(n, CW, NPAD)[:, :, WP:]
    # paired weights: rows 0-63 = tap (0,j), rows 64-127 = tap (1,j)
    wgtq2 = np.zeros((n, C, NPAD), np.float16)
    wgtq2[:, :CW] = wgtp.reshape(n, CW, NPAD)
    wgtq2[:, CW:, : NPAD - 1] = wgtp.reshape(n, CW, NPAD)[:, :, 1:]
    w1wq2 = np.concatenate(
        [
            w1w.reshape(CW, 3, 3, C)[:, 2, 0],
            w1w.reshape(CW, 3, 3, C)[:, 2, 1],
        ],
        axis=0,
    ).reshape(C, C)
    w1wq = np.concatenate(
        [
            w1w.reshape(CW, 3, 3, C)[:, 0],
            w1w.reshape(CW, 3, 3, C)[:, 1],
        ],
        axis=0,
    ).reshape(C, 3 * C)
    pkw = np.concatenate([w1wq, w1wq2], axis=1)
    # transpose column selectors: P1 even-k cols, P2 odd-k cols
    p1 = np.zeros((C, 18), np.float16)
    p2 = np.zeros((C, 18), np.float16)
    for kp in range(3):
        for m in range(NB):
            p1[kp * 12 + m, kp * NB + m] = 1.0
            p2[kp * 12 + NB + m, kp * NB + m] = 1.0
    pk = np.concatenate([w2i, coefTh, bndh, mask2, p1, p2], axis=1)
    pb = np.zeros((C, 3), np.float32)
    pb[:, 0:1] = b1h
    pb[:, 1:2] = b3h
    pb[:NBT, 2:3] = b2h
    shared = {"w1f": w1f, "pkw": pkw, "pk": pk,
              "w1w": np.ascontiguousarray(w1w[:, 8 * C:]), "pb": pb}
    return [
        {
            "featp": featp[i].reshape(C, NPAD).astype(np.float16),
            "wgtq": wgtq[i], "wgtq2": wgtq2[i],
            "fTd": fTdh[i],
            **shared,
        }
        for i in range(n)
    ]


def kernel(feat, weight, conv1_w, conv1_b, conv2_w, conv2_b, bases_buf, coef, bias,
           **run_kwargs):
    in_maps = _prep_maps(
        feat, weight, conv1_w, conv1_b, conv2_w, conv2_b, bases_buf, coef, bias
    )
    res = run_bass_kernel_spmd(
        _get_nc(), in_maps, core_ids=list(range(len(in_maps))), **run_kwargs
    )
    outp = np.stack([r["out"] for r in res.results], 0).astype(np.float32)
    outp = outp[:, :, :NPAD].reshape(-1, C, HP, WP)[:, :, 1 : H + 1, 1 : W + 1]
    _CACHE["last_results"] = res
    return np.ascontiguousarray(outp)


# revision 30
# speedup vs baseline: 1.5205x; 1.0077x over previous
"""Trainium2 Bass kernel for the DCF (dynamic conv filter) module.

Sharding: pure data-parallel over batch N=8 across 8 NeuronCores (one image
per core); all parameters replicated.

Pipeline per core (one 128x96x96 image):
  A:  conv1 (3x3, 192->128) + tanh -> hmid;  conv2 (1x1, 128->36) + tanh -> b
  B:  per 64-pixel group g:
        - SV: double transpose of the b rows + strided half-copies give the
          per-pixel scales in (k-parity x 64px, k-pair*6+m) layout
        - F: fixed-basis convs of feat via k-pair-interleaved banded matmuls
          on host-prepped transposed feature chunks (fTd), 3 di accumulated;
          output partitions are (k-parity*64 + pixel)
        - acc_m = sum_kpair fbS_kpair^T @ blockdiag2(s) -- each matmul
          contracts 2 k's at once (k lives in the contraction dim), so the
          k-reduction costs half the rows of the per-(m,k) diag scheme
        - out_group = sum_m coef_m @ acc_m (+bias)

Block-diagonals are built as tensor_scalar(mask2 * s) (DVE 4x mode); acc and
pso are double-buffered in PSUM so evacuation copies never stall the PE."""

from itertools import product

import numpy as np

import concourse.bass as bass
import concourse.tile as tile
from concourse import bacc, mybir
from concourse.bass_utils import run_bass_kernel_spmd
from concourse.masks import make_identity

fp16 = mybir.dt.float16
fp32 = mybir.dt.float32

N_CORES = 8
C = 128
CW = 64
H = W = 96
HP = WP = 98
NPIX = H * W
NPAD = HP * WP  # 9604
NB = 6
TEM = 6
L = 9
NBT = NB * TEM  # 36
RT = 4
FT = RT * W  # 384
NT = H // RT  # 24
GP = 64           # output pixels per group
NG = 152          # groups (cover padded idx 1 .. 1+152*64 = 9729)
BP = 9732         # padded bsb/out length
FEXT = 10000      # extended (host-side) padded feat length for fTd windows
FOFF = 98         # fTd window base offset inside the extended buffer
OG = 8            # output groups per store
FCH8 = 8 * 2 * C  # fTd cols per streamed chunk (8 groups)
NCH = (NG + 7) // 8  # 19 chunks

# build engine mix for the 18 blockdiag builds per group: 12 DVE (86ns in 4x
# mode), 4 Pool, 2 Act
_B18 = ["D", "P", "D", "P", "D", "P", "D", "D", "D",
        "P", "P", "D", "D", "P", "D", "P", "D", "D"]

_CACHE = {}


def build_nc():
    nc = bacc.Bacc("TRN2", target_bir_lowering=False, debug=False)

    featp = nc.dram_tensor("featp", [C, NPAD], fp16, kind="ExternalInput").ap()
    wgtq = nc.dram_tensor("wgtq", [C, NPAD], fp16, kind="ExternalInput").ap()
    fTd = nc.dram_tensor("fTd", [C, 2 * NG * C], fp16, kind="ExternalInput").ap()
    wgtq2 = nc.dram_tensor("wgtq2", [C, NPAD], fp16, kind="ExternalInput").ap()
    w1f = nc.dram_tensor("w1f", [C, L * C], fp16, kind="ExternalInput").ap()
    # conv1 weight-branch params: w1wq|w1wq2 = 384+128
    pkw = nc.dram_tensor("pkw", [C, 512], fp16, kind="ExternalInput").ap()
    # fp16 params packed: w2|coefT|bnd2|mask2|P1|P2 = 36+768+768+64+18+18
    pk = nc.dram_tensor("pk", [C, 1672], fp16, kind="ExternalInput").ap()
    w1w = nc.dram_tensor("w1w", [CW, C], fp16, kind="ExternalInput").ap()
    pb = nc.dram_tensor("pb", [C, 3], fp32, kind="ExternalInput").ap()
    out = nc.dram_tensor("out", [C, BP], fp16, kind="ExternalOutput").ap()

    Tanh = mybir.ActivationFunctionType.Tanh
    Ident = mybir.ActivationFunctionType.Identity
    Copy = mybir.ActivationFunctionType.Copy
    MUL = mybir.AluOpType.mult
    ADD = mybir.AluOpType.add

    with tile.TileContext(nc) as tc:
        with (
            tc.tile_pool(name="const", bufs=1) as const,
            tc.tile_pool(name="big", bufs=1) as big,
        ):
            featp_sb = big.tile([C, NPAD], fp16)
            wgtp_sb = big.tile([C, NPAD], fp16)
            cuts = [0, 2404, 4808, 7212, NPAD]
            # startup: tiny "head" pieces first so the first conv row-tile's
            # operands land within ~2.5us (DMA pipeline latency floor)
            HD = 686  # cols covering feat/wgt rows 0..6 (row-tile 0 + halo)
            w1f_sb = const.tile([C, L * C], fp16)
            nc.sync.dma_start(w1f_sb[:, : 4 * C], w1f[:, : 4 * C])
            nc.sync.dma_start(featp_sb[:, :HD], featp[:, :HD])
            nc.sync.dma_start(w1f_sb[:, 4 * C :], w1f[:, 4 * C :])
            pkw_sb = const.tile([C, 512], fp16)
            nc.sync.dma_start(pkw_sb[:], pkw)
            w1wq_sb = pkw_sb[:, 0:384]
            w1wq2_sb = pkw_sb[:, 384:512]
            wgtq2_sb = big.tile([C, NPAD], fp16)
            nc.sync.dma_start(wgtp_sb[:, :HD], wgtq[:, :HD])
            nc.sync.dma_start(wgtq2_sb[:, :HD], wgtq2[:, :HD])
            pb_sb = const.tile([C, 3], fp32)
            nc.sync.dma_start(pb_sb[:], pb)
            b1_sb = pb_sb[:, 0:1]
            b3_sb = pb_sb[:, 1:2]
            b2_sb = pb_sb[:NBT, 2:3]
            w1w_sb = const.tile([CW, C], fp16)
            nc.sync.dma_start(w1w_sb[:], w1w)
            nc.sync.dma_start(featp_sb[:, HD : cuts[1]], featp[:, HD : cuts[1]])
            pk_sb = const.tile([C, 1672], fp16)
            nc.sync.dma_start(pk_sb[:, 804:], pk[:, 804:])
            nc.sync.dma_start(pk_sb[:, :804], pk[:, :804])
            ft0_sb = big.tile([C, FCH8], fp16)
            nc.sync.dma_start(ft0_sb[:], fTd[:, :FCH8])
            w2_sb = pk_sb[:, 0:36]
            coefT_sb = pk_sb[:, 36:804]
            bnd2_sb = pk_sb[:, 804:1572]
            mask2_sb = pk_sb[:, 1572:1636]
            p1_sb = pk_sb[:NBT, 1636:1654]
            p2_sb = pk_sb[:NBT, 1654:1672]
            nc.sync.dma_start(wgtq2_sb[:, HD : cuts[1]], wgtq2[:, HD : cuts[1]])
            ft1_sb = big.tile([C, FCH8], fp16)
            nc.sync.dma_start(ft1_sb[:], fTd[:, FCH8 : 2 * FCH8])
            nc.sync.dma_start(wgtp_sb[:, HD : cuts[2]], wgtq[:, HD : cuts[2]])
            nc.sync.dma_start(
                featp_sb[:, cuts[1] : cuts[2]], featp[:, cuts[1] : cuts[2]]
            )
            nc.sync.dma_start(wgtq2_sb[:, cuts[1] :], wgtq2[:, cuts[1] :])
            nc.sync.dma_start(
                featp_sb[:, cuts[2] : cuts[3]], featp[:, cuts[2] : cuts[3]]
            )
            nc.sync.dma_start(wgtp_sb[:, cuts[2] :], wgtq[:, cuts[2] :])
            nc.sync.dma_start(
                featp_sb[:, cuts[3] :], featp[:, cuts[3] :]
            )
            bsb = big.tile([NBT, BP], fp16)
            # zero only the border/tail cells conv2 never writes (full memset
            # would hold Pool for 8us before the first b write)
            nc.gpsimd.memset(bsb[:, : WP + 2], 0.0)
            edge = bsb[:, 97 : 97 + 97 * WP].rearrange("c (r w) -> c r w", w=WP)
            nc.gpsimd.memset(edge[:, :, 0:2], 0.0)
            nc.gpsimd.memset(bsb[:, 97 * WP :], 0.0)

            b3d = bsb[:, :NPAD].rearrange("c (r w) -> c r w", w=WP)
            f3 = featp_sb[:].rearrange("c (r w) -> c r w", w=WP)
            w3 = wgtp_sb[:].rearrange("c (r w) -> c r w", w=WP)
            wq2 = wgtq2_sb[:].rearrange("c (r w) -> c r w", w=WP)

            # ---- fused pipeline: conv rows (phase A) stream in between the
            # software-pipelined per-group stages of phase B.
            with (
                tc.tile_pool(name="hmp", bufs=3) as hmp,
                tc.tile_pool(name="ftp", bufs=3) as ftp,
                tc.tile_pool(name="svsp", bufs=4) as svsp,
                tc.tile_pool(name="fbp", bufs=3) as fbp,
                tc.tile_pool(name="dgp", bufs=3) as dgp,
                tc.tile_pool(name="bop", bufs=3) as bop,
                tc.tile_pool(name="orp", bufs=2) as orp,
                tc.tile_pool(name="psB2", bufs=1, space="PSUM") as psB2,
            ):
                fbS_r, dg_r, boS_r, pso_r, acc_r = {}, {}, {}, {}, {}
                # two pso slots manually ring-buffered inside ONE psum bank
                psoD = psB2.tile([C, 2 * GP], fp32, tag="pso", bufs=1,
                                 name="psoD")
                hm_r = {}
                svS_r, ft_r = {}, {}
                orow_bufs = {}

                def emit_arow_f(t):
                    r0 = t * RT
                    ps = psB2.tile([C, FT], fp32, tag="psA", bufs=1, name="ps")
                    for kk, (i, j) in enumerate(product(range(3), range(3))):
                        nc.tensor.matmul(
                            ps[:],
                            w1f_sb[:, (i * 3 + j) * C : (i * 3 + j + 1) * C],
                            f3[:, r0 + i : r0 + i + RT, j : j + W],
                            start=(kk == 0),
                            stop=False,
                        )
                    return ps

                def emit_arow_w(t, ps):
                    r0 = t * RT
                    for j in range(3):
                        nc.tensor.matmul(
                            ps[:],
                            w1wq_sb[:, j * C : (j + 1) * C],
                            w3[:, r0 : r0 + RT, j : j + W],
                            start=False,
                            stop=False,
                        )
                    nc.tensor.matmul(
                        ps[:],
                        w1wq2_sb,
                        wq2[:, r0 + 2 : r0 + 2 + RT, 0 : W],
                        start=False,
                        stop=False,
                    )
                    nc.tensor.matmul(
                        ps[:],
                        w1w_sb[:],
                        w3[:64, r0 + 2 : r0 + 2 + RT, 2 : 2 + W],
                        start=False,
                        stop=True,
                    )
                    hm = hmp.tile([C, FT], fp16, tag="hm")
                    nc.scalar.activation(hm[:], ps[:], Tanh, bias=b1_sb)
                    hm_r[t] = hm

                def emit_arow_b(t):
                    r0 = t * RT
                    hm = hm_r.pop(t)
                    ps2 = psB2.tile([NBT, FT], fp32, tag="psB", bufs=1, name="ps2")
                    nc.tensor.matmul(ps2[:], w2_sb, hm[:], start=True, stop=True)
                    nc.scalar.activation(
                        b3d[:, r0 + 1 : r0 + 1 + RT, 1 : 1 + W],
                        ps2[:].rearrange("c (r w) -> c r w", w=W),
                        Tanh,
                        bias=b2_sb,
                    )

                def emit_sv(g):
                    # b rows (k-major: row = k*6+m) -> per-pixel scales in
                    # (k-parity*64+p, kpair*6+m) layout: the two transposes
                    # use column-selector matrices (even/odd k) so svp lands
                    # pre-interleaved; one contiguous copy evacuates it
                    svp = psB2.tile([C, 18], fp32, tag="svp", bufs=1,
                                    name="svp")
                    src = bsb[:, 1 + g * GP : 1 + (g + 1) * GP]
                    nc.tensor.matmul(svp[:GP, :], src, p1_sb,
                                     start=True, stop=True)
                    nc.tensor.matmul(svp[GP:, :], src, p2_sb,
                                     start=True, stop=True)
                    svS = svsp.tile([C, 18], fp32, tag="svS")
                    svS_r[g] = svS
                    nc.vector.tensor_copy(svS[:], svp[:])

                def emit_builds(g, dg, js):
                    svS = svS_r[g]
                    for j in js:
                        m, kp = divmod(j, 3)
                        sc = svS[:, kp * NB + m : kp * NB + m + 1]
                        dslice = dg[:, j * GP : (j + 1) * GP]
                        eng = _B18[j]
                        if eng == "D":
                            nc.vector.tensor_scalar(
                                dslice, mask2_sb, sc, None, MUL
                            )
                        elif eng == "P":
                            nc.gpsimd.tensor_scalar(
                                dslice, mask2_sb, sc, None, MUL
                            )
                        else:
                            nc.scalar.activation(dslice, mask2_sb, Copy,
                                                 scale=sc)

                # conv row-tile slots spread at cadence ~6.5 so conv filler
                # lasts until iteration ~138 (row 23 deadline is ~141)
                _fs = {int(_r * 6.5 + 0.5): _r + 2 for _r in range(NT - 2)}
                _ws = {k + 1: v for k, v in _fs.items()}
                _bs = {k + 2: v for k, v in _fs.items()}

                for r in range(2):
                    emit_arow_w(r, emit_arow_f(r))
                    emit_arow_b(r)
                # fTd chunks 0,1 were DMA'd in the startup sequence
                ft_r[0] = ft0_sb
                ft_r[1] = ft1_sb
                emit_sv(0)
                emit_sv(1)

                for i in range(NG + 3):
                    if i in _fs:
                        arow_ps = emit_arow_f(_fs[i])
                    elif i in _ws:
                        emit_arow_w(_ws[i], arow_ps)
                    elif i in _bs:
                        emit_arow_b(_bs[i])
                    # acc(i-2) psum -> SBUF (Pool)
                    if 0 <= i - 2 < NG:
                        boS = bop.tile([C, NB * GP], fp16, tag="boS")
                        boS_r[i - 2] = boS
                        acc = acc_r.pop(i - 2)
                        nc.vector.tensor_copy(boS[:, : 3 * GP], acc[:, : 3 * GP])
                        nc.scalar.copy(boS[:, 3 * GP :], acc[:, 3 * GP :])
                    # orow(i-3) + store
                    if 0 <= i - 3 < NG:
                        j = i - 3
                        g8 = j % OG
                        if g8 == 0:
                            orow_bufs[j] = orp.tile(
                                [C, OG * GP], fp16, tag="orow", name="orow_buf"
                            )
                        ob = orow_bufs[j - g8]
                        nc.scalar.activation(
                            ob[:, g8 * GP : (g8 + 1) * GP], pso_r.pop(j),
                            Ident, bias=b3_sb,
                        )
                        if g8 == OG - 1 or j == NG - 1:
                            t0 = j - g8
                            st0 = 4 * GP if j == NG - 1 else 0
                            nc.sync.dma_start(
                                out[:, 1 + t0 * GP + st0
                                    : 1 + t0 * GP + (g8 + 1) * GP],
                                ob[:, st0 : (g8 + 1) * GP],
                            )
                            del orow_bufs[t0]
                        elif j == NG - 5 and g8 == 3:
                            t0 = j - g8
                            nc.sync.dma_start(
                                out[:, 1 + t0 * GP : 1 + t0 * GP + 4 * GP],
                                ob[:, : 4 * GP],
                            )
                    # stream next fTd chunk
                    if i % 8 == 0 and i // 8 + 2 < NCH:
                        cch = i // 8 + 2
                        ft = ftp.tile([C, FCH8], fp16, tag="ft")
                        ft_r[cch] = ft
                        nc.sync.dma_start(
                            ft[:], fTd[:, cch * FCH8 : (cch + 1) * FCH8]
                        )
                    # per-pixel scales two groups ahead
                    if i + 2 < NG:
                        emit_sv(i + 2)
                    # F(i), blockdiag builds(i), F evac(i)
                    if i < NG:
                        psf = psB2.tile([C, 3 * C], fp32, tag="psf", bufs=2,
                                        name="psf")
                        ft = ft_r[i // 8]
                        for kp in range(3):
                            for ci in range(2):
                                nc.tensor.matmul(
                                    psf[:, kp * C : (kp + 1) * C],
                                    bnd2_sb[:, (kp * 2 + ci) * C
                                            : (kp * 2 + ci + 1) * C],
                                    ft[:, ((i % 8) * 2 + ci) * C
                                       : ((i % 8) * 2 + ci + 1) * C],
                                    start=(ci == 0),
                                    stop=(ci == 1),
                                )
                        dg = dgp.tile([C, 18 * GP], fp16, tag="dg")
                        dg_r[i] = dg
                        emit_builds(i, dg, range(9))
                        fbS = fbp.tile([C, 3 * C], fp16, tag="fbS")
                        nc.scalar.copy(fbS[:], psf[:])
                        fbS_r[i] = fbS
                        emit_builds(i, dg, range(9, 18))
                        if i % 8 == 7 or i == NG - 1:
                            ft_r.pop(i // 8)
                    # blockdiag matmuls (i-1): acc_m = sum_kp fbS_kp^T @ bd2
                    if 0 <= i - 1 < NG:
                        acc = psB2.tile([C, NB * GP], fp32, tag="accT", bufs=2,
                                        name="acc")
                        acc_r[i - 1] = acc
                        fbS = fbS_r.pop(i - 1)
                        dg = dg_r.pop(i - 1)
                        for m in range(NB):
                            for kp in range(3):
                                j = m * 3 + kp
                                nc.tensor.matmul(
                                    acc[:, m * GP : (m + 1) * GP],
                                    fbS[:, kp * C : (kp + 1) * C],
                                    dg[:, j * GP : (j + 1) * GP],
                                    start=(kp == 0),
                                    stop=(kp == 2),
                                )
                    # coef matmuls (i-2)
                    if 0 <= i - 2 < NG:
                        _s = ((i - 2) % 2) * GP
                        pso = psoD[:, _s : _s + GP]
                        pso_r[i - 2] = pso
                        boS = boS_r.pop(i - 2)
                        for m in range(NB):
                            nc.tensor.matmul(
                                pso[:],
                                coefT_sb[:, m * C : (m + 1) * C],
                                boS[:, m * GP : (m + 1) * GP],
                                start=(m == 0),
                                stop=(m == NB - 1),
                            )

    nc.compile()
    return nc


def _get_nc():
    if "nc" not in _CACHE:
        _CACHE["nc"] = build_nc()
    return _CACHE["nc"]


def _prep_maps(feat, weight, conv1_w, conv1_b, conv2_w, conv2_b, bases_buf, coef, bias):
    feat = np.asarray(feat, np.float32)
    weight = np.asarray(weight, np.float32)
    conv1_w = np.asarray(conv1_w, np.float32)
    conv2_w = np.asarray(conv2_w, np.float32)
    bases_buf = np.asarray(bases_buf, np.float32)
    coef = np.asarray(coef, np.float32)

    n = feat.shape[0]
    featp = np.zeros((n, C, HP, WP), np.float16)
    featp[:, :, 1 : H + 1, 1 : W + 1] = feat
    wgtp = np.zeros((n, CW, HP, WP), np.float16)
    wgtp[:, :, 1 : H + 1, 1 : W + 1] = weight

    # host-prepped transposed feature chunks: the 3 disjoint di input
    # windows (66 px each) packed into 2 chunks of 128 rows per group
    # fTd[q, (g*2+ci)*C + c] = fe[c, FOFF + 1 + g*GP + RELS[ci][q]]
    rels1 = list(range(-99, -33)) + list(range(-1, 61))
    rels2 = list(range(61, 65)) + list(range(97, 163))
    rels2 = rels2 + [163] * (C - len(rels2))
    RELS = [np.array(rels1), np.array(rels2)]
    fe = np.zeros((n, C, FEXT), np.float16)
    fe[:, :, FOFF : FOFF + NPAD] = featp.reshape(n, C, NPAD)
    fTdh = np.empty((n, 2 * NG, C, C), np.float16)
    for g in range(NG):
        for ci in range(2):
            idx = FOFF + 1 + g * GP + RELS[ci]
            fTdh[:, g * 2 + ci] = fe[:, :, idx].transpose(0, 2, 1)
    fTdh = np.ascontiguousarray(
        fTdh.transpose(0, 2, 1, 3).reshape(n, C, 2 * NG * C)
    )

    w1f = np.ascontiguousarray(
        conv1_w[:, :C].transpose(1, 2, 3, 0).reshape(C, L * C)
    ).astype(np.float16)
    w1w = np.ascontiguousarray(
        conv1_w[:, C:].transpose(1, 2, 3, 0).reshape(CW, L * C)
    ).astype(np.float16)
    # conv2 weights with k-major output-channel ordering (col = k*NB + m)
    w2h = np.ascontiguousarray(conv2_w[:, :, 0, 0].T).astype(np.float16)
    w2i = np.ascontiguousarray(
        w2h.reshape(C, NB, TEM).transpose(0, 2, 1).reshape(C, NBT)
    )
    # packed band blocks: bnd2[q, (kp*2+ci)*C + ki*GP + p] accumulates
    # bases_buf[2*kp+ki, di*3+dj] where chunk ci row q holds input offset
    # rel = p + (di-1)*WP + dj - 1
    pos = []
    for rels in RELS:
        d = {}
        for q, r in enumerate(rels.tolist()):
            if r not in d:
                d[r] = q
        pos.append(d)
    bndh = np.zeros((C, 3, 2, 2, GP), np.float32)
    for kp in range(3):
        for ki in range(2):
            k = 2 * kp + ki
            for di in range(3):
                for dj in range(3):
                    for p in range(GP):
                        rel = p + (di - 1) * WP + dj - 1
                        ci = 0 if rel in pos[0] else 1
                        bndh[pos[ci][rel], kp, ci, ki, p] += \
                            bases_buf[k, di * 3 + dj]
    bndh = bndh.reshape(C, 6 * C).astype(np.float16)
    # mask2: ones at (p,p) and (GP+p, p)
    mask2 = np.zeros((C, GP), np.float16)
    for p in range(GP):
        mask2[p, p] = 1.0
        mask2[GP + p, p] = 1.0
    coefTh = np.ascontiguousarray(
        coef[:, :, 0, 0].reshape(C, C, NB).transpose(1, 2, 0).reshape(C, NB * C)
    ).astype(np.float16)
    b1h = np.asarray(conv1_b, np.float32).reshape(C, 1)
    b2h = np.asarray(conv2_b, np.float32).reshape(NB, TEM).T.reshape(NBT, 1)
    b3h = np.asarray(bias, np.float32).reshape(C, 1)

    wgtq = np.zeros((n, C, NPAD), np.float16)
    wgtq[:, :CW] = wgtp.reshape(n, CW, NPAD)
    wgtq[:, CW:, : NPAD - WP] = wgtp.reshape(n, CW, NPAD)[:, :, WP:]
    # paired weights: rows 0-63 = tap (0,j), rows 64-127 = tap (1,j)
    wgtq2 = np.zeros((n, C, NPAD), np.float16)
    wgtq2[:, :CW] = wgtp.reshape(n, CW, NPAD)
    wgtq2[:, CW:, : NPAD - 1] = wgtp.reshape(n, CW, NPAD)[:, :, 1:]
    w1wq2 = np.concatenate(
        [
            w1w.reshape(CW, 3, 3, C)[:, 2, 0],
            w1w.reshape(CW, 3, 3, C)[:, 2, 1],
        ],
        axis=0,
    ).reshape(C, C)
    w1wq = np.concatenate(
        [
            w1w.reshape(CW, 3, 3, C)[:, 0],
            w1w.reshape(CW, 3, 3, C)[:, 1],
        ],
        axis=0,
    ).reshape(C, 3 * C)
    pkw = np.concatenate([w1wq, w1wq2], axis=1)
    # transpose column selectors: P1 even-k cols, P2 odd-k cols
    p1 = np.zeros((C, 18), np.float16)
    p2 = np.zeros((C, 18), np.float16)
    for kp in range(3):
        for m in range(NB):
            p1[kp * 12 + m, kp * NB + m] = 1.0
            p2[kp * 12 + NB + m, kp * NB + m] = 1.0
    pk = np.concatenate([w2i, coefTh, bndh, mask2, p1, p2], axis=1)
    pb = np.zeros((C, 3), np.float32)
    pb[:, 0:1] = b1h
    pb[:, 1:2] = b3h
    pb[:NBT, 2:3] = b2h
    shared = {"w1f": w1f, "pkw": pkw, "pk": pk,
              "w1w": np.ascontiguousarray(w1w[:, 8 * C:]), "pb": pb}
    return [
        {
            "featp": featp[i].reshape(C, NPAD).astype(np.float16),
            "wgtq": wgtq[i], "wgtq2": wgtq2[i],
            "fTd": fTdh[i],
            **shared,
        }
        for i in range(n)
    ]


def kernel(feat, weight, conv1_w, conv1_b, conv2_w, conv2_b, bases_buf, coef, bias,
           **run_kwargs):
    in_maps = _prep_maps(
        feat, weight, conv1_w, conv1_b, conv2_w, conv2_b, bases_buf, coef, bias
    )
    res = run_bass_kernel_spmd(
        _get_nc(), in_maps, core_ids=list(range(len(in_maps))), **run_kwargs
    )
    outp = np.stack([r["out"] for r in res.results], 0).astype(np.float32)
    outp = outp[:, :, :NPAD].reshape(-1, C, HP, WP)[:, :, 1 : H + 1, 1 : W + 1]
    _CACHE["last_results"] = res
    return np.ascontiguousarray(outp)


# revision 31
# speedup vs baseline: 1.5513x; 1.0203x over previous
"""Trainium2 Bass kernel for the DCF (dynamic conv filter) module.

Sharding: pure data-parallel over batch N=8 across 8 NeuronCores (one image
per core); all parameters replicated.

Pipeline per core (one 128x96x96 image):
  A:  conv1 (3x3, 192->128) + tanh -> hmid;  conv2 (1x1, 128->36) + tanh -> b
  B:  per 64-pixel group g:
        - SV: double transpose of the b rows + strided half-copies give the
          per-pixel scales in (k-parity x 64px, k-pair*6+m) layout
        - F: fixed-basis convs of feat via k-pair-interleaved banded matmuls
          on host-prepped transposed feature chunks (fTd), 3 di accumulated;
          output partitions are (k-parity*64 + pixel)
        - acc_m = sum_kpair fbS_kpair^T @ blockdiag2(s) -- each matmul
          contracts 2 k's at once (k lives in the contraction dim), so the
          k-reduction costs half the rows of the per-(m,k) diag scheme
        - out_group = sum_m coef_m @ acc_m (+bias)

Block-diagonals are built as tensor_scalar(mask2 * s) (DVE 4x mode); acc and
pso are double-buffered in PSUM so evacuation copies never stall the PE."""

from itertools import product

import numpy as np

import concourse.bass as bass
import concourse.tile as tile
from concourse import bacc, mybir
from concourse.bass_utils import run_bass_kernel_spmd
from concourse.masks import make_identity

fp16 = mybir.dt.float16
fp32 = mybir.dt.float32

N_CORES = 8
C = 128
CW = 64
H = W = 96
HP = WP = 98
NPIX = H * W
NPAD = HP * WP  # 9604
NB = 6
TEM = 6
L = 9
NBT = NB * TEM  # 36
RT = 4
FT = RT * W  # 384
NT = H // RT  # 24
GP = 64           # output pixels per group
NG = 152          # groups (cover padded idx 1 .. 1+152*64 = 9729)
BP = 9732         # padded bsb/out length
FEXT = 10000      # extended (host-side) padded feat length for fTd windows
FOFF = 98         # fTd window base offset inside the extended buffer
OG = 8            # output groups per store
FCH8 = 8 * 2 * C  # fTd cols per streamed chunk (8 groups)
NCH = (NG + 7) // 8  # 19 chunks

# build engine mix for the 18 blockdiag builds per group: 12 DVE (86ns in 4x
# mode), 4 Pool, 2 Act
_B18 = ["D", "P", "D", "P", "D", "P", "D", "D", "D",
        "P", "P", "D", "D", "P", "D", "P", "D", "D"]

_CACHE = {}


def build_nc():
    nc = bacc.Bacc("TRN2", target_bir_lowering=False, debug=False)

    featp = nc.dram_tensor("featp", [C, NPAD], fp16, kind="ExternalInput").ap()
    wgtq = nc.dram_tensor("wgtq", [C, NPAD], fp16, kind="ExternalInput").ap()
    fTd = nc.dram_tensor("fTd", [C, 2 * NG * C], fp16, kind="ExternalInput").ap()
    wgtq2 = nc.dram_tensor("wgtq2", [C, NPAD], fp16, kind="ExternalInput").ap()
    w1f = nc.dram_tensor("w1f", [C, L * C], fp16, kind="ExternalInput").ap()
    # conv1 weight-branch params: w1wq|w1wq2 = 384+128
    pkw = nc.dram_tensor("pkw", [C, 512], fp16, kind="ExternalInput").ap()
    # fp16 params packed: w2|coefT|bnd2|mask2|P1|P2 = 36+768+768+64+18+18
    pk = nc.dram_tensor("pk", [C, 1672], fp16, kind="ExternalInput").ap()
    w1w = nc.dram_tensor("w1w", [CW, C], fp16, kind="ExternalInput").ap()
    pb = nc.dram_tensor("pb", [C, 3], fp32, kind="ExternalInput").ap()
    out = nc.dram_tensor("out", [C, BP], fp16, kind="ExternalOutput").ap()

    Tanh = mybir.ActivationFunctionType.Tanh
    Ident = mybir.ActivationFunctionType.Identity
    Copy = mybir.ActivationFunctionType.Copy
    MUL = mybir.AluOpType.mult
    ADD = mybir.AluOpType.add

    with tile.TileContext(nc) as tc:
        with (
            tc.tile_pool(name="const", bufs=1) as const,
            tc.tile_pool(name="big", bufs=1) as big,
        ):
            featp_sb = big.tile([C, NPAD], fp16)
            wgtp_sb = big.tile([C, NPAD], fp16)
            cuts = [0, 2404, 4808, 7212, NPAD]
            # startup: tiny "head" pieces first so the first conv row-tile's
            # operands land within ~2.5us (DMA pipeline latency floor)
            HD = 686  # cols covering feat/wgt rows 0..6 (row-tile 0 + halo)
            w1f_sb = const.tile([C, L * C], fp16)
            nc.sync.dma_start(w1f_sb[:, : 4 * C], w1f[:, : 4 * C])
            nc.sync.dma_start(featp_sb[:, :HD], featp[:, :HD])
            nc.sync.dma_start(w1f_sb[:, 4 * C :], w1f[:, 4 * C :])
            pkw_sb = const.tile([C, 512], fp16)
            nc.sync.dma_start(pkw_sb[:], pkw)
            w1wq_sb = pkw_sb[:, 0:384]
            w1wq2_sb = pkw_sb[:, 384:512]
            wgtq2_sb = big.tile([C, NPAD], fp16)
            nc.sync.dma_start(wgtp_sb[:, :HD], wgtq[:, :HD])
            nc.sync.dma_start(wgtq2_sb[:, :HD], wgtq2[:, :HD])
            pb_sb = const.tile([C, 3], fp32)
            nc.sync.dma_start(pb_sb[:], pb)
            b1_sb = pb_sb[:, 0:1]
            b3_sb = pb_sb[:, 1:2]
            b2_sb = pb_sb[:NBT, 2:3]
            w1w_sb = const.tile([CW, C], fp16)
            nc.sync.dma_start(w1w_sb[:], w1w)
            nc.sync.dma_start(featp_sb[:, HD : cuts[1]], featp[:, HD : cuts[1]])
            pk_sb = const.tile([C, 1672], fp16)
            nc.sync.dma_start(pk_sb[:, 804:], pk[:, 804:])
            nc.sync.dma_start(pk_sb[:, :804], pk[:, :804])
            ft0_sb = big.tile([C, FCH8], fp16)
            nc.sync.dma_start(ft0_sb[:], fTd[:, :FCH8])
            w2_sb = pk_sb[:, 0:36]
            coefT_sb = pk_sb[:, 36:804]
            bnd2_sb = pk_sb[:, 804:1572]
            mask2_sb = pk_sb[:, 1572:1636]
            p1_sb = pk_sb[:NBT, 1636:1654]
            p2_sb = pk_sb[:NBT, 1654:1672]
            nc.sync.dma_start(wgtq2_sb[:, HD : cuts[1]], wgtq2[:, HD : cuts[1]])
            ft1_sb = big.tile([C, FCH8], fp16)
            nc.sync.dma_start(ft1_sb[:], fTd[:, FCH8 : 2 * FCH8])
            nc.sync.dma_start(wgtp_sb[:, HD : cuts[2]], wgtq[:, HD : cuts[2]])
            nc.sync.dma_start(
                featp_sb[:, cuts[1] : cuts[2]], featp[:, cuts[1] : cuts[2]]
            )
            nc.sync.dma_start(wgtq2_sb[:, cuts[1] :], wgtq2[:, cuts[1] :])
            nc.sync.dma_start(
                featp_sb[:, cuts[2] : cuts[3]], featp[:, cuts[2] : cuts[3]]
            )
            nc.sync.dma_start(wgtp_sb[:, cuts[2] :], wgtq[:, cuts[2] :])
            nc.sync.dma_start(
                featp_sb[:, cuts[3] :], featp[:, cuts[3] :]
            )
            bsb = big.tile([NBT, BP], fp16)
            # zero only the border/tail cells conv2 never writes (full memset
            # would hold Pool for 8us before the first b write)
            nc.gpsimd.memset(bsb[:, : WP + 2], 0.0)
            edge = bsb[:, 97 : 97 + 97 * WP].rearrange("c (r w) -> c r w", w=WP)
            nc.gpsimd.memset(edge[:, :, 0:2], 0.0)
            nc.gpsimd.memset(bsb[:, 97 * WP :], 0.0)

            b3d = bsb[:, :NPAD].rearrange("c (r w) -> c r w", w=WP)
            f3 = featp_sb[:].rearrange("c (r w) -> c r w", w=WP)
            w3 = wgtp_sb[:].rearrange("c (r w) -> c r w", w=WP)
            wq2 = wgtq2_sb[:].rearrange("c (r w) -> c r w", w=WP)

            # ---- fused pipeline: conv rows (phase A) stream in between the
            # software-pipelined per-group stages of phase B.
            with (
                tc.tile_pool(name="hmp", bufs=3) as hmp,
                tc.tile_pool(name="ftp", bufs=3) as ftp,
                tc.tile_pool(name="svsp", bufs=4) as svsp,
                tc.tile_pool(name="fbp", bufs=3) as fbp,
                tc.tile_pool(name="dgp", bufs=3) as dgp,
                tc.tile_pool(name="bop", bufs=3) as bop,
                tc.tile_pool(name="orp", bufs=2) as orp,
                tc.tile_pool(name="psB2", bufs=1, space="PSUM") as psB2,
            ):
                fbS_r, dg_r, boS_r, pso_r, acc_r = {}, {}, {}, {}, {}
                # two pso slots manually ring-buffered inside ONE psum bank
                psoD = psB2.tile([C, 2 * GP], fp32, tag="pso", bufs=1,
                                 name="psoD")
                hm_r = {}
                svS_r, ft_r = {}, {}
                orow_bufs = {}

                def emit_arow_f(t):
                    r0 = t * RT
                    ps = psB2.tile([C, FT], fp32, tag="psA", bufs=1, name="ps")
                    for kk, (i, j) in enumerate(product(range(3), range(3))):
                        nc.tensor.matmul(
                            ps[:],
                            w1f_sb[:, (i * 3 + j) * C : (i * 3 + j + 1) * C],
                            f3[:, r0 + i : r0 + i + RT, j : j + W],
                            start=(kk == 0),
                            stop=False,
                        )
                    return ps

                def emit_arow_w(t, ps):
                    r0 = t * RT
                    for j in range(3):
                        nc.tensor.matmul(
                            ps[:],
                            w1wq_sb[:, j * C : (j + 1) * C],
                            w3[:, r0 : r0 + RT, j : j + W],
                            start=False,
                            stop=False,
                        )
                    nc.tensor.matmul(
                        ps[:],
                        w1wq2_sb,
                        wq2[:, r0 + 2 : r0 + 2 + RT, 0 : W],
                        start=False,
                        stop=False,
                    )
                    nc.tensor.matmul(
                        ps[:],
                        w1w_sb[:],
                        w3[:64, r0 + 2 : r0 + 2 + RT, 2 : 2 + W],
                        start=False,
                        stop=True,
                    )
                    hm = hmp.tile([C, FT], fp16, tag="hm")
                    nc.scalar.activation(hm[:], ps[:], Tanh, bias=b1_sb)
                    hm_r[t] = hm

                def emit_arow_b(t):
                    r0 = t * RT
                    hm = hm_r.pop(t)
                    ps2 = psB2.tile([NBT, FT], fp32, tag="psB", bufs=1, name="ps2")
                    nc.tensor.matmul(ps2[:], w2_sb, hm[:], start=True, stop=True)
                    nc.scalar.activation(
                        b3d[:, r0 + 1 : r0 + 1 + RT, 1 : 1 + W],
                        ps2[:].rearrange("c (r w) -> c r w", w=W),
                        Tanh,
                        bias=b2_sb,
                    )

                def emit_sv(g):
                    # b rows (k-major: row = k*6+m) -> per-pixel scales in
                    # (k-parity*64+p, kpair*6+m) layout: the two transposes
                    # use column-selector matrices (even/odd k) so svp lands
                    # pre-interleaved; one contiguous copy evacuates it
                    svp = psB2.tile([C, 18], fp32, tag="svp", bufs=1,
                                    name="svp")
                    src = bsb[:, 1 + g * GP : 1 + (g + 1) * GP]
                    nc.tensor.matmul(svp[:GP, :], src, p1_sb,
                                     start=True, stop=True)
                    nc.tensor.matmul(svp[GP:, :], src, p2_sb,
                                     start=True, stop=True)
                    svS = svsp.tile([C, 18], fp32, tag="svS")
                    svS_r[g] = svS
                    nc.vector.tensor_copy(svS[:], svp[:])

                def emit_builds(g, dg, js):
                    svS = svS_r[g]
                    for j in js:
                        m, kp = divmod(j, 3)
                        sc = svS[:, kp * NB + m : kp * NB + m + 1]
                        dslice = dg[:, j * GP : (j + 1) * GP]
                        eng = _B18[j]
                        if eng == "D":
                            nc.vector.tensor_scalar(
                                dslice, mask2_sb, sc, None, MUL
                            )
                        elif eng == "P":
                            nc.gpsimd.tensor_scalar(
                                dslice, mask2_sb, sc, None, MUL
                            )
                        else:
                            nc.scalar.activation(dslice, mask2_sb, Copy,
                                                 scale=sc)

                # conv row-tile slots spread at cadence ~6.5 so conv filler
                # lasts until iteration ~138 (row 23 deadline is ~141)
                _fs = {int(_r * 6.5 + 0.5): _r + 2 for _r in range(NT - 2)}
                _ws = {k + 1: v for k, v in _fs.items()}
                _bs = {k + 2: v for k, v in _fs.items()}

                for r in range(2):
                    emit_arow_w(r, emit_arow_f(r))
                    emit_arow_b(r)
                # fTd chunks 0,1 were DMA'd in the startup sequence
                ft_r[0] = ft0_sb
                ft_r[1] = ft1_sb
                emit_sv(0)
                emit_sv(1)

                for i in range(NG + 4):
                    if i in _fs:
                        arow_ps = emit_arow_f(_fs[i])
                    elif i in _ws:
                        emit_arow_w(_ws[i], arow_ps)
                    elif i in _bs:
                        emit_arow_b(_bs[i])
                    # acc(i-2) psum -> SBUF (Pool)
                    if 0 <= i - 2 < NG:
                        boS = bop.tile([C, NB * GP], fp16, tag="boS")
                        boS_r[i - 2] = boS
                        acc = acc_r.pop(i - 2)
                        nc.vector.tensor_copy(boS[:, : 3 * GP], acc[:, : 3 * GP])
                        nc.scalar.copy(boS[:, 3 * GP :], acc[:, 3 * GP :])
                    # orow(i-3) + store
                    if 0 <= i - 4 < NG:
                        j = i - 4
                        g8 = j % OG
                        if g8 == 0:
                            orow_bufs[j] = orp.tile(
                                [C, OG * GP], fp16, tag="orow", name="orow_buf"
                            )
                        ob = orow_bufs[j - g8]
                        nc.scalar.activation(
                            ob[:, g8 * GP : (g8 + 1) * GP], pso_r.pop(j),
                            Ident, bias=b3_sb,
                        )
                        if g8 == OG - 1 or j == NG - 1:
                            t0 = j - g8
                            st0 = 4 * GP if j == NG - 1 else 0
                            nc.sync.dma_start(
                                out[:, 1 + t0 * GP + st0
                                    : 1 + t0 * GP + (g8 + 1) * GP],
                                ob[:, st0 : (g8 + 1) * GP],
                            )
                            del orow_bufs[t0]
                        elif j == NG - 5 and g8 == 3:
                            t0 = j - g8
                            nc.sync.dma_start(
                                out[:, 1 + t0 * GP : 1 + t0 * GP + 4 * GP],
                                ob[:, : 4 * GP],
                            )
                    # stream next fTd chunk
                    if i % 8 == 0 and i // 8 + 2 < NCH:
                        cch = i // 8 + 2
                        ft = ftp.tile([C, FCH8], fp16, tag="ft")
                        ft_r[cch] = ft
                        nc.sync.dma_start(
                            ft[:], fTd[:, cch * FCH8 : (cch + 1) * FCH8]
                        )
                    # per-pixel scales two groups ahead
                    if i + 2 < NG:
                        emit_sv(i + 2)
                    # F(i), blockdiag builds(i), F evac(i)
                    if i < NG:
                        psf = psB2.tile([C, 3 * C], fp32, tag="psf", bufs=2,
                                        name="psf")
                        ft = ft_r[i // 8]
                        for kp in range(3):
                            for ci in range(2):
                                nc.tensor.matmul(
                                    psf[:, kp * C : (kp + 1) * C],
                                    bnd2_sb[:, (kp * 2 + ci) * C
                                            : (kp * 2 + ci + 1) * C],
                                    ft[:, ((i % 8) * 2 + ci) * C
                                       : ((i % 8) * 2 + ci + 1) * C],
                                    start=(ci == 0),
                                    stop=(ci == 1),
                                )
                        dg = dgp.tile([C, 18 * GP], fp16, tag="dg")
                        dg_r[i] = dg
                        emit_builds(i, dg, range(9))
                        fbS = fbp.tile([C, 3 * C], fp16, tag="fbS")
                        nc.scalar.copy(fbS[:], psf[:])
                        fbS_r[i] = fbS
                        emit_builds(i, dg, range(9, 18))
                        if i % 8 == 7 or i == NG - 1:
                            ft_r.pop(i // 8)
                    # blockdiag matmuls (i-1): acc_m = sum_kp fbS_kp^T @ bd2
                    if 0 <= i - 1 < NG:
                        acc = psB2.tile([C, NB * GP], fp32, tag="accT", bufs=2,
                                        name="acc")
                        acc_r[i - 1] = acc
                        fbS = fbS_r.pop(i - 1)
                        dg = dg_r.pop(i - 1)
                        for m in range(NB):
                            for kp in range(3):
                                j = m * 3 + kp
                                nc.tensor.matmul(
                                    acc[:, m * GP : (m + 1) * GP],
                                    fbS[:, kp * C : (kp + 1) * C],
                                    dg[:, j * GP : (j + 1) * GP],
                                    start=(kp == 0),
                                    stop=(kp == 2),
                                )
                    # coef matmuls (i-2)
                    if 0 <= i - 3 < NG:
                        _s = ((i - 3) % 2) * GP
                        pso = psoD[:, _s : _s + GP]
                        pso_r[i - 3] = pso
                        boS = boS_r.pop(i - 3)
                        for m in range(NB):
                            nc.tensor.matmul(
                                pso[:],
                                coefT_sb[:, m * C : (m + 1) * C],
                                boS[:, m * GP : (m + 1) * GP],
                                start=(m == 0),
                                stop=(m == NB - 1),
                            )

    nc.compile()
    return nc


def _get_nc():
    if "nc" not in _CACHE:
        _CACHE["nc"] = build_nc()
    return _CACHE["nc"]


def _prep_maps(feat, weight, conv1_w, conv1_b, conv2_w, conv2_b, bases_buf, coef, bias):
    feat = np.asarray(feat, np.float32)
    weight = np.asarray(weight, np.float32)
    conv1_w = np.asarray(conv1_w, np.float32)
    conv2_w = np.asarray(conv2_w, np.float32)
    bases_buf = np.asarray(bases_buf, np.float32)
    coef = np.asarray(coef, np.float32)

    n = feat.shape[0]
    featp = np.zeros((n, C, HP, WP), np.float16)
    featp[:, :, 1 : H + 1, 1 : W + 1] = feat
    wgtp = np.zeros((n, CW, HP, WP), np.float16)
    wgtp[:, :, 1 : H + 1, 1 : W + 1] = weight

    # host-prepped transposed feature chunks: the 3 disjoint di input
    # windows (66 px each) packed into 2 chunks of 128 rows per group
    # fTd[q, (g*2+ci)*C + c] = fe[c, FOFF + 1 + g*GP + RELS[ci][q]]
    rels1 = list(range(-99, -33)) + list(range(-1, 61))
    rels2 = list(range(61, 65)) + list(range(97, 163))
    rels2 = rels2 + [163] * (C - len(rels2))
    RELS = [np.array(rels1), np.array(rels2)]
    fe = np.zeros((n, C, FEXT), np.float16)
    fe[:, :, FOFF : FOFF + NPAD] = featp.reshape(n, C, NPAD)
    fTdh = np.empty((n, 2 * NG, C, C), np.float16)
    for g in range(NG):
        for ci in range(2):
            idx = FOFF + 1 + g * GP + RELS[ci]
            fTdh[:, g * 2 + ci] = fe[:, :, idx].transpose(0, 2, 1)
    fTdh = np.ascontiguousarray(
        fTdh.transpose(0, 2, 1, 3).reshape(n, C, 2 * NG * C)
    )

    w1f = np.ascontiguousarray(
        conv1_w[:, :C].transpose(1, 2, 3, 0).reshape(C, L * C)
    ).astype(np.float16)
    w1w = np.ascontiguousarray(
        conv1_w[:, C:].transpose(1, 2, 3, 0).reshape(CW, L * C)
    ).astype(np.float16)
    # conv2 weights with k-major output-channel ordering (col = k*NB + m)
    w2h = np.ascontiguousarray(conv2_w[:, :, 0, 0].T).astype(np.float16)
    w2i = np.ascontiguousarray(
        w2h.reshape(C, NB, TEM).transpose(0, 2, 1).reshape(C, NBT)
    )
    # packed band blocks: bnd2[q, (kp*2+ci)*C + ki*GP + p] accumulates
    # bases_buf[2*kp+ki, di*3+dj] where chunk ci row q holds input offset
    # rel = p + (di-1)*WP + dj - 1
    pos = []
    for rels in RELS:
        d = {}
        for q, r in enumerate(rels.tolist()):
            if r not in d:
                d[r] = q
        pos.append(d)
    bndh = np.zeros((C, 3, 2, 2, GP), np.float32)
    for kp in range(3):
        for ki in range(2):
            k = 2 * kp + ki
            for di in range(3):
                for dj in range(3):
                    for p in range(GP):
                        rel = p + (di - 1) * WP + dj - 1
                        ci = 0 if rel in pos[0] else 1
                        bndh[pos[ci][rel], kp, ci, ki, p] += \
                            bases_buf[k, di * 3 + dj]
    bndh = bndh.reshape(C, 6 * C).astype(np.float16)
    # mask2: ones at (p,p) and (GP+p, p)
    mask2 = np.zeros((C, GP), np.float16)
    for p in range(GP):
        mask2[p, p] = 1.0
        mask2[GP + p, p] = 1.0
    coefTh = np.ascontiguousarray(
        coef[:, :, 0, 0].reshape(C, C, NB).transpose(1, 2, 0).reshape(C, NB * C)
    ).astype(np.float16)
    b1h = np.asarray(conv1_b, np.float32).reshape(C, 1)
    b2h = np.asarray(conv2_b, np.float32).reshape(NB, TEM).T.reshape(NBT, 1)
    b3h = np.asarray(bias, np.float32).reshape(C, 1)

    wgtq = np.zeros((n, C, NPAD), np.float16)
    wgtq[:, :CW] = wgtp.reshape(n, CW, NPAD)
    wgtq[:, CW:, : NPAD - WP] = wgtp.reshape(n, CW, NPAD)[:, :, WP:]
    # paired weights: rows 0-63 = tap (0,j), rows 64-127 = tap (1,j)
    wgtq2 = np.zeros((n, C, NPAD), np.float16)
    wgtq2[:, :CW] = wgtp.reshape(n, CW, NPAD)
    wgtq2[:, CW:, : NPAD - 1] = wgtp.reshape(n, CW, NPAD)[:, :, 1:]
    w1wq2 = np.concatenate(
        [
            w1w.reshape(CW, 3, 3, C)[:, 2, 0],
            w1w.reshape(CW, 3, 3, C)[:, 2, 1],
        ],
        axis=0,
    ).reshape(C, C)
    w1wq = np.concatenate(
        [
            w1w.reshape(CW, 3, 3, C)[:, 0],
            w1w.reshape(CW, 3, 3, C)[:, 1],
        ],
        axis=0,
    ).reshape(C, 3 * C)
    pkw = np.concatenate([w1wq, w1wq2], axis=1)
    # transpose column selectors: P1 even-k cols, P2 odd-k cols
    p1 = np.zeros((C, 18), np.float16)
    p2 = np.zeros((C, 18), np.float16)
    for kp in range(3):
        for m in range(NB):
            p1[kp * 12 + m, kp * NB + m] = 1.0
            p2[kp * 12 + NB + m, kp * NB + m] = 1.0
    pk = np.concatenate([w2i, coefTh, bndh, mask2, p1, p2], axis=1)
    pb = np.zeros((C, 3), np.float32)
    pb[:, 0:1] = b1h
    pb[:, 1:2] = b3h
    pb[:NBT, 2:3] = b2h
    shared = {"w1f": w1f, "pkw": pkw, "pk": pk,
              "w1w": np.ascontiguousarray(w1w[:, 8 * C:]), "pb": pb}
    return [
        {
            "featp": featp[i].reshape(C, NPAD).astype(np.float16),
            "wgtq": wgtq[i], "wgtq2": wgtq2[i],
            "fTd": fTdh[i],
            **shared,
        }
        for i in range(n)
    ]


def kernel(feat, weight, conv1_w, conv1_b, conv2_w, conv2_b, bases_buf, coef, bias,
           **run_kwargs):
    in_maps = _prep_maps(
        feat, weight, conv1_w, conv1_b, conv2_w, conv2_b, bases_buf, coef, bias
    )
    res = run_bass_kernel_spmd(
        _get_nc(), in_maps, core_ids=list(range(len(in_maps))), **run_kwargs
    )
    outp = np.stack([r["out"] for r in res.results], 0).astype(np.float32)
    outp = outp[:, :, :NPAD].reshape(-1, C, HP, WP)[:, :, 1 : H + 1, 1 : W + 1]
    _CACHE["last_results"] = res
    return np.ascontiguousarray(outp)


# revision 32
# speedup vs baseline: 1.5516x; 1.0002x over previous
"""Trainium2 Bass kernel for the DCF (dynamic conv filter) module.

Sharding: pure data-parallel over batch N=8 across 8 NeuronCores (one image
per core); all parameters replicated.

Pipeline per core (one 128x96x96 image):
  A:  conv1 (3x3, 192->128) + tanh -> hmid;  conv2 (1x1, 128->36) + tanh -> b
  B:  per 64-pixel group g:
        - SV: double transpose of the b rows + strided half-copies give the
          per-pixel scales in (k-parity x 64px, k-pair*6+m) layout
        - F: fixed-basis convs of feat via k-pair-interleaved banded matmuls
          on host-prepped transposed feature chunks (fTd), 3 di accumulated;
          output partitions are (k-parity*64 + pixel)
        - acc_m = sum_kpair fbS_kpair^T @ blockdiag2(s) -- each matmul
          contracts 2 k's at once (k lives in the contraction dim), so the
          k-reduction costs half the rows of the per-(m,k) diag scheme
        - out_group = sum_m coef_m @ acc_m (+bias)

Block-diagonals are built as tensor_scalar(mask2 * s) (DVE 4x mode); acc and
pso are double-buffered in PSUM so evacuation copies never stall the PE."""

from itertools import product

import numpy as np

import concourse.bass as bass
import concourse.tile as tile
from concourse import bacc, mybir
from concourse.bass_utils import run_bass_kernel_spmd
from concourse.masks import make_identity

fp16 = mybir.dt.float16
fp32 = mybir.dt.float32

N_CORES = 8
C = 128
CW = 64
H = W = 96
HP = WP = 98
NPIX = H * W
NPAD = HP * WP  # 9604
NB = 6
TEM = 6
L = 9
NBT = NB * TEM  # 36
RT = 4
FT = RT * W  # 384
NT = H // RT  # 24
GP = 64           # output pixels per group
NG = 152          # groups (cover padded idx 1 .. 1+152*64 = 9729)
BP = 9732         # padded bsb/out length
FEXT = 10000      # extended (host-side) padded feat length for fTd windows
FOFF = 98         # fTd window base offset inside the extended buffer
OG = 8            # output groups per store
FCH8 = 8 * 2 * C  # fTd cols per streamed chunk (8 groups)
NCH = (NG + 7) // 8  # 19 chunks

# build engine mix for the 18 blockdiag builds per group: 12 DVE (86ns in 4x
# mode), 4 Pool, 2 Act
_B18 = ["D", "P", "D", "P", "D", "P", "D", "D", "D",
        "P", "P", "D", "D", "P", "D", "P", "D", "D"]

_CACHE = {}


def build_nc():
    nc = bacc.Bacc("TRN2", target_bir_lowering=False, debug=False)

    featp = nc.dram_tensor("featp", [C, NPAD], fp16, kind="ExternalInput").ap()
    wgtq = nc.dram_tensor("wgtq", [C, NPAD], fp16, kind="ExternalInput").ap()
    fTd = nc.dram_tensor("fTd", [C, 2 * NG * C], fp16, kind="ExternalInput").ap()
    wgtq2 = nc.dram_tensor("wgtq2", [C, NPAD], fp16, kind="ExternalInput").ap()
    w1f = nc.dram_tensor("w1f", [C, L * C], fp16, kind="ExternalInput").ap()
    # conv1 weight-branch params: w1wq|w1wq2 = 384+128
    pkw = nc.dram_tensor("pkw", [C, 512], fp16, kind="ExternalInput").ap()
    # fp16 params packed: w2|coefT|bnd2|mask2|P1|P2 = 36+768+768+64+18+18
    pk = nc.dram_tensor("pk", [C, 1672], fp16, kind="ExternalInput").ap()
    w1w = nc.dram_tensor("w1w", [CW, C], fp16, kind="ExternalInput").ap()
    pb = nc.dram_tensor("pb", [C, 3], fp32, kind="ExternalInput").ap()
    out = nc.dram_tensor("out", [C, BP], fp16, kind="ExternalOutput").ap()

    Tanh = mybir.ActivationFunctionType.Tanh
    Ident = mybir.ActivationFunctionType.Identity
    Copy = mybir.ActivationFunctionType.Copy
    MUL = mybir.AluOpType.mult
    ADD = mybir.AluOpType.add

    with tile.TileContext(nc) as tc:
        with (
            tc.tile_pool(name="const", bufs=1) as const,
            tc.tile_pool(name="big", bufs=1) as big,
        ):
            featp_sb = big.tile([C, NPAD], fp16)
            wgtp_sb = big.tile([C, NPAD], fp16)
            cuts = [0, 2404, 4808, 7212, NPAD]
            # startup: tiny "head" pieces first so the first conv row-tile's
            # operands land within ~2.5us (DMA pipeline latency floor)
            HD = 686  # cols covering feat/wgt rows 0..6 (row-tile 0 + halo)
            w1f_sb = const.tile([C, L * C], fp16)
            nc.sync.dma_start(w1f_sb[:, : 4 * C], w1f[:, : 4 * C])
            nc.sync.dma_start(featp_sb[:, :HD], featp[:, :HD])
            nc.sync.dma_start(w1f_sb[:, 4 * C :], w1f[:, 4 * C :])
            pkw_sb = const.tile([C, 512], fp16)
            nc.sync.dma_start(pkw_sb[:], pkw)
            w1wq_sb = pkw_sb[:, 0:384]
            w1wq2_sb = pkw_sb[:, 384:512]
            wgtq2_sb = big.tile([C, NPAD], fp16)
            nc.sync.dma_start(wgtp_sb[:, :HD], wgtq[:, :HD])
            nc.sync.dma_start(wgtq2_sb[:, :HD], wgtq2[:, :HD])
            pb_sb = const.tile([C, 3], fp32)
            nc.sync.dma_start(pb_sb[:], pb)
            b1_sb = pb_sb[:, 0:1]
            b3_sb = pb_sb[:, 1:2]
            b2_sb = pb_sb[:NBT, 2:3]
            w1w_sb = const.tile([CW, C], fp16)
            nc.sync.dma_start(w1w_sb[:], w1w)
            nc.sync.dma_start(featp_sb[:, HD : cuts[1]], featp[:, HD : cuts[1]])
            pk_sb = const.tile([C, 1672], fp16)
            nc.sync.dma_start(pk_sb[:, 804:], pk[:, 804:])
            nc.sync.dma_start(pk_sb[:, :804], pk[:, :804])
            ft0_sb = big.tile([C, FCH8], fp16)
            nc.sync.dma_start(ft0_sb[:], fTd[:, :FCH8])
            w2_sb = pk_sb[:, 0:36]
            coefT_sb = pk_sb[:, 36:804]
            bnd2_sb = pk_sb[:, 804:1572]
            mask2_sb = pk_sb[:, 1572:1636]
            p1_sb = pk_sb[:NBT, 1636:1654]
            p2_sb = pk_sb[:NBT, 1654:1672]
            nc.sync.dma_start(wgtq2_sb[:, HD : cuts[1]], wgtq2[:, HD : cuts[1]])
            ft1_sb = big.tile([C, FCH8], fp16)
            nc.sync.dma_start(ft1_sb[:], fTd[:, FCH8 : 2 * FCH8])
            nc.sync.dma_start(wgtp_sb[:, HD : cuts[2]], wgtq[:, HD : cuts[2]])
            nc.sync.dma_start(
                featp_sb[:, cuts[1] : cuts[2]], featp[:, cuts[1] : cuts[2]]
            )
            nc.sync.dma_start(wgtq2_sb[:, cuts[1] :], wgtq2[:, cuts[1] :])
            nc.sync.dma_start(
                featp_sb[:, cuts[2] : cuts[3]], featp[:, cuts[2] : cuts[3]]
            )
            nc.sync.dma_start(wgtp_sb[:, cuts[2] :], wgtq[:, cuts[2] :])
            nc.sync.dma_start(
                featp_sb[:, cuts[3] :], featp[:, cuts[3] :]
            )
            bsb = big.tile([NBT, BP], fp16)
            # zero only the border/tail cells conv2 never writes (full memset
            # would hold Pool for 8us before the first b write)
            nc.gpsimd.memset(bsb[:, : WP + 2], 0.0)
            edge = bsb[:, 97 : 97 + 97 * WP].rearrange("c (r w) -> c r w", w=WP)
            nc.gpsimd.memset(edge[:, :, 0:2], 0.0)
            nc.gpsimd.memset(bsb[:, 97 * WP :], 0.0)

            b3d = bsb[:, :NPAD].rearrange("c (r w) -> c r w", w=WP)
            f3 = featp_sb[:].rearrange("c (r w) -> c r w", w=WP)
            w3 = wgtp_sb[:].rearrange("c (r w) -> c r w", w=WP)
            wq2 = wgtq2_sb[:].rearrange("c (r w) -> c r w", w=WP)

            # ---- fused pipeline: conv rows (phase A) stream in between the
            # software-pipelined per-group stages of phase B.
            with (
                tc.tile_pool(name="hmp", bufs=3) as hmp,
                tc.tile_pool(name="ftp", bufs=3) as ftp,
                tc.tile_pool(name="svsp", bufs=4) as svsp,
                tc.tile_pool(name="fbp", bufs=3) as fbp,
                tc.tile_pool(name="dgp", bufs=3) as dgp,
                tc.tile_pool(name="bop", bufs=3) as bop,
                tc.tile_pool(name="orp", bufs=2) as orp,
                tc.tile_pool(name="psB2", bufs=1, space="PSUM") as psB2,
            ):
                fbS_r, dg_r, boS_r, pso_r, acc_r = {}, {}, {}, {}, {}
                # two pso slots manually ring-buffered inside ONE psum bank
                psoD = psB2.tile([C, 2 * GP], fp32, tag="pso", bufs=1,
                                 name="psoD")
                hm_r = {}
                svS_r, ft_r = {}, {}
                orow_bufs = {}

                def emit_arow_f(t):
                    r0 = t * RT
                    ps = psB2.tile([C, FT], fp32, tag="psA", bufs=1, name="ps")
                    for kk, (i, j) in enumerate(product(range(3), range(3))):
                        nc.tensor.matmul(
                            ps[:],
                            w1f_sb[:, (i * 3 + j) * C : (i * 3 + j + 1) * C],
                            f3[:, r0 + i : r0 + i + RT, j : j + W],
                            start=(kk == 0),
                            stop=False,
                        )
                    return ps

                def emit_arow_w(t, ps):
                    r0 = t * RT
                    for j in range(3):
                        nc.tensor.matmul(
                            ps[:],
                            w1wq_sb[:, j * C : (j + 1) * C],
                            w3[:, r0 : r0 + RT, j : j + W],
                            start=False,
                            stop=False,
                        )
                    nc.tensor.matmul(
                        ps[:],
                        w1wq2_sb,
                        wq2[:, r0 + 2 : r0 + 2 + RT, 0 : W],
                        start=False,
                        stop=False,
                    )
                    nc.tensor.matmul(
                        ps[:],
                        w1w_sb[:],
                        w3[:64, r0 + 2 : r0 + 2 + RT, 2 : 2 + W],
                        start=False,
                        stop=True,
                    )
                    hm = hmp.tile([C, FT], fp16, tag="hm")
                    nc.scalar.activation(hm[:], ps[:], Tanh, bias=b1_sb)
                    hm_r[t] = hm

                def emit_arow_b(t):
                    r0 = t * RT
                    hm = hm_r.pop(t)
                    ps2 = psB2.tile([NBT, FT], fp32, tag="psB", bufs=1, name="ps2")
                    nc.tensor.matmul(ps2[:], w2_sb, hm[:], start=True, stop=True)
                    nc.scalar.activation(
                        b3d[:, r0 + 1 : r0 + 1 + RT, 1 : 1 + W],
                        ps2[:].rearrange("c (r w) -> c r w", w=W),
                        Tanh,
                        bias=b2_sb,
                    )

                def emit_sv(g):
                    # b rows (k-major: row = k*6+m) -> per-pixel scales in
                    # (k-parity*64+p, kpair*6+m) layout: the two transposes
                    # use column-selector matrices (even/odd k) so svp lands
                    # pre-interleaved; one contiguous copy evacuates it
                    svp = psB2.tile([C, 18], fp32, tag="svp", bufs=1,
                                    name="svp")
                    src = bsb[:, 1 + g * GP : 1 + (g + 1) * GP]
                    nc.tensor.matmul(svp[:GP, :], src, p1_sb,
                                     start=True, stop=True)
                    nc.tensor.matmul(svp[GP:, :], src, p2_sb,
                                     start=True, stop=True)
                    svS = svsp.tile([C, 18], fp32, tag="svS")
                    svS_r[g] = svS
                    nc.vector.tensor_copy(svS[:], svp[:])

                def emit_builds(g, dg, js):
                    svS = svS_r[g]
                    for j in js:
                        m, kp = divmod(j, 3)
                        sc = svS[:, kp * NB + m : kp * NB + m + 1]
                        dslice = dg[:, j * GP : (j + 1) * GP]
                        eng = _B18[j]
                        if eng == "D":
                            nc.vector.tensor_scalar(
                                dslice, mask2_sb, sc, None, MUL
                            )
                        elif eng == "P":
                            nc.gpsimd.tensor_scalar(
                                dslice, mask2_sb, sc, None, MUL
                            )
                        else:
                            nc.scalar.activation(dslice, mask2_sb, Copy,
                                                 scale=sc)

                # conv row-tile slots spread at cadence ~6.5 so conv filler
                # lasts until iteration ~138 (row 23 deadline is ~141)
                _fs = {int(_r * 6.5 + 0.5): _r + 2 for _r in range(NT - 2)}
                _ws = {k + 1: v for k, v in _fs.items()}
                _bs = {k + 2: v for k, v in _fs.items()}

                for r in range(2):
                    emit_arow_w(r, emit_arow_f(r))
                    emit_arow_b(r)
                # fTd chunks 0,1 were DMA'd in the startup sequence
                ft_r[0] = ft0_sb
                ft_r[1] = ft1_sb
                emit_sv(0)
                emit_sv(1)

                for i in range(NG + 4):
                    if i in _fs:
                        arow_ps = emit_arow_f(_fs[i])
                    elif i in _ws:
                        emit_arow_w(_ws[i], arow_ps)
                    elif i in _bs:
                        emit_arow_b(_bs[i])
                    # acc(i-2) psum -> SBUF (Pool)
                    if 0 <= i - 2 < NG:
                        boS = bop.tile([C, NB * GP], fp16, tag="boS")
                        boS_r[i - 2] = boS
                        acc = acc_r.pop(i - 2)
                        nc.vector.tensor_copy(boS[:, : 3 * GP], acc[:, : 3 * GP])
                        nc.scalar.copy(boS[:, 3 * GP :], acc[:, 3 * GP :])
                    # orow(i-3) + store
                    if 0 <= i - 4 < NG:
                        j = i - 4
                        g8 = j % OG
                        if g8 == 0:
                            orow_bufs[j] = orp.tile(
                                [C, OG * GP], fp16, tag="orow", name="orow_buf"
                            )
                        ob = orow_bufs[j - g8]
                        nc.scalar.activation(
                            ob[:, g8 * GP : (g8 + 1) * GP], pso_r.pop(j),
                            Ident, bias=b3_sb,
                        )
                        if j >= NG - 4:
                            # tail: store each group as soon as it lands so
                            # the final DMA only waits on the last orow
                            nc.sync.dma_start(
                                out[:, 1 + j * GP : 1 + (j + 1) * GP],
                                ob[:, g8 * GP : (g8 + 1) * GP],
                            )
                            if j == NG - 1:
                                del orow_bufs[j - g8]
                        elif g8 == OG - 1:
                            t0 = j - g8
                            nc.sync.dma_start(
                                out[:, 1 + t0 * GP : 1 + t0 * GP + OG * GP],
                                ob[:, : OG * GP],
                            )
                            del orow_bufs[t0]
                        elif j == NG - 5 and g8 == 3:
                            t0 = j - g8
                            nc.sync.dma_start(
                                out[:, 1 + t0 * GP : 1 + t0 * GP + 4 * GP],
                                ob[:, : 4 * GP],
                            )
                    # stream next fTd chunk
                    if i % 8 == 0 and i // 8 + 2 < NCH:
                        cch = i // 8 + 2
                        ft = ftp.tile([C, FCH8], fp16, tag="ft")
                        ft_r[cch] = ft
                        nc.sync.dma_start(
                            ft[:], fTd[:, cch * FCH8 : (cch + 1) * FCH8]
                        )
                    # per-pixel scales two groups ahead
                    if i + 2 < NG:
                        emit_sv(i + 2)
                    # F(i), blockdiag builds(i), F evac(i)
                    if i < NG:
                        psf = psB2.tile([C, 3 * C], fp32, tag="psf", bufs=2,
                                        name="psf")
                        ft = ft_r[i // 8]
                        for kp in range(3):
                            for ci in range(2):
                                nc.tensor.matmul(
                                    psf[:, kp * C : (kp + 1) * C],
                                    bnd2_sb[:, (kp * 2 + ci) * C
                                            : (kp * 2 + ci + 1) * C],
                                    ft[:, ((i % 8) * 2 + ci) * C
                                       : ((i % 8) * 2 + ci + 1) * C],
                                    start=(ci == 0),
                                    stop=(ci == 1),
                                )
                        dg = dgp.tile([C, 18 * GP], fp16, tag="dg")
                        dg_r[i] = dg
                        emit_builds(i, dg, range(9))
                        fbS = fbp.tile([C, 3 * C], fp16, tag="fbS")
                        nc.scalar.copy(fbS[:], psf[:])
                        fbS_r[i] = fbS
                        emit_builds(i, dg, range(9, 18))
                        if i % 8 == 7 or i == NG - 1:
                            ft_r.pop(i // 8)
                    # blockdiag matmuls (i-1): acc_m = sum_kp fbS_kp^T @ bd2
                    if 0 <= i - 1 < NG:
                        acc = psB2.tile([C, NB * GP], fp32, tag="accT", bufs=2,
                                        name="acc")
                        acc_r[i - 1] = acc
                        fbS = fbS_r.pop(i - 1)
                        dg = dg_r.pop(i - 1)
                        for m in range(NB):
                            for kp in range(3):
                                j = m * 3 + kp
                                nc.tensor.matmul(
                                    acc[:, m * GP : (m + 1) * GP],
                                    fbS[:, kp * C : (kp + 1) * C],
                                    dg[:, j * GP : (j + 1) * GP],
                                    start=(kp == 0),
                                    stop=(kp == 2),
                                )
                    # coef matmuls (i-2)
                    if 0 <= i - 3 < NG:
                        _s = ((i - 3) % 2) * GP
                        pso = psoD[:, _s : _s + GP]
                        pso_r[i - 3] = pso
                        boS = boS_r.pop(i - 3)
                        for m in range(NB):
                            nc.tensor.matmul(
                                pso[:],
                                coefT_sb[:, m * C : (m + 1) * C],
                                boS[:, m * GP : (m + 1) * GP],
                                start=(m == 0),
                                stop=(m == NB - 1),
                            )

    nc.compile()
    return nc


def _get_nc():
    if "nc" not in _CACHE:
        _CACHE["nc"] = build_nc()
    return _CACHE["nc"]


def _prep_maps(feat, weight, conv1_w, conv1_b, conv2_w, conv2_b, bases_buf, coef, bias):
    feat = np.asarray(feat, np.float32)
    weight = np.asarray(weight, np.float32)
    conv1_w = np.asarray(conv1_w, np.float32)
    conv2_w = np.asarray(conv2_w, np.float32)
    bases_buf = np.asarray(bases_buf, np.float32)
    coef = np.asarray(coef, np.float32)

    n = feat.shape[0]
    featp = np.zeros((n, C, HP, WP), np.float16)
    featp[:, :, 1 : H + 1, 1 : W + 1] = feat
    wgtp = np.zeros((n, CW, HP, WP), np.float16)
    wgtp[:, :, 1 : H + 1, 1 : W + 1] = weight

    # host-prepped transposed feature chunks: the 3 disjoint di input
    # windows (66 px each) packed into 2 chunks of 128 rows per group
    # fTd[q, (g*2+ci)*C + c] = fe[c, FOFF + 1 + g*GP + RELS[ci][q]]
    rels1 = list(range(-99, -33)) + list(range(-1, 61))
    rels2 = list(range(61, 65)) + list(range(97, 163))
    rels2 = rels2 + [163] * (C - len(rels2))
    RELS = [np.array(rels1), np.array(rels2)]
    fe = np.zeros((n, C, FEXT), np.float16)
    fe[:, :, FOFF : FOFF + NPAD] = featp.reshape(n, C, NPAD)
    fTdh = np.empty((n, 2 * NG, C, C), np.float16)
    for g in range(NG):
        for ci in range(2):
            idx = FOFF + 1 + g * GP + RELS[ci]
            fTdh[:, g * 2 + ci] = fe[:, :, idx].transpose(0, 2, 1)
    fTdh = np.ascontiguousarray(
        fTdh.transpose(0, 2, 1, 3).reshape(n, C, 2 * NG * C)
    )

    w1f = np.ascontiguousarray(
        conv1_w[:, :C].transpose(1, 2, 3, 0).reshape(C, L * C)
    ).astype(np.float16)
    w1w = np.ascontiguousarray(
        conv1_w[:, C:].transpose(1, 2, 3, 0).reshape(CW, L * C)
    ).astype(np.float16)
    # conv2 weights with k-major output-channel ordering (col = k*NB + m)
    w2h = np.ascontiguousarray(conv2_w[:, :, 0, 0].T).astype(np.float16)
    w2i = np.ascontiguousarray(
        w2h.reshape(C, NB, TEM).transpose(0, 2, 1).reshape(C, NBT)
    )
    # packed band blocks: bnd2[q, (kp*2+ci)*C + ki*GP + p] accumulates
    # bases_buf[2*kp+ki, di*3+dj] where chunk ci row q holds input offset
    # rel = p + (di-1)*WP + dj - 1
    pos = []
    for rels in RELS:
        d = {}
        for q, r in enumerate(rels.tolist()):
            if r not in d:
                d[r] = q
        pos.append(d)
    bndh = np.zeros((C, 3, 2, 2, GP), np.float32)
    for kp in range(3):
        for ki in range(2):
            k = 2 * kp + ki
            for di in range(3):
                for dj in range(3):
                    for p in range(GP):
                        rel = p + (di - 1) * WP + dj - 1
                        ci = 0 if rel in pos[0] else 1
                        bndh[pos[ci][rel], kp, ci, ki, p] += \
                            bases_buf[k, di * 3 + dj]
    bndh = bndh.reshape(C, 6 * C).astype(np.float16)
    # mask2: ones at (p,p) and (GP+p, p)
    mask2 = np.zeros((C, GP), np.float16)
    for p in range(GP):
        mask2[p, p] = 1.0
        mask2[GP + p, p] = 1.0
    coefTh = np.ascontiguousarray(
        coef[:, :, 0, 0].reshape(C, C, NB).transpose(1, 2, 0).reshape(C, NB * C)
    ).astype(np.float16)
    b1h = np.asarray(conv1_b, np.float32).reshape(C, 1)
    b2h = np.asarray(conv2_b, np.float32).reshape(NB, TEM).T.reshape(NBT, 1)
    b3h = np.asarray(bias, np.float32).reshape(C, 1)

    wgtq = np.zeros((n, C, NPAD), np.float16)
    wgtq[:, :CW] = wgtp.reshape(n, CW, NPAD)
    wgtq[:, CW:, : NPAD - WP] = wgtp.reshape(n, CW, NPAD)[:, :, WP:]
    # paired weights: rows 0-63 = tap (0,j), rows 64-127 = tap (1,j)
    wgtq2 = np.zeros((n, C, NPAD), np.float16)
    wgtq2[:, :CW] = wgtp.reshape(n, CW, NPAD)
    wgtq2[:, CW:, : NPAD - 1] = wgtp.reshape(n, CW, NPAD)[:, :, 1:]
    w1wq2 = np.concatenate(
        [
            w1w.reshape(CW, 3, 3, C)[:, 2, 0],
            w1w.reshape(CW, 3, 3, C)[:, 2, 1],
        ],
        axis=0,
    ).reshape(C, C)
    w1wq = np.concatenate(
        [
            w1w.reshape(CW, 3, 3, C)[:, 0],
            w1w.reshape(CW, 3, 3, C)[:, 1],
        ],
        axis=0,
    ).reshape(C, 3 * C)
    pkw = np.concatenate([w1wq, w1wq2], axis=1)
    # transpose column selectors: P1 even-k cols, P2 odd-k cols
    p1 = np.zeros((C, 18), np.float16)
    p2 = np.zeros((C, 18), np.float16)
    for kp in range(3):
        for m in range(NB):
            p1[kp * 12 + m, kp * NB + m] = 1.0
            p2[kp * 12 + NB + m, kp * NB + m] = 1.0
    pk = np.concatenate([w2i, coefTh, bndh, mask2, p1, p2], axis=1)
    pb = np.zeros((C, 3), np.float32)
    pb[:, 0:1] = b1h
    pb[:, 1:2] = b3h
    pb[:NBT, 2:3] = b2h
    shared = {"w1f": w1f, "pkw": pkw, "pk": pk,
              "w1w": np.ascontiguousarray(w1w[:, 8 * C:]), "pb": pb}
    return [
        {
            "featp": featp[i].reshape(C, NPAD).astype(np.float16),
            "wgtq": wgtq[i], "wgtq2": wgtq2[i],
            "fTd": fTdh[i],
            **shared,
        }
        for i in range(n)
    ]


def kernel(feat, weight, conv1_w, conv1_b, conv2_w, conv2_b, bases_buf, coef, bias,
           **run_kwargs):
    in_maps = _prep_maps(
        feat, weight, conv1_w, conv1_b, conv2_w, conv2_b, bases_buf, coef, bias
    )
    res = run_bass_kernel_spmd(
        _get_nc(), in_maps, core_ids=list(range(len(in_maps))), **run_kwargs
    )
    outp = np.stack([r["out"] for r in res.results], 0).astype(np.float32)
    outp = outp[:, :, :NPAD].reshape(-1, C, HP, WP)[:, :, 1 : H + 1, 1 : W + 1]
    _CACHE["last_results"] = res
    return np.ascontiguousarray(outp)


# revision 34
# speedup vs baseline: 1.5550x; 1.0022x over previous
"""Trainium2 Bass kernel for the DCF (dynamic conv filter) module.

Sharding: pure data-parallel over batch N=8 across 8 NeuronCores (one image
per core); all parameters replicated.

Pipeline per core (one 128x96x96 image):
  A:  conv1 (3x3, 192->128) + tanh -> hmid;  conv2 (1x1, 128->36) + tanh -> b
  B:  per 64-pixel group g:
        - SV: double transpose of the b rows + strided half-copies give the
          per-pixel scales in (k-parity x 64px, k-pair*6+m) layout
        - F: fixed-basis convs of feat via k-pair-interleaved banded matmuls
          on host-prepped transposed feature chunks (fTd), 3 di accumulated;
          output partitions are (k-parity*64 + pixel)
        - acc_m = sum_kpair fbS_kpair^T @ blockdiag2(s) -- each matmul
          contracts 2 k's at once (k lives in the contraction dim), so the
          k-reduction costs half the rows of the per-(m,k) diag scheme
        - out_group = sum_m coef_m @ acc_m (+bias)

Block-diagonals are built as tensor_scalar(mask2 * s) (DVE 4x mode); acc and
pso are double-buffered in PSUM so evacuation copies never stall the PE."""

from itertools import product

import numpy as np

import concourse.bass as bass
import concourse.tile as tile
from concourse import bacc, mybir
from concourse.bass_utils import run_bass_kernel_spmd
from concourse.masks import make_identity

fp16 = mybir.dt.float16
fp32 = mybir.dt.float32

N_CORES = 8
C = 128
CW = 64
H = W = 96
HP = WP = 98
NPIX = H * W
NPAD = HP * WP  # 9604
NB = 6
TEM = 6
L = 9
NBT = NB * TEM  # 36
RT = 4
FT = RT * W  # 384
NT = H // RT  # 24
GP = 64           # output pixels per group
NG = 152          # groups (cover padded idx 1 .. 1+152*64 = 9729)
BP = 9732         # padded bsb/out length
FEXT = 10000      # extended (host-side) padded feat length for fTd windows
FOFF = 98         # fTd window base offset inside the extended buffer
OG = 8            # output groups per store
FCH8 = 8 * 2 * C  # fTd cols per streamed chunk (8 groups)
NCH = (NG + 7) // 8  # 19 chunks

# build engine mix for the 18 blockdiag builds per group: 12 DVE (86ns in 4x
# mode), 4 Pool, 2 Act
_B18 = ["D", "P", "D", "P", "D", "P", "D", "D", "D",
        "P", "P", "D", "D", "P", "D", "P", "D", "D"]

_CACHE = {}


def build_nc():
    nc = bacc.Bacc("TRN2", target_bir_lowering=False, debug=False)

    featp = nc.dram_tensor("featp", [C, NPAD], fp16, kind="ExternalInput").ap()
    wgtq = nc.dram_tensor("wgtq", [C, NPAD], fp16, kind="ExternalInput").ap()
    fTd = nc.dram_tensor("fTd", [C, 2 * NG * C], fp16, kind="ExternalInput").ap()
    wgtq2 = nc.dram_tensor("wgtq2", [C, NPAD], fp16, kind="ExternalInput").ap()
    w1f = nc.dram_tensor("w1f", [C, L * C], fp16, kind="ExternalInput").ap()
    # conv1 weight-branch params + SV selectors: w1wq|w1wq2|P1|P2
    pkw = nc.dram_tensor("pkw", [C, 548], fp16, kind="ExternalInput").ap()
    # fp16 params packed: w2|coefT|bnd2|mask2 = 36+768+768+64
    pk = nc.dram_tensor("pk", [C, 1636], fp16, kind="ExternalInput").ap()
    w1w = nc.dram_tensor("w1w", [CW, C], fp16, kind="ExternalInput").ap()
    pb = nc.dram_tensor("pb", [C, 3], fp32, kind="ExternalInput").ap()
    out = nc.dram_tensor("out", [C, BP], fp16, kind="ExternalOutput").ap()

    Tanh = mybir.ActivationFunctionType.Tanh
    Ident = mybir.ActivationFunctionType.Identity
    Copy = mybir.ActivationFunctionType.Copy
    MUL = mybir.AluOpType.mult
    ADD = mybir.AluOpType.add

    with tile.TileContext(nc) as tc:
        with (
            tc.tile_pool(name="const", bufs=1) as const,
            tc.tile_pool(name="big", bufs=1) as big,
        ):
            featp_sb = big.tile([C, NPAD], fp16)
            wgtp_sb = big.tile([C, NPAD], fp16)
            cuts = [0, 2404, 4808, 7212, NPAD]
            # startup: tiny "head" pieces first so the first conv row-tile's
            # operands land within ~2.5us (DMA pipeline latency floor)
            HD = 686  # cols covering feat/wgt rows 0..6 (row-tile 0 + halo)
            w1f_sb = const.tile([C, L * C], fp16)
            nc.sync.dma_start(w1f_sb[:, : 4 * C], w1f[:, : 4 * C])
            nc.sync.dma_start(featp_sb[:, :HD], featp[:, :HD])
            nc.sync.dma_start(w1f_sb[:, 4 * C :], w1f[:, 4 * C :])
            pkw_sb = const.tile([C, 548], fp16)
            nc.sync.dma_start(pkw_sb[:], pkw)
            w1wq_sb = pkw_sb[:, 0:384]
            w1wq2_sb = pkw_sb[:, 384:512]
            p1_sb = pkw_sb[:NBT, 512:530]
            p2_sb = pkw_sb[:NBT, 530:548]
            wgtq2_sb = big.tile([C, NPAD], fp16)
            nc.sync.dma_start(wgtp_sb[:, :HD], wgtq[:, :HD])
            nc.sync.dma_start(wgtq2_sb[:, :HD], wgtq2[:, :HD])
            pb_sb = const.tile([C, 3], fp32)
            nc.sync.dma_start(pb_sb[:], pb)
            b1_sb = pb_sb[:, 0:1]
            b3_sb = pb_sb[:, 1:2]
            b2_sb = pb_sb[:NBT, 2:3]
            w1w_sb = const.tile([CW, C], fp16)
            nc.sync.dma_start(w1w_sb[:], w1w)
            nc.sync.dma_start(featp_sb[:, HD : cuts[1]], featp[:, HD : cuts[1]])
            pk_sb = const.tile([C, 1636], fp16)
            nc.sync.dma_start(pk_sb[:, 804:], pk[:, 804:])
            nc.sync.dma_start(pk_sb[:, :804], pk[:, :804])
            ft0_sb = big.tile([C, FCH8], fp16)
            nc.sync.dma_start(ft0_sb[:], fTd[:, :FCH8])
            w2_sb = pk_sb[:, 0:36]
            coefT_sb = pk_sb[:, 36:804]
            bnd2_sb = pk_sb[:, 804:1572]
            mask2_sb = pk_sb[:, 1572:1636]
            nc.sync.dma_start(wgtq2_sb[:, HD : cuts[1]], wgtq2[:, HD : cuts[1]])
            ft1_sb = big.tile([C, FCH8], fp16)
            nc.sync.dma_start(ft1_sb[:], fTd[:, FCH8 : 2 * FCH8])
            nc.sync.dma_start(wgtp_sb[:, HD : cuts[2]], wgtq[:, HD : cuts[2]])
            nc.sync.dma_start(
                featp_sb[:, cuts[1] : cuts[2]], featp[:, cuts[1] : cuts[2]]
            )
            nc.sync.dma_start(wgtq2_sb[:, cuts[1] :], wgtq2[:, cuts[1] :])
            nc.sync.dma_start(
                featp_sb[:, cuts[2] : cuts[3]], featp[:, cuts[2] : cuts[3]]
            )
            nc.sync.dma_start(wgtp_sb[:, cuts[2] :], wgtq[:, cuts[2] :])
            nc.sync.dma_start(
                featp_sb[:, cuts[3] :], featp[:, cuts[3] :]
            )
            bsb = big.tile([NBT, BP], fp16)
            # zero only the border/tail cells conv2 never writes (full memset
            # would hold Pool for 8us before the first b write)
            nc.gpsimd.memset(bsb[:, : WP + 2], 0.0)
            edge = bsb[:, 97 : 97 + 97 * WP].rearrange("c (r w) -> c r w", w=WP)
            nc.gpsimd.memset(edge[:, :, 0:2], 0.0)
            nc.gpsimd.memset(bsb[:, 97 * WP :], 0.0)

            b3d = bsb[:, :NPAD].rearrange("c (r w) -> c r w", w=WP)
            f3 = featp_sb[:].rearrange("c (r w) -> c r w", w=WP)
            w3 = wgtp_sb[:].rearrange("c (r w) -> c r w", w=WP)
            wq2 = wgtq2_sb[:].rearrange("c (r w) -> c r w", w=WP)

            # ---- fused pipeline: conv rows (phase A) stream in between the
            # software-pipelined per-group stages of phase B.
            with (
                tc.tile_pool(name="hmp", bufs=3) as hmp,
                tc.tile_pool(name="ftp", bufs=3) as ftp,
                tc.tile_pool(name="svsp", bufs=4) as svsp,
                tc.tile_pool(name="fbp", bufs=3) as fbp,
                tc.tile_pool(name="dgp", bufs=3) as dgp,
                tc.tile_pool(name="bop", bufs=3) as bop,
                tc.tile_pool(name="orp", bufs=2) as orp,
                tc.tile_pool(name="psB2", bufs=1, space="PSUM") as psB2,
            ):
                fbS_r, dg_r, boS_r, pso_r, acc_r = {}, {}, {}, {}, {}
                # two pso slots manually ring-buffered inside ONE psum bank
                psoD = psB2.tile([C, 2 * GP], fp32, tag="pso", bufs=1,
                                 name="psoD")
                hm_r = {}
                svS_r, ft_r = {}, {}
                orow_bufs = {}

                def emit_arow_f(t):
                    r0 = t * RT
                    ps = psB2.tile([C, FT], fp32, tag="psA", bufs=1, name="ps")
                    for kk, (i, j) in enumerate(product(range(3), range(3))):
                        nc.tensor.matmul(
                            ps[:],
                            w1f_sb[:, (i * 3 + j) * C : (i * 3 + j + 1) * C],
                            f3[:, r0 + i : r0 + i + RT, j : j + W],
                            start=(kk == 0),
                            stop=False,
                        )
                    return ps

                def emit_arow_w(t, ps):
                    r0 = t * RT
                    for j in range(3):
                        nc.tensor.matmul(
                            ps[:],
                            w1wq_sb[:, j * C : (j + 1) * C],
                            w3[:, r0 : r0 + RT, j : j + W],
                            start=False,
                            stop=False,
                        )
                    nc.tensor.matmul(
                        ps[:],
                        w1wq2_sb,
                        wq2[:, r0 + 2 : r0 + 2 + RT, 0 : W],
                        start=False,
                        stop=False,
                    )
                    nc.tensor.matmul(
                        ps[:],
                        w1w_sb[:],
                        w3[:64, r0 + 2 : r0 + 2 + RT, 2 : 2 + W],
                        start=False,
                        stop=True,
                    )
                    hm = hmp.tile([C, FT], fp16, tag="hm")
                    nc.scalar.activation(hm[:], ps[:], Tanh, bias=b1_sb)
                    hm_r[t] = hm

                def emit_arow_b(t):
                    r0 = t * RT
                    hm = hm_r.pop(t)
                    ps2 = psB2.tile([NBT, FT], fp32, tag="psB", bufs=1, name="ps2")
                    nc.tensor.matmul(ps2[:], w2_sb, hm[:], start=True, stop=True)
                    nc.scalar.activation(
                        b3d[:, r0 + 1 : r0 + 1 + RT, 1 : 1 + W],
                        ps2[:].rearrange("c (r w) -> c r w", w=W),
                        Tanh,
                        bias=b2_sb,
                    )

                def emit_sv(g):
                    # b rows (k-major: row = k*6+m) -> per-pixel scales in
                    # (k-parity*64+p, kpair*6+m) layout: the two transposes
                    # use column-selector matrices (even/odd k) so svp lands
                    # pre-interleaved; one contiguous copy evacuates it
                    svp = psB2.tile([C, 18], fp32, tag="svp", bufs=1,
                                    name="svp")
                    src = bsb[:, 1 + g * GP : 1 + (g + 1) * GP]
                    nc.tensor.matmul(svp[:GP, :], src, p1_sb,
                                     start=True, stop=True)
                    nc.tensor.matmul(svp[GP:, :], src, p2_sb,
                                     start=True, stop=True)
                    svS = svsp.tile([C, 18], fp32, tag="svS")
                    svS_r[g] = svS
                    nc.vector.tensor_copy(svS[:], svp[:])

                def emit_builds(g, dg, js):
                    svS = svS_r[g]
                    for j in js:
                        m, kp = divmod(j, 3)
                        sc = svS[:, kp * NB + m : kp * NB + m + 1]
                        dslice = dg[:, j * GP : (j + 1) * GP]
                        eng = _B18[j]
                        if eng == "D":
                            nc.vector.tensor_scalar(
                                dslice, mask2_sb, sc, None, MUL
                            )
                        elif eng == "P":
                            nc.gpsimd.tensor_scalar(
                                dslice, mask2_sb, sc, None, MUL
                            )
                        else:
                            nc.scalar.activation(dslice, mask2_sb, Copy,
                                                 scale=sc)

                # conv row-tile slots spread at cadence ~6.5 so conv filler
                # lasts until iteration ~138 (row 23 deadline is ~141)
                _fs = {int(_r * 6.5 + 0.5): _r + 2 for _r in range(NT - 2)}
                _ws = {k + 1: v for k, v in _fs.items()}
                _bs = {k + 2: v for k, v in _fs.items()}

                for r in range(2):
                    emit_arow_w(r, emit_arow_f(r))
                    emit_arow_b(r)
                # fTd chunks 0,1 were DMA'd in the startup sequence
                ft_r[0] = ft0_sb
                ft_r[1] = ft1_sb
                emit_sv(0)
                emit_sv(1)

                for i in range(NG + 4):
                    if i in _fs:
                        arow_ps = emit_arow_f(_fs[i])
                    elif i in _ws:
                        emit_arow_w(_ws[i], arow_ps)
                    elif i in _bs:
                        emit_arow_b(_bs[i])
                    # acc(i-2) psum -> SBUF (Pool)
                    if 0 <= i - 2 < NG:
                        boS = bop.tile([C, NB * GP], fp16, tag="boS")
                        boS_r[i - 2] = boS
                        acc = acc_r.pop(i - 2)
                        nc.vector.tensor_copy(boS[:, : 3 * GP], acc[:, : 3 * GP])
                        nc.scalar.copy(boS[:, 3 * GP :], acc[:, 3 * GP :])
                    # orow(i-3) + store
                    if 0 <= i - 4 < NG:
                        j = i - 4
                        g8 = j % OG
                        if g8 == 0:
                            orow_bufs[j] = orp.tile(
                                [C, OG * GP], fp16, tag="orow", name="orow_buf"
                            )
                        ob = orow_bufs[j - g8]
                        nc.scalar.activation(
                            ob[:, g8 * GP : (g8 + 1) * GP], pso_r.pop(j),
                            Ident, bias=b3_sb,
                        )
                        if j >= NG - 4:
                            # tail: store each group as soon as it lands so
                            # the final DMA only waits on the last orow
                            nc.sync.dma_start(
                                out[:, 1 + j * GP : 1 + (j + 1) * GP],
                                ob[:, g8 * GP : (g8 + 1) * GP],
                            )
                            if j == NG - 1:
                                del orow_bufs[j - g8]
                        elif g8 == OG - 1:
                            t0 = j - g8
                            nc.sync.dma_start(
                                out[:, 1 + t0 * GP : 1 + t0 * GP + OG * GP],
                                ob[:, : OG * GP],
                            )
                            del orow_bufs[t0]
                        elif j == NG - 5 and g8 == 3:
                            t0 = j - g8
                            nc.sync.dma_start(
                                out[:, 1 + t0 * GP : 1 + t0 * GP + 4 * GP],
                                ob[:, : 4 * GP],
                            )
                    # stream next fTd chunk
                    if i % 8 == 0 and i // 8 + 2 < NCH:
                        cch = i // 8 + 2
                        ft = ftp.tile([C, FCH8], fp16, tag="ft")
                        ft_r[cch] = ft
                        nc.sync.dma_start(
                            ft[:], fTd[:, cch * FCH8 : (cch + 1) * FCH8]
                        )
                    # per-pixel scales two groups ahead
                    if i + 2 < NG:
                        emit_sv(i + 2)
                    # F(i), blockdiag builds(i), F evac(i)
                    if i < NG:
                        psf = psB2.tile([C, 3 * C], fp32, tag="psf", bufs=2,
                                        name="psf")
                        ft = ft_r[i // 8]
                        for kp in range(3):
                            for ci in range(2):
                                nc.tensor.matmul(
                                    psf[:, kp * C : (kp + 1) * C],
                                    bnd2_sb[:, (kp * 2 + ci) * C
                                            : (kp * 2 + ci + 1) * C],
                                    ft[:, ((i % 8) * 2 + ci) * C
                                       : ((i % 8) * 2 + ci + 1) * C],
                                    start=(ci == 0),
                                    stop=(ci == 1),
                                )
                        dg = dgp.tile([C, 18 * GP], fp16, tag="dg")
                        dg_r[i] = dg
                        emit_builds(i, dg, range(9))
                        fbS = fbp.tile([C, 3 * C], fp16, tag="fbS")
                        nc.scalar.copy(fbS[:], psf[:])
                        fbS_r[i] = fbS
                        emit_builds(i, dg, range(9, 18))
                        if i % 8 == 7 or i == NG - 1:
                            ft_r.pop(i // 8)
                    # blockdiag matmuls (i-1): acc_m = sum_kp fbS_kp^T @ bd2
                    if 0 <= i - 1 < NG:
                        acc = psB2.tile([C, NB * GP], fp32, tag="accT", bufs=2,
                                        name="acc")
                        acc_r[i - 1] = acc
                        fbS = fbS_r.pop(i - 1)
                        dg = dg_r.pop(i - 1)
                        for m in range(NB):
                            for kp in range(3):
                                j = m * 3 + kp
                                nc.tensor.matmul(
                                    acc[:, m * GP : (m + 1) * GP],
                                    fbS[:, kp * C : (kp + 1) * C],
                                    dg[:, j * GP : (j + 1) * GP],
                                    start=(kp == 0),
                                    stop=(kp == 2),
                                )
                    # coef matmuls (i-2)
                    if 0 <= i - 3 < NG:
                        _s = ((i - 3) % 2) * GP
                        pso = psoD[:, _s : _s + GP]
                        pso_r[i - 3] = pso
                        boS = boS_r.pop(i - 3)
                        for m in range(NB):
                            nc.tensor.matmul(
                                pso[:],
                                coefT_sb[:, m * C : (m + 1) * C],
                                boS[:, m * GP : (m + 1) * GP],
                                start=(m == 0),
                                stop=(m == NB - 1),
                            )

    nc.compile()
    return nc


def _get_nc():
    if "nc" not in _CACHE:
        _CACHE["nc"] = build_nc()
    return _CACHE["nc"]


def _prep_maps(feat, weight, conv1_w, conv1_b, conv2_w, conv2_b, bases_buf, coef, bias):
    feat = np.asarray(feat, np.float32)
    weight = np.asarray(weight, np.float32)
    conv1_w = np.asarray(conv1_w, np.float32)
    conv2_w = np.asarray(conv2_w, np.float32)
    bases_buf = np.asarray(bases_buf, np.float32)
    coef = np.asarray(coef, np.float32)

    n = feat.shape[0]
    featp = np.zeros((n, C, HP, WP), np.float16)
    featp[:, :, 1 : H + 1, 1 : W + 1] = feat
    wgtp = np.zeros((n, CW, HP, WP), np.float16)
    wgtp[:, :, 1 : H + 1, 1 : W + 1] = weight

    # host-prepped transposed feature chunks: the 3 disjoint di input
    # windows (66 px each) packed into 2 chunks of 128 rows per group
    # fTd[q, (g*2+ci)*C + c] = fe[c, FOFF + 1 + g*GP + RELS[ci][q]]
    rels1 = list(range(-99, -33)) + list(range(-1, 61))
    rels2 = list(range(61, 65)) + list(range(97, 163))
    rels2 = rels2 + [163] * (C - len(rels2))
    RELS = [np.array(rels1), np.array(rels2)]
    fe = np.zeros((n, C, FEXT), np.float16)
    fe[:, :, FOFF : FOFF + NPAD] = featp.reshape(n, C, NPAD)
    fTdh = np.empty((n, 2 * NG, C, C), np.float16)
    for g in range(NG):
        for ci in range(2):
            idx = FOFF + 1 + g * GP + RELS[ci]
            fTdh[:, g * 2 + ci] = fe[:, :, idx].transpose(0, 2, 1)
    fTdh = np.ascontiguousarray(
        fTdh.transpose(0, 2, 1, 3).reshape(n, C, 2 * NG * C)
    )

    w1f = np.ascontiguousarray(
        conv1_w[:, :C].transpose(1, 2, 3, 0).reshape(C, L * C)
    ).astype(np.float16)
    w1w = np.ascontiguousarray(
        conv1_w[:, C:].transpose(1, 2, 3, 0).reshape(CW, L * C)
    ).astype(np.float16)
    # conv2 weights with k-major output-channel ordering (col = k*NB + m)
    w2h = np.ascontiguousarray(conv2_w[:, :, 0, 0].T).astype(np.float16)
    w2i = np.ascontiguousarray(
        w2h.reshape(C, NB, TEM).transpose(0, 2, 1).reshape(C, NBT)
    )
    # packed band blocks: bnd2[q, (kp*2+ci)*C + ki*GP + p] accumulates
    # bases_buf[2*kp+ki, di*3+dj] where chunk ci row q holds input offset
    # rel = p + (di-1)*WP + dj - 1
    pos = []
    for rels in RELS:
        d = {}
        for q, r in enumerate(rels.tolist()):
            if r not in d:
                d[r] = q
        pos.append(d)
    bndh = np.zeros((C, 3, 2, 2, GP), np.float32)
    for kp in range(3):
        for ki in range(2):
            k = 2 * kp + ki
            for di in range(3):
                for dj in range(3):
                    for p in range(GP):
                        rel = p + (di - 1) * WP + dj - 1
                        ci = 0 if rel in pos[0] else 1
                        bndh[pos[ci][rel], kp, ci, ki, p] += \
                            bases_buf[k, di * 3 + dj]
    bndh = bndh.reshape(C, 6 * C).astype(np.float16)
    # mask2: ones at (p,p) and (GP+p, p)
    mask2 = np.zeros((C, GP), np.float16)
    for p in range(GP):
        mask2[p, p] = 1.0
        mask2[GP + p, p] = 1.0
    coefTh = np.ascontiguousarray(
        coef[:, :, 0, 0].reshape(C, C, NB).transpose(1, 2, 0).reshape(C, NB * C)
    ).astype(np.float16)
    b1h = np.asarray(conv1_b, np.float32).reshape(C, 1)
    b2h = np.asarray(conv2_b, np.float32).reshape(NB, TEM).T.reshape(NBT, 1)
    b3h = np.asarray(bias, np.float32).reshape(C, 1)

    wgtq = np.zeros((n, C, NPAD), np.float16)
    wgtq[:, :CW] = wgtp.reshape(n, CW, NPAD)
    wgtq[:, CW:, : NPAD - WP] = wgtp.reshape(n, CW, NPAD)[:, :, WP:]
    # paired weights: rows 0-63 = tap (0,j), rows 64-127 = tap (1,j)
    wgtq2 = np.zeros((n, C, NPAD), np.float16)
    wgtq2[:, :CW] = wgtp.reshape(n, CW, NPAD)
    wgtq2[:, CW:, : NPAD - 1] = wgtp.reshape(n, CW, NPAD)[:, :, 1:]
    w1wq2 = np.concatenate(
        [
            w1w.reshape(CW, 3, 3, C)[:, 2, 0],
            w1w.reshape(CW, 3, 3, C)[:, 2, 1],
        ],
        axis=0,
    ).reshape(C, C)
    w1wq = np.concatenate(
        [
            w1w.reshape(CW, 3, 3, C)[:, 0],
            w1w.reshape(CW, 3, 3, C)[:, 1],
        ],
        axis=0,
    ).reshape(C, 3 * C)

    # transpose column selectors: P1 even-k cols, P2 odd-k cols
    p1 = np.zeros((C, 18), np.float16)
    p2 = np.zeros((C, 18), np.float16)
    for kp in range(3):
        for m in range(NB):
            p1[kp * 12 + m, kp * NB + m] = 1.0
            p2[kp * 12 + NB + m, kp * NB + m] = 1.0
    pkw = np.concatenate([w1wq, w1wq2, p1, p2], axis=1)
    pk = np.concatenate([w2i, coefTh, bndh, mask2], axis=1)
    pb = np.zeros((C, 3), np.float32)
    pb[:, 0:1] = b1h
    pb[:, 1:2] = b3h
    pb[:NBT, 2:3] = b2h
    shared = {"w1f": w1f, "pkw": pkw, "pk": pk,
              "w1w": np.ascontiguousarray(w1w[:, 8 * C:]), "pb": pb}
    return [
        {
            "featp": featp[i].reshape(C, NPAD).astype(np.float16),
            "wgtq": wgtq[i], "wgtq2": wgtq2[i],
            "fTd": fTdh[i],
            **shared,
        }
        for i in range(n)
    ]


def kernel(feat, weight, conv1_w, conv1_b, conv2_w, conv2_b, bases_buf, coef, bias,
           **run_kwargs):
    in_maps = _prep_maps(
        feat, weight, conv1_w, conv1_b, conv2_w, conv2_b, bases_buf, coef, bias
    )
    res = run_bass_kernel_spmd(
        _get_nc(), in_maps, core_ids=list(range(len(in_maps))), **run_kwargs
    )
    outp = np.stack([r["out"] for r in res.results], 0).astype(np.float32)
    outp = outp[:, :, :NPAD].reshape(-1, C, HP, WP)[:, :, 1 : H + 1, 1 : W + 1]
    _CACHE["last_results"] = res
    return np.ascontiguousarray(outp)


# revision 35
# speedup vs baseline: 1.5597x; 1.0030x over previous
"""Trainium2 Bass kernel for the DCF (dynamic conv filter) module.

Sharding: pure data-parallel over batch N=8 across 8 NeuronCores (one image
per core); all parameters replicated.

Pipeline per core (one 128x96x96 image):
  A:  conv1 (3x3, 192->128) + tanh -> hmid;  conv2 (1x1, 128->36) + tanh -> b
  B:  per 64-pixel group g:
        - SV: double transpose of the b rows + strided half-copies give the
          per-pixel scales in (k-parity x 64px, k-pair*6+m) layout
        - F: fixed-basis convs of feat via k-pair-interleaved banded matmuls
          on host-prepped transposed feature chunks (fTd), 3 di accumulated;
          output partitions are (k-parity*64 + pixel)
        - acc_m = sum_kpair fbS_kpair^T @ blockdiag2(s) -- each matmul
          contracts 2 k's at once (k lives in the contraction dim), so the
          k-reduction costs half the rows of the per-(m,k) diag scheme
        - out_group = sum_m coef_m @ acc_m (+bias)

Block-diagonals are built as tensor_scalar(mask2 * s) (DVE 4x mode); acc and
pso are double-buffered in PSUM so evacuation copies never stall the PE."""

from itertools import product

import numpy as np

import concourse.bass as bass
import concourse.tile as tile
from concourse import bacc, mybir
from concourse.bass_utils import run_bass_kernel_spmd
from concourse.masks import make_identity

fp16 = mybir.dt.float16
fp32 = mybir.dt.float32

N_CORES = 8
C = 128
CW = 64
H = W = 96
HP = WP = 98
NPIX = H * W
NPAD = HP * WP  # 9604
NB = 6
TEM = 6
L = 9
NBT = NB * TEM  # 36
RT = 4
FT = RT * W  # 384
NT = H // RT  # 24
GP = 64           # output pixels per group
NG = 152          # groups (cover padded idx 1 .. 1+152*64 = 9729)
BP = 9732         # padded bsb/out length
FEXT = 10000      # extended (host-side) padded feat length for fTd windows
FOFF = 98         # fTd window base offset inside the extended buffer
OG = 8            # output groups per store
FCH8 = 8 * 2 * C  # fTd cols per streamed chunk (8 groups)
NCH = (NG + 7) // 8  # 19 chunks

# build engine mix for the 18 blockdiag builds per group: 12 DVE (86ns in 4x
# mode), 4 Pool, 2 Act
_B18 = ["D", "P", "D", "P", "D", "P", "D", "D", "D",
        "P", "P", "D", "D", "P", "D", "P", "D", "D"]

_CACHE = {}


def build_nc():
    nc = bacc.Bacc("TRN2", target_bir_lowering=False, debug=False)

    featp = nc.dram_tensor("featp", [C, NPAD], fp16, kind="ExternalInput").ap()
    wgtq = nc.dram_tensor("wgtq", [C, NPAD], fp16, kind="ExternalInput").ap()
    fTd = nc.dram_tensor("fTd", [C, 2 * NG * C], fp16, kind="ExternalInput").ap()
    wgtq2 = nc.dram_tensor("wgtq2", [C, NPAD], fp16, kind="ExternalInput").ap()
    w1f = nc.dram_tensor("w1f", [C, L * C], fp16, kind="ExternalInput").ap()
    # conv1 weight-branch params + SV selectors: w1wq|w1wq2|P1|P2
    pkw = nc.dram_tensor("pkw", [C, 548], fp16, kind="ExternalInput").ap()
    # fp16 params packed: w2|coefT|bnd2|mask2 = 36+768+768+64
    pk = nc.dram_tensor("pk", [C, 1636], fp16, kind="ExternalInput").ap()
    w1w = nc.dram_tensor("w1w", [CW, C], fp16, kind="ExternalInput").ap()
    pb = nc.dram_tensor("pb", [C, 3], fp32, kind="ExternalInput").ap()
    out = nc.dram_tensor("out", [C, BP], fp16, kind="ExternalOutput").ap()

    Tanh = mybir.ActivationFunctionType.Tanh
    Ident = mybir.ActivationFunctionType.Identity
    Copy = mybir.ActivationFunctionType.Copy
    MUL = mybir.AluOpType.mult
    ADD = mybir.AluOpType.add

    with tile.TileContext(nc) as tc:
        with (
            tc.tile_pool(name="const", bufs=1) as const,
            tc.tile_pool(name="big", bufs=1) as big,
        ):
            featp_sb = big.tile([C, NPAD], fp16)
            wgtp_sb = big.tile([C, NPAD], fp16)
            cuts = [0, 2404, 4808, 7212, NPAD]
            # startup: tiny "head" pieces first so the first conv row-tile's
            # operands land within ~2.5us (DMA pipeline latency floor)
            HD = 686  # cols covering feat/wgt rows 0..6 (row-tile 0 + halo)
            w1f_sb = const.tile([C, L * C], fp16)
            nc.sync.dma_start(w1f_sb[:, : 4 * C], w1f[:, : 4 * C])
            nc.sync.dma_start(featp_sb[:, :HD], featp[:, :HD])
            nc.sync.dma_start(w1f_sb[:, 4 * C :], w1f[:, 4 * C :])
            pkw_sb = const.tile([C, 548], fp16)
            nc.sync.dma_start(pkw_sb[:], pkw)
            w1wq_sb = pkw_sb[:, 0:384]
            w1wq2_sb = pkw_sb[:, 384:512]
            p1_sb = pkw_sb[:NBT, 512:530]
            p2_sb = pkw_sb[:NBT, 530:548]
            wgtq2_sb = big.tile([C, NPAD], fp16)
            nc.sync.dma_start(wgtp_sb[:, :HD], wgtq[:, :HD])
            nc.sync.dma_start(wgtq2_sb[:, :HD], wgtq2[:, :HD])
            pb_sb = const.tile([C, 3], fp32)
            nc.sync.dma_start(pb_sb[:], pb)
            b1_sb = pb_sb[:, 0:1]
            b3_sb = pb_sb[:, 1:2]
            b2_sb = pb_sb[:NBT, 2:3]
            w1w_sb = const.tile([CW, C], fp16)
            nc.sync.dma_start(w1w_sb[:], w1w)
            nc.sync.dma_start(featp_sb[:, HD : cuts[1]], featp[:, HD : cuts[1]])
            pk_sb = const.tile([C, 1636], fp16)
            nc.sync.dma_start(pk_sb[:, 804:], pk[:, 804:])
            nc.sync.dma_start(pk_sb[:, :804], pk[:, :804])
            ft0_sb = big.tile([C, FCH8], fp16)
            nc.sync.dma_start(ft0_sb[:], fTd[:, :FCH8])
            w2_sb = pk_sb[:, 0:36]
            coefT_sb = pk_sb[:, 36:804]
            bnd2_sb = pk_sb[:, 804:1572]
            mask2_sb = pk_sb[:, 1572:1636]
            nc.sync.dma_start(wgtq2_sb[:, HD : cuts[1]], wgtq2[:, HD : cuts[1]])
            ft1_sb = big.tile([C, FCH8], fp16)
            nc.sync.dma_start(ft1_sb[:], fTd[:, FCH8 : 2 * FCH8])
            nc.sync.dma_start(wgtp_sb[:, HD : cuts[2]], wgtq[:, HD : cuts[2]])
            nc.sync.dma_start(
                featp_sb[:, cuts[1] : cuts[2]], featp[:, cuts[1] : cuts[2]]
            )
            nc.sync.dma_start(wgtq2_sb[:, cuts[1] :], wgtq2[:, cuts[1] :])
            nc.sync.dma_start(
                featp_sb[:, cuts[2] : cuts[3]], featp[:, cuts[2] : cuts[3]]
            )
            nc.sync.dma_start(wgtp_sb[:, cuts[2] :], wgtq[:, cuts[2] :])
            nc.sync.dma_start(
                featp_sb[:, cuts[3] :], featp[:, cuts[3] :]
            )
            bsb = big.tile([NBT, BP], fp16)
            # zero only the border/tail cells conv2 never writes (full memset
            # would hold Pool for 8us before the first b write)
            nc.gpsimd.memset(bsb[:, : WP + 2], 0.0)
            edge = bsb[:, 97 : 97 + 97 * WP].rearrange("c (r w) -> c r w", w=WP)
            nc.gpsimd.memset(edge[:, :, 0:2], 0.0)
            nc.gpsimd.memset(bsb[:, 97 * WP :], 0.0)

            b3d = bsb[:, :NPAD].rearrange("c (r w) -> c r w", w=WP)
            f3 = featp_sb[:].rearrange("c (r w) -> c r w", w=WP)
            w3 = wgtp_sb[:].rearrange("c (r w) -> c r w", w=WP)
            wq2 = wgtq2_sb[:].rearrange("c (r w) -> c r w", w=WP)

            # ---- fused pipeline: conv rows (phase A) stream in between the
            # software-pipelined per-group stages of phase B.
            with (
                tc.tile_pool(name="hmp", bufs=3) as hmp,
                tc.tile_pool(name="ftp", bufs=3) as ftp,
                tc.tile_pool(name="svsp", bufs=4) as svsp,
                tc.tile_pool(name="fbp", bufs=3) as fbp,
                tc.tile_pool(name="dgp", bufs=3) as dgp,
                tc.tile_pool(name="bop", bufs=3) as bop,
                tc.tile_pool(name="orp", bufs=2) as orp,
                tc.tile_pool(name="psB2", bufs=1, space="PSUM") as psB2,
            ):
                fbS_r, dg_r, boS_r, pso_r, acc_r = {}, {}, {}, {}, {}
                # two pso slots manually ring-buffered inside ONE psum bank
                psoD = psB2.tile([C, 2 * GP], fp32, tag="pso", bufs=1,
                                 name="psoD")
                hm_r = {}
                ps_r, ps2_r = {}, {}
                _SPLIT = set(range(2, 14))  # rows with b-deadline slack
                svS_r, ft_r = {}, {}
                orow_bufs = {}

                def emit_arow_f(t):
                    r0 = t * RT
                    ps = psB2.tile([C, FT], fp32, tag="psA", bufs=1, name="ps")
                    for kk, (i, j) in enumerate(product(range(3), range(3))):
                        nc.tensor.matmul(
                            ps[:],
                            w1f_sb[:, (i * 3 + j) * C : (i * 3 + j + 1) * C],
                            f3[:, r0 + i : r0 + i + RT, j : j + W],
                            start=(kk == 0),
                            stop=False,
                        )
                    return ps

                def emit_arow_w(t, ps):
                    r0 = t * RT
                    for j in range(3):
                        nc.tensor.matmul(
                            ps[:],
                            w1wq_sb[:, j * C : (j + 1) * C],
                            w3[:, r0 : r0 + RT, j : j + W],
                            start=False,
                            stop=False,
                        )
                    nc.tensor.matmul(
                        ps[:],
                        w1wq2_sb,
                        wq2[:, r0 + 2 : r0 + 2 + RT, 0 : W],
                        start=False,
                        stop=False,
                    )
                    nc.tensor.matmul(
                        ps[:],
                        w1w_sb[:],
                        w3[:64, r0 + 2 : r0 + 2 + RT, 2 : 2 + W],
                        start=False,
                        stop=True,
                    )
                    hm = hmp.tile([C, FT], fp16, tag="hm")
                    if t in _SPLIT:
                        nc.scalar.activation(hm[:, : FT // 2], ps[:, : FT // 2],
                                             Tanh, bias=b1_sb)
                        ps_r[t] = ps
                    else:
                        nc.scalar.activation(hm[:], ps[:], Tanh, bias=b1_sb)
                    hm_r[t] = hm

                def emit_arow_h2(t):
                    ps = ps_r.pop(t)
                    hm = hm_r[t]
                    nc.scalar.activation(hm[:, FT // 2 :], ps[:, FT // 2 :],
                                         Tanh, bias=b1_sb)

                def emit_arow_b(t, half=None):
                    r0 = t * RT
                    if half in (None, 0):
                        hm = hm_r.pop(t)
                        ps2 = psB2.tile([NBT, FT], fp32, tag="psB", bufs=1,
                                        name="ps2")
                        nc.tensor.matmul(ps2[:], w2_sb, hm[:], start=True,
                                         stop=True)
                    if half == 0:
                        ps2_r[t] = ps2
                    elif half == 1:
                        ps2 = ps2_r.pop(t)
                    p3 = ps2[:].rearrange("c (r w) -> c r w", w=W)
                    if half is None:
                        nc.scalar.activation(
                            b3d[:, r0 + 1 : r0 + 1 + RT, 1 : 1 + W],
                            p3, Tanh, bias=b2_sb)
                    elif half == 0:
                        nc.scalar.activation(
                            b3d[:, r0 + 1 : r0 + 3, 1 : 1 + W],
                            p3[:, : RT // 2], Tanh, bias=b2_sb)
                    else:
                        nc.scalar.activation(
                            b3d[:, r0 + 3 : r0 + 5, 1 : 1 + W],
                            p3[:, RT // 2 :], Tanh, bias=b2_sb)

                def emit_sv(g):
                    # b rows (k-major: row = k*6+m) -> per-pixel scales in
                    # (k-parity*64+p, kpair*6+m) layout: the two transposes
                    # use column-selector matrices (even/odd k) so svp lands
                    # pre-interleaved; one contiguous copy evacuates it
                    svp = psB2.tile([C, 18], fp32, tag="svp", bufs=1,
                                    name="svp")
                    src = bsb[:, 1 + g * GP : 1 + (g + 1) * GP]
                    nc.tensor.matmul(svp[:GP, :], src, p1_sb,
                                     start=True, stop=True)
                    nc.tensor.matmul(svp[GP:, :], src, p2_sb,
                                     start=True, stop=True)
                    svS = svsp.tile([C, 18], fp32, tag="svS")
                    svS_r[g] = svS
                    nc.vector.tensor_copy(svS[:], svp[:])

                def emit_builds(g, dg, js):
                    svS = svS_r[g]
                    for j in js:
                        m, kp = divmod(j, 3)
                        sc = svS[:, kp * NB + m : kp * NB + m + 1]
                        dslice = dg[:, j * GP : (j + 1) * GP]
                        eng = _B18[j]
                        if eng == "D":
                            nc.vector.tensor_scalar(
                                dslice, mask2_sb, sc, None, MUL
                            )
                        elif eng == "P":
                            nc.gpsimd.tensor_scalar(
                                dslice, mask2_sb, sc, None, MUL
                            )
                        else:
                            nc.scalar.activation(dslice, mask2_sb, Copy,
                                                 scale=sc)

                # conv row-tile slots spread at cadence ~6.5 so conv filler
                # lasts until iteration ~138 (row 23 deadline is ~141)
                _fs = {int(_r * 6.5 + 0.5): _r + 2 for _r in range(NT - 2)}
                _ws = {k + 1: v for k, v in _fs.items()}
                _h2 = {k + 2: v for k, v in _fs.items() if v in _SPLIT}
                _bs = {k + 2: v for k, v in _fs.items() if v not in _SPLIT}
                _b1 = {k + 3: v for k, v in _fs.items() if v in _SPLIT}
                _b2x = {k + 4: v for k, v in _fs.items() if v in _SPLIT}

                for r in range(2):
                    emit_arow_w(r, emit_arow_f(r))
                    emit_arow_b(r)
                # fTd chunks 0,1 were DMA'd in the startup sequence
                ft_r[0] = ft0_sb
                ft_r[1] = ft1_sb
                emit_sv(0)
                emit_sv(1)

                for i in range(NG + 4):
                    if i in _fs:
                        arow_ps = emit_arow_f(_fs[i])
                    elif i in _ws:
                        emit_arow_w(_ws[i], arow_ps)
                    elif i in _h2:
                        emit_arow_h2(_h2[i])
                    elif i in _bs:
                        emit_arow_b(_bs[i])
                    elif i in _b1:
                        emit_arow_b(_b1[i], half=0)
                    elif i in _b2x:
                        emit_arow_b(_b2x[i], half=1)
                    # acc(i-2) psum -> SBUF (Pool)
                    if 0 <= i - 2 < NG:
                        boS = bop.tile([C, NB * GP], fp16, tag="boS")
                        boS_r[i - 2] = boS
                        acc = acc_r.pop(i - 2)
                        nc.vector.tensor_copy(boS[:, : 3 * GP], acc[:, : 3 * GP])
                        nc.scalar.copy(boS[:, 3 * GP :], acc[:, 3 * GP :])
                    # orow(i-3) + store
                    if 0 <= i - 4 < NG:
                        j = i - 4
                        g8 = j % OG
                        if g8 == 0:
                            orow_bufs[j] = orp.tile(
                                [C, OG * GP], fp16, tag="orow", name="orow_buf"
                            )
                        ob = orow_bufs[j - g8]
                        nc.scalar.activation(
                            ob[:, g8 * GP : (g8 + 1) * GP], pso_r.pop(j),
                            Ident, bias=b3_sb,
                        )
                        if j >= NG - 4:
                            # tail: store each group as soon as it lands so
                            # the final DMA only waits on the last orow
                            nc.sync.dma_start(
                                out[:, 1 + j * GP : 1 + (j + 1) * GP],
                                ob[:, g8 * GP : (g8 + 1) * GP],
                            )
                            if j == NG - 1:
                                del orow_bufs[j - g8]
                        elif g8 == OG - 1:
                            t0 = j - g8
                            nc.sync.dma_start(
                                out[:, 1 + t0 * GP : 1 + t0 * GP + OG * GP],
                                ob[:, : OG * GP],
                            )
                            del orow_bufs[t0]
                        elif j == NG - 5 and g8 == 3:
                            t0 = j - g8
                            nc.sync.dma_start(
                                out[:, 1 + t0 * GP : 1 + t0 * GP + 4 * GP],
                                ob[:, : 4 * GP],
                            )
                    # stream next fTd chunk
                    if i % 8 == 0 and i // 8 + 2 < NCH:
                        cch = i // 8 + 2
                        ft = ftp.tile([C, FCH8], fp16, tag="ft")
                        ft_r[cch] = ft
                        nc.sync.dma_start(
                            ft[:], fTd[:, cch * FCH8 : (cch + 1) * FCH8]
                        )
                    # per-pixel scales two groups ahead
                    if i + 2 < NG:
                        emit_sv(i + 2)
                    # F(i), blockdiag builds(i), F evac(i)
                    if i < NG:
                        psf = psB2.tile([C, 3 * C], fp32, tag="psf", bufs=2,
                                        name="psf")
                        ft = ft_r[i // 8]
                        for kp in range(3):
                            for ci in range(2):
                                nc.tensor.matmul(
                                    psf[:, kp * C : (kp + 1) * C],
                                    bnd2_sb[:, (kp * 2 + ci) * C
                                            : (kp * 2 + ci + 1) * C],
                                    ft[:, ((i % 8) * 2 + ci) * C
                                       : ((i % 8) * 2 + ci + 1) * C],
                                    start=(ci == 0),
                                    stop=(ci == 1),
                                )
                        dg = dgp.tile([C, 18 * GP], fp16, tag="dg")
                        dg_r[i] = dg
                        emit_builds(i, dg, range(9))
                        fbS = fbp.tile([C, 3 * C], fp16, tag="fbS")
                        nc.scalar.copy(fbS[:], psf[:])
                        fbS_r[i] = fbS
                        emit_builds(i, dg, range(9, 18))
                        if i % 8 == 7 or i == NG - 1:
                            ft_r.pop(i // 8)
                    # blockdiag matmuls (i-1): acc_m = sum_kp fbS_kp^T @ bd2
                    if 0 <= i - 1 < NG:
                        acc = psB2.tile([C, NB * GP], fp32, tag="accT", bufs=2,
                                        name="acc")
                        acc_r[i - 1] = acc
                        fbS = fbS_r.pop(i - 1)
                        dg = dg_r.pop(i - 1)
                        for m in range(NB):
                            for kp in range(3):
                                j = m * 3 + kp
                                nc.tensor.matmul(
                                    acc[:, m * GP : (m + 1) * GP],
                                    fbS[:, kp * C : (kp + 1) * C],
                                    dg[:, j * GP : (j + 1) * GP],
                                    start=(kp == 0),
                                    stop=(kp == 2),
                                )
                    # coef matmuls (i-2)
                    if 0 <= i - 3 < NG:
                        _s = ((i - 3) % 2) * GP
                        pso = psoD[:, _s : _s + GP]
                        pso_r[i - 3] = pso
                        boS = boS_r.pop(i - 3)
                        for m in range(NB):
                            nc.tensor.matmul(
                                pso[:],
                                coefT_sb[:, m * C : (m + 1) * C],
                                boS[:, m * GP : (m + 1) * GP],
                                start=(m == 0),
                                stop=(m == NB - 1),
                            )

    nc.compile()
    return nc


def _get_nc():
    if "nc" not in _CACHE:
        _CACHE["nc"] = build_nc()
    return _CACHE["nc"]


def _prep_maps(feat, weight, conv1_w, conv1_b, conv2_w, conv2_b, bases_buf, coef, bias):
    feat = np.asarray(feat, np.float32)
    weight = np.asarray(weight, np.float32)
    conv1_w = np.asarray(conv1_w, np.float32)
    conv2_w = np.asarray(conv2_w, np.float32)
    bases_buf = np.asarray(bases_buf, np.float32)
    coef = np.asarray(coef, np.float32)

    n = feat.shape[0]
    featp = np.zeros((n, C, HP, WP), np.float16)
    featp[:, :, 1 : H + 1, 1 : W + 1] = feat
    wgtp = np.zeros((n, CW, HP, WP), np.float16)
    wgtp[:, :, 1 : H + 1, 1 : W + 1] = weight

    # host-prepped transposed feature chunks: the 3 disjoint di input
    # windows (66 px each) packed into 2 chunks of 128 rows per group
    # fTd[q, (g*2+ci)*C + c] = fe[c, FOFF + 1 + g*GP + RELS[ci][q]]
    rels1 = list(range(-99, -33)) + list(range(-1, 61))
    rels2 = list(range(61, 65)) + list(range(97, 163))
    rels2 = rels2 + [163] * (C - len(rels2))
    RELS = [np.array(rels1), np.array(rels2)]
    fe = np.zeros((n, C, FEXT), np.float16)
    fe[:, :, FOFF : FOFF + NPAD] = featp.reshape(n, C, NPAD)
    fTdh = np.empty((n, 2 * NG, C, C), np.float16)
    for g in range(NG):
        for ci in range(2):
            idx = FOFF + 1 + g * GP + RELS[ci]
            fTdh[:, g * 2 + ci] = fe[:, :, idx].transpose(0, 2, 1)
    fTdh = np.ascontiguousarray(
        fTdh.transpose(0, 2, 1, 3).reshape(n, C, 2 * NG * C)
    )

    w1f = np.ascontiguousarray(
        conv1_w[:, :C].transpose(1, 2, 3, 0).reshape(C, L * C)
    ).astype(np.float16)
    w1w = np.ascontiguousarray(
        conv1_w[:, C:].transpose(1, 2, 3, 0).reshape(CW, L * C)
    ).astype(np.float16)
    # conv2 weights with k-major output-channel ordering (col = k*NB + m)
    w2h = np.ascontiguousarray(conv2_w[:, :, 0, 0].T).astype(np.float16)
    w2i = np.ascontiguousarray(
        w2h.reshape(C, NB, TEM).transpose(0, 2, 1).reshape(C, NBT)
    )
    # packed band blocks: bnd2[q, (kp*2+ci)*C + ki*GP + p] accumulates
    # bases_buf[2*kp+ki, di*3+dj] where chunk ci row q holds input offset
    # rel = p + (di-1)*WP + dj - 1
    pos = []
    for rels in RELS:
        d = {}
        for q, r in enumerate(rels.tolist()):
            if r not in d:
                d[r] = q
        pos.append(d)
    bndh = np.zeros((C, 3, 2, 2, GP), np.float32)
    for kp in range(3):
        for ki in range(2):
            k = 2 * kp + ki
            for di in range(3):
                for dj in range(3):
                    for p in range(GP):
                        rel = p + (di - 1) * WP + dj - 1
                        ci = 0 if rel in pos[0] else 1
                        bndh[pos[ci][rel], kp, ci, ki, p] += \
                            bases_buf[k, di * 3 + dj]
    bndh = bndh.reshape(C, 6 * C).astype(np.float16)
    # mask2: ones at (p,p) and (GP+p, p)
    mask2 = np.zeros((C, GP), np.float16)
    for p in range(GP):
        mask2[p, p] = 1.0
        mask2[GP + p, p] = 1.0
    coefTh = np.ascontiguousarray(
        coef[:, :, 0, 0].reshape(C, C, NB).transpose(1, 2, 0).reshape(C, NB * C)
    ).astype(np.float16)
    b1h = np.asarray(conv1_b, np.float32).reshape(C, 1)
    b2h = np.asarray(conv2_b, np.float32).reshape(NB, TEM).T.reshape(NBT, 1)
    b3h = np.asarray(bias, np.float32).reshape(C, 1)

    wgtq = np.zeros((n, C, NPAD), np.float16)
    wgtq[:, :CW] = wgtp.reshape(n, CW, NPAD)
    wgtq[:, CW:, : NPAD - WP] = wgtp.reshape(n, CW, NPAD)[:, :, WP:]
    # paired weights: rows 0-63 = tap (0,j), rows 64-127 = tap (1,j)
    wgtq2 = np.zeros((n, C, NPAD), np.float16)
    wgtq2[:, :CW] = wgtp.reshape(n, CW, NPAD)
    wgtq2[:, CW:, : NPAD - 1] = wgtp.reshape(n, CW, NPAD)[:, :, 1:]
    w1wq2 = np.concatenate(
        [
            w1w.reshape(CW, 3, 3, C)[:, 2, 0],
            w1w.reshape(CW, 3, 3, C)[:, 2, 1],
        ],
        axis=0,
    ).reshape(C, C)
    w1wq = np.concatenate(
        [
            w1w.reshape(CW, 3, 3, C)[:, 0],
            w1w.reshape(CW, 3, 3, C)[:, 1],
        ],
        axis=0,
    ).reshape(C, 3 * C)

    # transpose column selectors: P1 even-k cols, P2 odd-k cols
    p1 = np.zeros((C, 18), np.float16)
    p2 = np.zeros((C, 18), np.float16)
    for kp in range(3):
        for m in range(NB):
            p1[kp * 12 + m, kp * NB + m] = 1.0
            p2[kp * 12 + NB + m, kp * NB + m] = 1.0
    pkw = np.concatenate([w1wq, w1wq2, p1, p2], axis=1)
    pk = np.concatenate([w2i, coefTh, bndh, mask2], axis=1)
    pb = np.zeros((C, 3), np.float32)
    pb[:, 0:1] = b1h
    pb[:, 1:2] = b3h
    pb[:NBT, 2:3] = b2h
    shared = {"w1f": w1f, "pkw": pkw, "pk": pk,
              "w1w": np.ascontiguousarray(w1w[:, 8 * C:]), "pb": pb}
    return [
        {
            "featp": featp[i].reshape(C, NPAD).astype(np.float16),
            "wgtq": wgtq[i], "wgtq2": wgtq2[i],
            "fTd": fTdh[i],
            **shared,
        }
        for i in range(n)
    ]


def kernel(feat, weight, conv1_w, conv1_b, conv2_w, conv2_b, bases_buf, coef, bias,
           **run_kwargs):
    in_maps = _prep_maps(
        feat, weight, conv1_w, conv1_b, conv2_w, conv2_b, bases_buf, coef, bias
    )
    res = run_bass_kernel_spmd(
        _get_nc(), in_maps, core_ids=list(range(len(in_maps))), **run_kwargs
    )
    outp = np.stack([r["out"] for r in res.results], 0).astype(np.float32)
    outp = outp[:, :, :NPAD].reshape(-1, C, HP, WP)[:, :, 1 : H + 1, 1 : W + 1]
    _CACHE["last_results"] = res
    return np.ascontiguousarray(outp)


# revision 37
# speedup vs baseline: 1.5602x; 1.0003x over previous
"""Trainium2 Bass kernel for the DCF (dynamic conv filter) module.

Sharding: pure data-parallel over batch N=8 across 8 NeuronCores (one image
per core); all parameters replicated.

Pipeline per core (one 128x96x96 image):
  A:  conv1 (3x3, 192->128) + tanh -> hmid;  conv2 (1x1, 128->36) + tanh -> b
  B:  per 64-pixel group g:
        - SV: double transpose of the b rows + strided half-copies give the
          per-pixel scales in (k-parity x 64px, k-pair*6+m) layout
        - F: fixed-basis convs of feat via k-pair-interleaved banded matmuls
          on host-prepped transposed feature chunks (fTd), 3 di accumulated;
          output partitions are (k-parity*64 + pixel)
        - acc_m = sum_kpair fbS_kpair^T @ blockdiag2(s) -- each matmul
          contracts 2 k's at once (k lives in the contraction dim), so the
          k-reduction costs half the rows of the per-(m,k) diag scheme
        - out_group = sum_m coef_m @ acc_m (+bias)

Block-diagonals are built as tensor_scalar(mask2 * s) (DVE 4x mode); acc and
pso are double-buffered in PSUM so evacuation copies never stall the PE."""

from itertools import product

import numpy as np

import concourse.bass as bass
import concourse.tile as tile
from concourse import bacc, mybir
from concourse.bass_utils import run_bass_kernel_spmd
from concourse.masks import make_identity

fp16 = mybir.dt.float16
fp32 = mybir.dt.float32

N_CORES = 8
C = 128
CW = 64
H = W = 96
HP = WP = 98
NPIX = H * W
NPAD = HP * WP  # 9604
NB = 6
TEM = 6
L = 9
NBT = NB * TEM  # 36
RT = 4
FT = RT * W  # 384
NT = H // RT  # 24
GP = 64           # output pixels per group
NG = 152          # groups (cover padded idx 1 .. 1+152*64 = 9729)
BP = 9732         # padded bsb/out length
FEXT = 10000      # extended (host-side) padded feat length for fTd windows
FOFF = 98         # fTd window base offset inside the extended buffer
OG = 8            # output groups per store
FCH8 = 8 * 2 * C  # fTd cols per streamed chunk (8 groups)
NCH = (NG + 7) // 8  # 19 chunks

# build engine mix for the 18 blockdiag builds per group: 12 DVE (86ns in 4x
# mode), 4 Pool, 2 Act
_B18 = ["D", "P", "D", "P", "D", "P", "D", "D", "D",
        "P", "P", "D", "D", "P", "D", "P", "D", "D"]

_CACHE = {}


def build_nc():
    nc = bacc.Bacc("TRN2", target_bir_lowering=False, debug=False)

    featp = nc.dram_tensor("featp", [C, NPAD], fp16, kind="ExternalInput").ap()
    wgtq = nc.dram_tensor("wgtq", [C, NPAD], fp16, kind="ExternalInput").ap()
    fTd = nc.dram_tensor("fTd", [C, 2 * NG * C], fp16, kind="ExternalInput").ap()
    wgtq2 = nc.dram_tensor("wgtq2", [C, NPAD], fp16, kind="ExternalInput").ap()
    w1f = nc.dram_tensor("w1f", [C, L * C], fp16, kind="ExternalInput").ap()
    # conv1 weight-branch params + SV selectors: w1wq|w1wq2|P1|P2
    pkw = nc.dram_tensor("pkw", [C, 548], fp16, kind="ExternalInput").ap()
    # fp16 params packed: w2|coefT|bnd2|mask2 = 36+768+768+64
    pk = nc.dram_tensor("pk", [C, 1636], fp16, kind="ExternalInput").ap()
    w1w = nc.dram_tensor("w1w", [CW, C], fp16, kind="ExternalInput").ap()
    pb = nc.dram_tensor("pb", [C, 3], fp32, kind="ExternalInput").ap()
    out = nc.dram_tensor("out", [C, BP], fp16, kind="ExternalOutput").ap()

    Tanh = mybir.ActivationFunctionType.Tanh
    Ident = mybir.ActivationFunctionType.Identity
    Copy = mybir.ActivationFunctionType.Copy
    MUL = mybir.AluOpType.mult
    ADD = mybir.AluOpType.add

    with tile.TileContext(nc) as tc:
        with (
            tc.tile_pool(name="const", bufs=1) as const,
            tc.tile_pool(name="big", bufs=1) as big,
        ):
            featp_sb = big.tile([C, NPAD], fp16)
            wgtp_sb = big.tile([C, NPAD], fp16)
            cuts = [0, 2404, 4808, 7212, NPAD]
            # startup: tiny "head" pieces first so the first conv row-tile's
            # operands land within ~2.5us (DMA pipeline latency floor)
            HD = 686  # cols covering feat/wgt rows 0..6 (row-tile 0 + halo)
            w1f_sb = const.tile([C, L * C], fp16)
            nc.sync.dma_start(w1f_sb[:, : 4 * C], w1f[:, : 4 * C])
            nc.sync.dma_start(featp_sb[:, :HD], featp[:, :HD])
            nc.sync.dma_start(w1f_sb[:, 4 * C :], w1f[:, 4 * C :])
            pkw_sb = const.tile([C, 548], fp16)
            nc.sync.dma_start(pkw_sb[:], pkw)
            w1wq_sb = pkw_sb[:, 0:384]
            w1wq2_sb = pkw_sb[:, 384:512]
            p1_sb = pkw_sb[:NBT, 512:530]
            p2_sb = pkw_sb[:NBT, 530:548]
            wgtq2_sb = big.tile([C, NPAD], fp16)
            nc.sync.dma_start(wgtp_sb[:, :HD], wgtq[:, :HD])
            nc.sync.dma_start(wgtq2_sb[:, :HD], wgtq2[:, :HD])
            pb_sb = const.tile([C, 3], fp32)
            nc.sync.dma_start(pb_sb[:], pb)
            b1_sb = pb_sb[:, 0:1]
            b3_sb = pb_sb[:, 1:2]
            b2_sb = pb_sb[:NBT, 2:3]
            w1w_sb = const.tile([CW, C], fp16)
            nc.sync.dma_start(w1w_sb[:], w1w)
            nc.sync.dma_start(featp_sb[:, HD : cuts[1]], featp[:, HD : cuts[1]])
            pk_sb = const.tile([C, 1636], fp16)
            nc.sync.dma_start(pk_sb[:, 804:], pk[:, 804:])
            nc.sync.dma_start(pk_sb[:, :804], pk[:, :804])
            ft0_sb = big.tile([C, FCH8], fp16)
            nc.sync.dma_start(ft0_sb[:], fTd[:, :FCH8])
            w2_sb = pk_sb[:, 0:36]
            coefT_sb = pk_sb[:, 36:804]
            bnd2_sb = pk_sb[:, 804:1572]
            mask2_sb = pk_sb[:, 1572:1636]
            nc.sync.dma_start(wgtq2_sb[:, HD : cuts[1]], wgtq2[:, HD : cuts[1]])
            ft1_sb = big.tile([C, FCH8], fp16)
            nc.sync.dma_start(ft1_sb[:], fTd[:, FCH8 : 2 * FCH8])
            nc.sync.dma_start(wgtp_sb[:, HD : cuts[2]], wgtq[:, HD : cuts[2]])
            nc.sync.dma_start(
                featp_sb[:, cuts[1] : cuts[2]], featp[:, cuts[1] : cuts[2]]
            )
            nc.sync.dma_start(wgtq2_sb[:, cuts[1] :], wgtq2[:, cuts[1] :])
            nc.sync.dma_start(
                featp_sb[:, cuts[2] : cuts[3]], featp[:, cuts[2] : cuts[3]]
            )
            nc.sync.dma_start(wgtp_sb[:, cuts[2] :], wgtq[:, cuts[2] :])
            nc.sync.dma_start(
                featp_sb[:, cuts[3] :], featp[:, cuts[3] :]
            )
            bsb = big.tile([NBT, BP], fp16)
            # zero only the border/tail cells conv2 never writes (full memset
            # would hold Pool for 8us before the first b write)
            nc.gpsimd.memset(bsb[:, : WP + 2], 0.0)
            edge = bsb[:, 97 : 97 + 97 * WP].rearrange("c (r w) -> c r w", w=WP)
            nc.gpsimd.memset(edge[:, :, 0:2], 0.0)
            nc.gpsimd.memset(bsb[:, 97 * WP :], 0.0)

            b3d = bsb[:, :NPAD].rearrange("c (r w) -> c r w", w=WP)
            f3 = featp_sb[:].rearrange("c (r w) -> c r w", w=WP)
            w3 = wgtp_sb[:].rearrange("c (r w) -> c r w", w=WP)
            wq2 = wgtq2_sb[:].rearrange("c (r w) -> c r w", w=WP)

            # ---- fused pipeline: conv rows (phase A) stream in between the
            # software-pipelined per-group stages of phase B.
            with (
                tc.tile_pool(name="hmp", bufs=3) as hmp,
                tc.tile_pool(name="ftp", bufs=3) as ftp,
                tc.tile_pool(name="svsp", bufs=6) as svsp,
                tc.tile_pool(name="fbp", bufs=3) as fbp,
                tc.tile_pool(name="dgp", bufs=3) as dgp,
                tc.tile_pool(name="bop", bufs=3) as bop,
                tc.tile_pool(name="orp", bufs=2) as orp,
                tc.tile_pool(name="psB2", bufs=1, space="PSUM") as psB2,
            ):
                fbS_r, dg_r, boS_r, pso_r, acc_r = {}, {}, {}, {}, {}
                # two pso slots manually ring-buffered inside ONE psum bank
                psoD = psB2.tile([C, 2 * GP], fp32, tag="pso", bufs=1,
                                 name="psoD")
                hm_r = {}
                ps_r, ps2_r = {}, {}
                _SPLIT = set(range(2, 22))  # rows with b-deadline slack
                svS_r, ft_r = {}, {}
                orow_bufs = {}

                def emit_arow_f(t):
                    r0 = t * RT
                    ps = psB2.tile([C, FT], fp32, tag="psA", bufs=1, name="ps")
                    for kk, (i, j) in enumerate(product(range(3), range(3))):
                        nc.tensor.matmul(
                            ps[:],
                            w1f_sb[:, (i * 3 + j) * C : (i * 3 + j + 1) * C],
                            f3[:, r0 + i : r0 + i + RT, j : j + W],
                            start=(kk == 0),
                            stop=False,
                        )
                    return ps

                def emit_arow_w(t, ps):
                    r0 = t * RT
                    for j in range(3):
                        nc.tensor.matmul(
                            ps[:],
                            w1wq_sb[:, j * C : (j + 1) * C],
                            w3[:, r0 : r0 + RT, j : j + W],
                            start=False,
                            stop=False,
                        )
                    nc.tensor.matmul(
                        ps[:],
                        w1wq2_sb,
                        wq2[:, r0 + 2 : r0 + 2 + RT, 0 : W],
                        start=False,
                        stop=False,
                    )
                    nc.tensor.matmul(
                        ps[:],
                        w1w_sb[:],
                        w3[:64, r0 + 2 : r0 + 2 + RT, 2 : 2 + W],
                        start=False,
                        stop=True,
                    )
                    hm = hmp.tile([C, FT], fp16, tag="hm")
                    if t in _SPLIT:
                        nc.scalar.activation(hm[:, : FT // 2], ps[:, : FT // 2],
                                             Tanh, bias=b1_sb)
                        ps_r[t] = ps
                    else:
                        nc.scalar.activation(hm[:], ps[:], Tanh, bias=b1_sb)
                    hm_r[t] = hm

                def emit_arow_h2(t):
                    ps = ps_r.pop(t)
                    hm = hm_r[t]
                    nc.scalar.activation(hm[:, FT // 2 :], ps[:, FT // 2 :],
                                         Tanh, bias=b1_sb)

                def emit_arow_b(t, half=None):
                    r0 = t * RT
                    if half in (None, 0):
                        hm = hm_r.pop(t)
                        ps2 = psB2.tile([NBT, FT], fp32, tag="psB", bufs=1,
                                        name="ps2")
                        nc.tensor.matmul(ps2[:], w2_sb, hm[:], start=True,
                                         stop=True)
                    if half == 0:
                        ps2_r[t] = ps2
                    elif half == 1:
                        ps2 = ps2_r.pop(t)
                    p3 = ps2[:].rearrange("c (r w) -> c r w", w=W)
                    if half is None:
                        nc.scalar.activation(
                            b3d[:, r0 + 1 : r0 + 1 + RT, 1 : 1 + W],
                            p3, Tanh, bias=b2_sb)
                    elif half == 0:
                        nc.scalar.activation(
                            b3d[:, r0 + 1 : r0 + 3, 1 : 1 + W],
                            p3[:, : RT // 2], Tanh, bias=b2_sb)
                    else:
                        nc.scalar.activation(
                            b3d[:, r0 + 3 : r0 + 5, 1 : 1 + W],
                            p3[:, RT // 2 :], Tanh, bias=b2_sb)

                def emit_sv(g):
                    # b rows (k-major: row = k*6+m) -> per-pixel scales in
                    # (k-parity*64+p, kpair*6+m) layout: the two transposes
                    # use column-selector matrices (even/odd k) so svp lands
                    # pre-interleaved; one contiguous copy evacuates it
                    svp = psB2.tile([C, 18], fp32, tag="svp", bufs=1,
                                    name="svp")
                    src = bsb[:, 1 + g * GP : 1 + (g + 1) * GP]
                    nc.tensor.matmul(svp[:GP, :], src, p1_sb,
                                     start=True, stop=True)
                    nc.tensor.matmul(svp[GP:, :], src, p2_sb,
                                     start=True, stop=True)
                    svS = svsp.tile([C, 18], fp32, tag="svS")
                    svS_r[g] = svS
                    nc.vector.tensor_copy(svS[:], svp[:])

                def emit_builds(g, dg, js):
                    svS = svS_r[g]
                    for j in js:
                        m, kp = divmod(j, 3)
                        sc = svS[:, kp * NB + m : kp * NB + m + 1]
                        dslice = dg[:, j * GP : (j + 1) * GP]
                        eng = _B18[j]
                        if eng == "D":
                            nc.vector.tensor_scalar(
                                dslice, mask2_sb, sc, None, MUL
                            )
                        elif eng == "P":
                            nc.gpsimd.tensor_scalar(
                                dslice, mask2_sb, sc, None, MUL
                            )
                        else:
                            nc.scalar.activation(dslice, mask2_sb, Copy,
                                                 scale=sc)

                # conv row-tile slots spread at cadence ~6.5 so conv filler
                # lasts until iteration ~138 (row 23 deadline is ~141)
                _fs = {int(_r * 6.5 + 0.5): _r + 2 for _r in range(NT - 2)}
                _ws = {k + 1: v for k, v in _fs.items()}
                _h2 = {k + 2: v for k, v in _fs.items() if v in _SPLIT}
                _bs = {k + 2: v for k, v in _fs.items() if v not in _SPLIT}
                _b1 = {k + 3: v for k, v in _fs.items() if v in _SPLIT}
                _b2x = {k + 4: v for k, v in _fs.items() if v in _SPLIT}

                for r in range(2):
                    emit_arow_w(r, emit_arow_f(r))
                    emit_arow_b(r)
                # fTd chunks 0,1 were DMA'd in the startup sequence
                ft_r[0] = ft0_sb
                ft_r[1] = ft1_sb
                emit_sv(0)
                emit_sv(1)

                for i in range(NG + 4):
                    if i in _fs:
                        arow_ps = emit_arow_f(_fs[i])
                    elif i in _ws:
                        emit_arow_w(_ws[i], arow_ps)
                    elif i in _h2:
                        emit_arow_h2(_h2[i])
                    elif i in _bs:
                        emit_arow_b(_bs[i])
                    elif i in _b1:
                        emit_arow_b(_b1[i], half=0)
                    elif i in _b2x:
                        emit_arow_b(_b2x[i], half=1)
                    # acc(i-2) psum -> SBUF (Pool)
                    if 0 <= i - 2 < NG:
                        boS = bop.tile([C, NB * GP], fp16, tag="boS")
                        boS_r[i - 2] = boS
                        acc = acc_r.pop(i - 2)
                        nc.vector.tensor_copy(boS[:, : 3 * GP], acc[:, : 3 * GP])
                        nc.scalar.copy(boS[:, 3 * GP :], acc[:, 3 * GP :])
                    # orow(i-3) + store
                    if 0 <= i - 4 < NG:
                        j = i - 4
                        g8 = j % OG
                        if g8 == 0:
                            orow_bufs[j] = orp.tile(
                                [C, OG * GP], fp16, tag="orow", name="orow_buf"
                            )
                        ob = orow_bufs[j - g8]
                        nc.scalar.activation(
                            ob[:, g8 * GP : (g8 + 1) * GP], pso_r.pop(j),
                            Ident, bias=b3_sb,
                        )
                        if j >= NG - 4:
                            # tail: store each group as soon as it lands so
                            # the final DMA only waits on the last orow
                            nc.sync.dma_start(
                                out[:, 1 + j * GP : 1 + (j + 1) * GP],
                                ob[:, g8 * GP : (g8 + 1) * GP],
                            )
                            if j == NG - 1:
                                del orow_bufs[j - g8]
                        elif g8 == OG - 1:
                            t0 = j - g8
                            nc.sync.dma_start(
                                out[:, 1 + t0 * GP : 1 + t0 * GP + OG * GP],
                                ob[:, : OG * GP],
                            )
                            del orow_bufs[t0]
                        elif j == NG - 5 and g8 == 3:
                            t0 = j - g8
                            nc.sync.dma_start(
                                out[:, 1 + t0 * GP : 1 + t0 * GP + 4 * GP],
                                ob[:, : 4 * GP],
                            )
                    # stream next fTd chunk
                    if i % 8 == 0 and i // 8 + 2 < NCH:
                        cch = i // 8 + 2
                        ft = ftp.tile([C, FCH8], fp16, tag="ft")
                        ft_r[cch] = ft
                        nc.sync.dma_start(
                            ft[:], fTd[:, cch * FCH8 : (cch + 1) * FCH8]
                        )
                    # per-pixel scales two groups ahead
                    if i + 2 < NG:
                        emit_sv(i + 2)
                    # F(i), blockdiag builds(i), F evac(i)
                    if i < NG:
                        psf = psB2.tile([C, 3 * C], fp32, tag="psf", bufs=2,
                                        name="psf")
                        ft = ft_r[i // 8]
                        for kp in range(3):
                            for ci in range(2):
                                nc.tensor.matmul(
                                    psf[:, kp * C : (kp + 1) * C],
                                    bnd2_sb[:, (kp * 2 + ci) * C
                                            : (kp * 2 + ci + 1) * C],
                                    ft[:, ((i % 8) * 2 + ci) * C
                                       : ((i % 8) * 2 + ci + 1) * C],
                                    start=(ci == 0),
                                    stop=(ci == 1),
                                )
                        dg = dgp.tile([C, 18 * GP], fp16, tag="dg")
                        dg_r[i] = dg
                        emit_builds(i, dg, range(9))
                        fbS = fbp.tile([C, 3 * C], fp16, tag="fbS")
                        nc.scalar.copy(fbS[:], psf[:])
                        fbS_r[i] = fbS
                        emit_builds(i, dg, range(9, 18))
                        if i % 8 == 7 or i == NG - 1:
                            ft_r.pop(i // 8)
                    # blockdiag matmuls (i-1): acc_m = sum_kp fbS_kp^T @ bd2
                    if 0 <= i - 1 < NG:
                        acc = psB2.tile([C, NB * GP], fp32, tag="accT", bufs=2,
                                        name="acc")
                        acc_r[i - 1] = acc
                        fbS = fbS_r.pop(i - 1)
                        dg = dg_r.pop(i - 1)
                        for m in range(NB):
                            for kp in range(3):
                                j = m * 3 + kp
                                nc.tensor.matmul(
                                    acc[:, m * GP : (m + 1) * GP],
                                    fbS[:, kp * C : (kp + 1) * C],
                                    dg[:, j * GP : (j + 1) * GP],
                                    start=(kp == 0),
                                    stop=(kp == 2),
                                )
                    # coef matmuls (i-2)
                    if 0 <= i - 3 < NG:
                        _s = ((i - 3) % 2) * GP
                        pso = psoD[:, _s : _s + GP]
                        pso_r[i - 3] = pso
                        boS = boS_r.pop(i - 3)
                        for m in range(NB):
                            nc.tensor.matmul(
                                pso[:],
                                coefT_sb[:, m * C : (m + 1) * C],
                                boS[:, m * GP : (m + 1) * GP],
                                start=(m == 0),
                                stop=(m == NB - 1),
                            )

    nc.compile()
    return nc


def _get_nc():
    if "nc" not in _CACHE:
        _CACHE["nc"] = build_nc()
    return _CACHE["nc"]


def _prep_maps(feat, weight, conv1_w, conv1_b, conv2_w, conv2_b, bases_buf, coef, bias):
    feat = np.asarray(feat, np.float32)
    weight = np.asarray(weight, np.float32)
    conv1_w = np.asarray(conv1_w, np.float32)
    conv2_w = np.asarray(conv2_w, np.float32)
    bases_buf = np.asarray(bases_buf, np.float32)
    coef = np.asarray(coef, np.float32)

    n = feat.shape[0]
    featp = np.zeros((n, C, HP, WP), np.float16)
    featp[:, :, 1 : H + 1, 1 : W + 1] = feat
    wgtp = np.zeros((n, CW, HP, WP), np.float16)
    wgtp[:, :, 1 : H + 1, 1 : W + 1] = weight

    # host-prepped transposed feature chunks: the 3 disjoint di input
    # windows (66 px each) packed into 2 chunks of 128 rows per group
    # fTd[q, (g*2+ci)*C + c] = fe[c, FOFF + 1 + g*GP + RELS[ci][q]]
    rels1 = list(range(-99, -33)) + list(range(-1, 61))
    rels2 = list(range(61, 65)) + list(range(97, 163))
    rels2 = rels2 + [163] * (C - len(rels2))
    RELS = [np.array(rels1), np.array(rels2)]
    fe = np.zeros((n, C, FEXT), np.float16)
    fe[:, :, FOFF : FOFF + NPAD] = featp.reshape(n, C, NPAD)
    fTdh = np.empty((n, 2 * NG, C, C), np.float16)
    for g in range(NG):
        for ci in range(2):
            idx = FOFF + 1 + g * GP + RELS[ci]
            fTdh[:, g * 2 + ci] = fe[:, :, idx].transpose(0, 2, 1)
    fTdh = np.ascontiguousarray(
        fTdh.transpose(0, 2, 1, 3).reshape(n, C, 2 * NG * C)
    )

    w1f = np.ascontiguousarray(
        conv1_w[:, :C].transpose(1, 2, 3, 0).reshape(C, L * C)
    ).astype(np.float16)
    w1w = np.ascontiguousarray(
        conv1_w[:, C:].transpose(1, 2, 3, 0).reshape(CW, L * C)
    ).astype(np.float16)
    # conv2 weights with k-major output-channel ordering (col = k*NB + m)
    w2h = np.ascontiguousarray(conv2_w[:, :, 0, 0].T).astype(np.float16)
    w2i = np.ascontiguousarray(
        w2h.reshape(C, NB, TEM).transpose(0, 2, 1).reshape(C, NBT)
    )
    # packed band blocks: bnd2[q, (kp*2+ci)*C + ki*GP + p] accumulates
    # bases_buf[2*kp+ki, di*3+dj] where chunk ci row q holds input offset
    # rel = p + (di-1)*WP + dj - 1
    pos = []
    for rels in RELS:
        d = {}
        for q, r in enumerate(rels.tolist()):
            if r not in d:
                d[r] = q
        pos.append(d)
    bndh = np.zeros((C, 3, 2, 2, GP), np.float32)
    for kp in range(3):
        for ki in range(2):
            k = 2 * kp + ki
            for di in range(3):
                for dj in range(3):
                    for p in range(GP):
                        rel = p + (di - 1) * WP + dj - 1
                        ci = 0 if rel in pos[0] else 1
                        bndh[pos[ci][rel], kp, ci, ki, p] += \
                            bases_buf[k, di * 3 + dj]
    bndh = bndh.reshape(C, 6 * C).astype(np.float16)
    # mask2: ones at (p,p) and (GP+p, p)
    mask2 = np.zeros((C, GP), np.float16)
    for p in range(GP):
        mask2[p, p] = 1.0
        mask2[GP + p, p] = 1.0
    coefTh = np.ascontiguousarray(
        coef[:, :, 0, 0].reshape(C, C, NB).transpose(1, 2, 0).reshape(C, NB * C)
    ).astype(np.float16)
    b1h = np.asarray(conv1_b, np.float32).reshape(C, 1)
    b2h = np.asarray(conv2_b, np.float32).reshape(NB, TEM).T.reshape(NBT, 1)
    b3h = np.asarray(bias, np.float32).reshape(C, 1)

    wgtq = np.zeros((n, C, NPAD), np.float16)
    wgtq[:, :CW] = wgtp.reshape(n, CW, NPAD)
    wgtq[:, CW:, : NPAD - WP] = wgtp.reshape(n, CW, NPAD)[:, :, WP:]
    # paired weights: rows 0-63 = tap (0,j), rows 64-127 = tap (1,j)
    wgtq2 = np.zeros((n, C, NPAD), np.float16)
    wgtq2[:, :CW] = wgtp.reshape(n, CW, NPAD)
    wgtq2[:, CW:, : NPAD - 1] = wgtp.reshape(n, CW, NPAD)[:, :, 1:]
    w1wq2 = np.concatenate(
        [
            w1w.reshape(CW, 3, 3, C)[:, 2, 0],
            w1w.reshape(CW, 3, 3, C)[:, 2, 1],
        ],
        axis=0,
    ).reshape(C, C)
    w1wq = np.concatenate(
        [
            w1w.reshape(CW, 3, 3, C)[:, 0],
            w1w.reshape(CW, 3, 3, C)[:, 1],
        ],
        axis=0,
    ).reshape(C, 3 * C)

    # transpose column selectors: P1 even-k cols, P2 odd-k cols
    p1 = np.zeros((C, 18), np.float16)
    p2 = np.zeros((C, 18), np.float16)
    for kp in range(3):
        for m in range(NB):
            p1[kp * 12 + m, kp * NB + m] = 1.0
            p2[kp * 12 + NB + m, kp * NB + m] = 1.0
    pkw = np.concatenate([w1wq, w1wq2, p1, p2], axis=1)
    pk = np.concatenate([w2i, coefTh, bndh, mask2], axis=1)
    pb = np.zeros((C, 3), np.float32)
    pb[:, 0:1] = b1h
    pb[:, 1:2] = b3h
    pb[:NBT, 2:3] = b2h
    shared = {"w1f": w1f, "pkw": pkw, "pk": pk,
              "w1w": np.ascontiguousarray(w1w[:, 8 * C:]), "pb": pb}
    return [
        {
            "featp": featp[i].reshape(C, NPAD).astype(np.float16),
            "wgtq": wgtq[i], "wgtq2": wgtq2[i],
            "fTd": fTdh[i],
            **shared,
        }
        for i in range(n)
    ]


def kernel(feat, weight, conv1_w, conv1_b, conv2_w, conv2_b, bases_buf, coef, bias,
           **run_kwargs):
    in_maps = _prep_maps(
        feat, weight, conv1_w, conv1_b, conv2_w, conv2_b, bases_buf, coef, bias
    )
    res = run_bass_kernel_spmd(
        _get_nc(), in_maps, core_ids=list(range(len(in_maps))), **run_kwargs
    )
    outp = np.stack([r["out"] for r in res.results], 0).astype(np.float32)
    outp = outp[:, :, :NPAD].reshape(-1, C, HP, WP)[:, :, 1 : H + 1, 1 : W + 1]
    _CACHE["last_results"] = res
    return np.ascontiguousarray(outp)
